# revision 43
# baseline (speedup 1.0000x reference)
# Trainium2 Bass kernel for nn_CrossAttention_6579889897579 (sparse segment-
# neighbor cross-attention + FFN block).
#
# Sharding: the S=512 queries map 1:1 onto 512 contiguous 32-frame segments of
# the T=16384 memory (action_idx encodes the segmentation; seg boundaries are
# recomputed from it on the host). Query s attends segments {s-1,s,s+1} =
# frames [32s-32, 32s+64). Sharding S across 8 cores (64 queries/core) makes
# attention block-local: core c only needs frames [2048c-64, 2048c+2112) (a
# 2176-frame slab, zero-padded at the global edges). No collectives.
#
# v4 design notes (cost-model-driven; v3 was 27437ns):
# - DMA is the serialized bottleneck (360 GB/s aggregate, one transfer at a
#   time), so the big streams (k, v, mask, weights) travel as fp8 e4m3.
# - All large matmuls use fp8 DoubleRow perf mode (two 128-deep fp8 matmuls
#   summed per instruction at 0.5 cycles/row = 4x bf16 throughput). Moving
#   operands (q, attn, relu(ctx), x1, h) are SPLIT fp8: hi = f8(x),
#   lo = f8(x - hi); hi+lo restores ~bf16 accuracy (verified: end-to-end rel
#   err 0.0149 == bf16 baseline), while each half streams at fp8 DR speed.
# - attn is stored as exp(scores - ln32) (fp8 range safety); the 1/32 scale
#   cancels exactly through the r = sum(attn) normalization.
# - FFN1 runs directly on quantized x1 (not x1-mu): h = W1@x1q + [w1sum;b1]
#   K-pair fix outer with rhs [-mu; std/8] (w1sum = quantized-W1 row sums), so
#   the x1->FFN1 chain does not wait for the mean/var statistics.
# - w2 is stored dc-major ([128, ND, NM, 128]) and DMAed in two dc-halves so
#   the final DMA only gates the last quarter of FFN2 + LN2 tail.
# - PSUM rules: (a) at most ONE matmul accumulation group open per 2KB PSUM
#   bank, (b) a group OVERWRITES its region when it closes, (c) pipeline
#   stages that overlap in time use separate tiles.
# - PE p-state ramps 0.65->1.2->2.4GHz with sustained-busy time and resets on
#   idle; warm-filler matmuls spin it up while the first k chunk streams in.
# - DMA issue order == consumption order (single HWDGE ring, 625ns per issue).
import sys

sys.path.insert(0, "/opt/trn_rl_repo")

import numpy as np
import ml_dtypes

import concourse.bass as bass
import concourse.mybir as mybir
import concourse.tile as tile
from concourse.bass_utils import run_bass_kernel_spmd
from concourse.masks import make_identity

# ---- Workaround: neuronxcc walrus rejects any instruction carrying more than
# one semaphore wait ("Too many sync wait commands"). Two pieces: (1) the Tile
# tail drain gets its waits split onto single-wait sync NOPs; (2) a post-pass
# splits multi-wait body instructions the same way.
import concourse.mybir as _mybir
from bass_rust import ScopedClock as _ScopedClock


def _drain_and_barrier(self, tick_clock, wait_clock):
    probe = self.nc.sync.nop(nofuse=True, hint="tail_wait_probe")
    wait_clock.add_sem_waits(probe.ins, _ScopedClock({None: tick_clock.global_clock}))
    waits = list(probe.ins.sync_info.on_wait)
    if waits:
        probe.ins.sync_info.on_wait = [waits[0]]
        for w in waits[1:]:
            n2 = self.nc.sync.nop(nofuse=True, hint="tail_wait_split")
            n2.ins.sync_info = _mybir.SyncInfo(on_wait=[w], on_update=[])
    self.nc.sync.drain()
    self.nc.all_engine_barrier()
    assert self.sems is not None
    popped = self.nc._tile_sem_poison_stack.pop()
    assert popped is self._sem_poison
    self.nc.clear_and_free_semaphores(list(self.sems.allocated().values()))
    self.nc.all_engine_barrier()


tile.TileContext._drain_and_barrier = _drain_and_barrier


def _split_multi_waits(nc, max_waits=1):
    uid = [0]
    for f in nc.m.functions:
        for bb in f.blocks:
            out = []
            for inst in bb.instructions:
                si = getattr(inst, "sync_info", None)
                if si is not None and si.on_wait and len(si.on_wait) > max_waits:
                    waits = list(si.on_wait)
                    for w in waits[:-max_waits]:
                        uid[0] += 1
                        nop = _mybir.InstNoOp(
                            name=f"I-waitsplit-{uid[0]}",
                            engine=inst.engine,
                            bass_nofuse=True,
                            ins=[], outs=[],
                            sync_info=_mybir.SyncInfo(on_wait=[w], on_update=[]),
                        )
                        out.append(nop)
                    inst.sync_info = _mybir.SyncInfo(
                        on_wait=waits[-max_waits:], on_update=list(si.on_update)
                    )
                out.append(inst)
            bb.instructions = out


S, T, D, DFF = 512, 16384, 512, 2048
NCORES = 8
SL = S // NCORES          # 64 queries per core
TSH = T // NCORES         # 2048 frames per core
HALO = 64
SLAB = TSH + 2 * HALO     # 2176 = 17 * 128
NTC = SLAB // 128         # 17 t-chunks
ND = D // 128             # 4 d-chunks
NM = DFF // 128           # 16 dff-chunks
F32 = mybir.dt.float32
BF16 = mybir.dt.bfloat16
FP8 = mybir.dt.float8e4
F8 = ml_dtypes.float8_e4m3fn
BF = ml_dtypes.bfloat16
AOP = mybir.AluOpType
DR = mybir.MatmulPerfMode.DoubleRow
LNA = float(np.log(256.0))   # attn = exp(s - ln256): max exp ~208 < fp8 448

# scores/AV chunk grouping over the 17 t-chunks, aligned to PSUM banks.
TGROUPS = [(0, 8), (8, 16), (16, 17)]
WARMN = 6     # warm-filler matmuls (512 cols each) before first scores


def _bcast(ap, n, axis_insert=1):
    """Insert a stride-0 dim of size n into an AP (middle broadcast)."""
    new_ap = list(ap.ap)
    new_ap.insert(axis_insert, [0, n])
    return bass.AP(tensor=ap.tensor, offset=ap.offset, ap=new_ap)


def _build_nc(apply_affine=True):
    nc = bass.Bass()
    io = {}
    io["qT2"] = nc.dram_tensor("qT2", [128, 2, ND, SL], FP8, kind="ExternalInput")
    io["kT"] = nc.dram_tensor("kT", [128, ND, SLAB], FP8, kind="ExternalInput")
    io["v_r"] = nc.dram_tensor("v_r", [NTC, 128, D], FP8, kind="ExternalInput")
    io["maskT"] = nc.dram_tensor("maskT", [128, NTC, SL], FP8, kind="ExternalInput")
    io["w1T"] = nc.dram_tensor("w1T", [128, ND, DFF], FP8, kind="ExternalInput")
    # w2 dc-major so dc-halves are contiguous 2KB-per-partition DMAs
    io["w2T"] = nc.dram_tensor("w2T", [128, ND, NM, 128], FP8, kind="ExternalInput")
    # wtT carries a 513th column per d-chunk: the Wt column sums (for the
    # early mean path  sum_d tgt2_raw = wtcol . ctxrT)
    io["wtT"] = nc.dram_tensor("wtT", [128, ND, D + 32], FP8,
                               kind="ExternalInput")
    # wext row pairs: [0:NM] = ([w1sum_fc],[8*b1eff_fc]); [NM:NM+ND] =
    # ([8*b2_dc],[0])
    io["wext"] = nc.dram_tensor("wext", [1, NM + ND, 2, 128], FP8,
                                kind="ExternalInput")
    # rows: [tgtb_rowsum ; tgtb_sq_rowsum] f32
    io["rows"] = nc.dram_tensor("rows", [1, 2, SL], F32, kind="ExternalInput")
    io["tgtbT"] = nc.dram_tensor("tgtbT", [128, ND, SL], BF16, kind="ExternalInput")
    if apply_affine:
        for nm in ("g2v", "be2v", "g3v", "be3v"):
            io[nm] = nc.dram_tensor(nm, [D], F32, kind="ExternalInput")
    out_h = nc.dram_tensor("out", [SL, D], F32, kind="ExternalOutput")
    import os as _os
    _dbg = bool(_os.environ.get("KDBG"))
    if _dbg:
        io_dbg = {
            "d_mu": nc.dram_tensor("d_mu", [1, SL], F32, kind="ExternalOutput"),
            "d_std": nc.dram_tensor("d_std", [1, SL], F32, kind="ExternalOutput"),
            "d_rrec": nc.dram_tensor("d_rrec", [1, SL], F32, kind="ExternalOutput"),
            "d_x1": nc.dram_tensor("d_x1", [128, ND, SL], F32, kind="ExternalOutput"),
            "d_h": nc.dram_tensor("d_h", [128, NM, SL], F32, kind="ExternalOutput"),
            "d_o2": nc.dram_tensor("d_o2", [128, ND, SL], F32, kind="ExternalOutput"),
            "d_xhat": nc.dram_tensor("d_xhat", [SL, D], F32, kind="ExternalOutput"),
            "d_x2": nc.dram_tensor("d_x2", [SL, D], F32, kind="ExternalOutput"),
        }

    with tile.TileContext(nc) as tc:
        with (
            tc.tile_pool(name="cst", bufs=1) as cst,
            tc.tile_pool(name="ps", bufs=1, space="PSUM") as psp,
        ):
            # ---- SBUF tiles
            qT2 = cst.tile([128, 2, ND, SL], FP8, tag="qT2")
            kT = cst.tile([128, ND, SLAB], FP8, tag="kT")
            v_sb = cst.tile([128, NTC, D], FP8, tag="v")
            maskT = cst.tile([128, NTC, SL], FP8, tag="maskT")
            wtT = cst.tile([128, ND, D + 32], FP8, tag="wt")
            w1T = cst.tile([128, ND, DFF], FP8, tag="w1")
            w2T = cst.tile([128, ND, NM, 128], FP8, tag="w2")
            wext = cst.tile([1, NM + ND, 2, 128], FP8, tag="wext")
            rows = cst.tile([1, 2, SL], F32, tag="rows")
            tgtbT = cst.tile([128, ND, SL], BF16, tag="tgtbT")

            # ---- DMA issue order == consumption order (single HWDGE ring).
            def kdma(gi):
                t0, t1 = TGROUPS[gi]
                nc.sync.dma_start(out=kT[:, :, t0 * 128:t1 * 128],
                                  in_=io["kT"][:][:, :, t0 * 128:t1 * 128])

            def vdma(gi):
                t0, t1 = TGROUPS[gi]
                nc.sync.dma_start(
                    out=v_sb[:, t0:t1, :],
                    in_=io["v_r"][t0:t1].rearrange("c p d -> p c d"))

            kdma(0)
            nc.sync.dma_start(out=qT2, in_=io["qT2"][:])
            nc.sync.dma_start(out=maskT, in_=io["maskT"][:])
            kdma(1)
            vdma(0)
            kdma(2)
            vdma(2)
            vdma(1)
            nc.sync.dma_start(out=wtT, in_=io["wtT"][:])
            nc.sync.dma_start(out=tgtbT, in_=io["tgtbT"][:])
            nc.sync.dma_start(out=rows, in_=io["rows"][:])
            nc.sync.dma_start(out=wext, in_=io["wext"][:])
            nc.sync.dma_start(out=w1T[:, :, 0:1024], in_=io["w1T"][:][:, :, 0:1024])
            nc.sync.dma_start(out=w1T[:, :, 1024:2048],
                              in_=io["w1T"][:][:, :, 1024:2048])
            # w2 dc-major: [dc0-2] then [dc3] so the final DMA gates only the
            # last quarter of FFN2 + the LN2 tail
            nc.sync.dma_start(out=w2T[:, 0:3], in_=io["w2T"][:][:, 0:3])
            nc.sync.dma_start(out=w2T[:, 3:4], in_=io["w2T"][:][:, 3:4])
            bvec = {}
            if apply_affine:
                for nm in ("g2v", "be2v", "g3v", "be3v"):
                    bvec[nm] = cst.tile([SL, D], F32, tag=nm, name=nm + "_b")
                    src = io[nm][:]
                    bcast = bass.AP(tensor=src.tensor, offset=src.offset,
                                    ap=[[0, SL]] + list(src.ap))
                    nc.gpsimd.dma_start(out=bvec[nm], in_=bcast)

            # ---- constants
            onesc = cst.tile([128, 1], FP8, tag="onesc")
            nc.vector.memset(onesc, 1.0)
            ones21 = cst.tile([128, 2, 32], FP8, tag="ones21")
            nc.vector.memset(ones21.rearrange("p a b -> p (a b)"), 1.0)
            onesb = cst.tile([128, 1], BF16, tag="onesb")
            nc.vector.memset(onesb, 1.0)
            twosb = cst.tile([128, 1], BF16, tag="twosb")
            nc.vector.memset(twosb, 2.0)
            ones_rf = cst.tile([1, 128], F32, tag="ones_rf")
            nc.vector.memset(ones_rf, 1.0)
            epsc1 = cst.tile([1, 1], F32, tag="epsc1")
            nc.vector.memset(epsc1, 1e-5)
            epsc64 = cst.tile([1, 1], F32, tag="epsc64")
            nc.vector.memset(epsc64, 1e-5 / 64.0)
            epsc = cst.tile([SL, 1], F32, tag="eps")
            nc.vector.memset(epsc, 1e-5)
            expb = cst.tile([128, 1], F32, tag="expb")
            nc.vector.memset(expb, -LNA)
            identf1 = cst.tile([1, 1], F32, tag="identf1")
            nc.vector.memset(identf1, 1.0)
            invD_row = cst.tile([1, SL], F32, tag="invD_row")
            nc.vector.memset(invD_row, 1.0 / D)
            negD_row = cst.tile([1, SL], F32, tag="negD_row")
            nc.vector.memset(negD_row, -float(D))
            identf = cst.tile([128, 128], F32, tag="identf")
            make_identity(nc, identf)
            identb = cst.tile([128, 128], BF16, tag="identb")
            make_identity(nc, identb)

            # ---- PSUM tiles (8 banks; see header notes)
            ps_sc = [
                psp.tile([128, 8, SL], F32, tag="scA", name="ps_scA"),
                psp.tile([128, 8, SL], F32, tag="scB", name="ps_scB"),
                psp.tile([128, 1, SL], F32, tag="sm", name="ps_scC"),
            ]
            ps_ctxT = psp.tile([128, ND, SL], F32, tag="med", name="ps_ctxT")
            ps_h = [
                psp.tile([128, 8, SL], F32, tag="hA", name="ps_hA"),
                psp.tile([128, 8, SL], F32, tag="hB", name="ps_hB"),
            ]
            ps_t2T = psp.tile([128, ND, SL], F32, tag="t2T", name="ps_t2T")
            ps_r = psp.tile([1, SL], F32, tag="aux", name="ps_r")
            ps_rb = psp.tile([128, SL], F32, tag="aux", name="ps_rb")
            ps_stat = psp.tile([1, 192], F32, tag="sm", name="ps_stat")

            # warm fillers: 512-col zero matmuls into the hA bank (untouched
            # until FFN1-A; groups closed immediately, WAW-safe).
            wzero = cst.tile([SL, 512], BF16, tag="wzero")
            nc.vector.memset(wzero.rearrange("p f -> p f"), 0.0)
            warm_out = ps_h[0].rearrange("p c s -> p (c s)")[0:SL, :]

            def warm(n):
                for _ in range(n):
                    nc.tensor.matmul(warm_out, lhsT=wzero[:, 0:SL], rhs=wzero,
                                     start=True, stop=True,
                                     skip_group_check=True)

            warm(3)

            # ---- attention: scoresT (kT chunks stationary, q hi/lo moving,
            # DoubleRow over dc pairs) -> +mask (DVE) -> exp hi fp8 + exp bf16
            # (ACT, bias -ln32) -> lo = bf - hi (DVE) -> AV (DoubleRow over tc
            # pairs, v stationary) with attn row sums via ones DR matmuls.
            # attn is SINGLE fp8 (exp writes fp8 directly): r is computed from
            # the same quantized attn, so the softmax normalization stays
            # exact and only the weighting carries the fp8 noise (modeled
            # end-to-end rel err 0.0157 < 2e-2 gate).
            attn_hi = cst.tile([128, NTC, SL], FP8, tag="attn_hi")

            def sc_group(gi):
                t0, t1 = TGROUPS[gi]
                ps = ps_sc[gi]
                for tcn in range(t0, t1):
                    k = 0
                    for hv in range(2):
                        for dcp in range(0, ND, 2):
                            nc.tensor.matmul(
                                ps[:, tcn - t0, :],
                                lhsT=kT[:, dcp:dcp + 2, tcn * 128:(tcn + 1) * 128],
                                rhs=qT2[:, hv, dcp:dcp + 2, :],
                                start=(k == 0), stop=(k == 3),
                                perf_mode=DR,
                            )
                            k += 1
                nc.vector.tensor_add(ps[:, 0:t1 - t0, :], ps[:, 0:t1 - t0, :],
                                     maskT[:, t0:t1, :])
                nc.scalar.activation(out=attn_hi[:, t0:t1, :],
                                     in_=ps[:, 0:t1 - t0, :],
                                     func=mybir.ActivationFunctionType.Exp,
                                     bias=expb, scale=1.0)

            def av_group(gi, first, last):
                t0, t1 = TGROUPS[gi]
                if t1 - t0 == 8:
                    for tcp in range(t0, t1, 2):
                        for dc in range(ND):
                            nc.tensor.matmul(
                                ps_ctxT[:, dc, :],
                                lhsT=v_sb[:, tcp:tcp + 2,
                                          dc * 128:(dc + 1) * 128],
                                rhs=attn_hi[:, tcp:tcp + 2, :],
                                start=(first and tcp == t0 and dc == 0),
                                stop=(last and tcp == t1 - 2 and dc == ND - 1),
                                perf_mode=DR,
                                skip_group_check=True,
                            )
                        nc.tensor.matmul(
                            ps_r, lhsT=ones21[:, :, 0:1],
                            rhs=attn_hi[:, tcp:tcp + 2, :],
                            start=(first and tcp == t0),
                            stop=(last and tcp == t1 - 2),
                            perf_mode=DR,
                            skip_group_check=True,
                        )
                else:  # single chunk: plain fp8 matmuls
                    for dc in range(ND):
                        nc.tensor.matmul(
                            ps_ctxT[:, dc, :],
                            lhsT=v_sb[:, t0, dc * 128:(dc + 1) * 128],
                            rhs=attn_hi[:, t0, :],
                            start=(first and dc == 0),
                            stop=(last and dc == ND - 1),
                            skip_group_check=True,
                        )
                    nc.tensor.matmul(
                        ps_r, lhsT=onesc, rhs=attn_hi[:, t0, :],
                        start=first, stop=last,
                        skip_group_check=True,
                    )

            # av order [0, 2, 1]: v1 is the LAST v transfer, so av_group(1)
            # closes the ctx/r accumulation; the small g2 tail (mask2/exp2/
            # av2) hides under the v1 transfer.
            warm(4)
            sc_group(0)
            sc_group(1)
            av_group(0, True, False)
            sc_group(2)
            av_group(2, False, False)
            av_group(1, False, True)

            # r^-1 row first on DVE (only needs ps_r), then ctx lo
            rrec = cst.tile([1, SL], F32, tag="rrec")
            nc.vector.reciprocal(out=rrec, in_=ps_r)
            # partition broadcast of r^-1 (K=1 fp32 outer) + sbuf copy
            nc.tensor.matmul(ps_rb, lhsT=ones_rf, rhs=rrec,
                             start=True, stop=True, skip_group_check=True)
            rb_sb = cst.tile([128, SL], F32, tag="rb_sb")
            nc.vector.tensor_copy(out=rb_sb, in_=ps_rb)
            rb_bc = _bcast(rb_sb[:], ND)

            # ctx stays UN-normalized (r^-1 column scaling commutes through
            # Wt and folds into x1): ctx_hi = fp8 relu straight off psum on
            # ACT; ctx_lo = second psum read on DVE, overlapping the hi DRs
            ctx_hi = cst.tile([128, ND, SL], FP8, tag="ctx_hi")
            ctx_lo = cst.tile([128, ND, SL], FP8, tag="ctx_lo")
            nc.scalar.activation(out=ctx_hi.rearrange("p c s -> p (c s)"),
                                 in_=ps_ctxT.rearrange("p c s -> p (c s)"),
                                 func=mybir.ActivationFunctionType.Relu)
            nc.vector.scalar_tensor_tensor(
                out=ctx_lo.rearrange("p c s -> p (c s)"),
                in0=ps_ctxT.rearrange("p c s -> p (c s)"),
                scalar=0.0,
                in1=ctx_hi.rearrange("p c s -> p (c s)"),
                op0=AOP.max, op1=AOP.subtract)

            # early mean path: S1 = wtcol . ctxn = sum_d tgt2T
            k = 0
            for hv, ctx in ((0, ctx_hi), (1, ctx_lo)):
                for dcp in range(0, ND, 2):
                    nc.tensor.matmul(ps_stat[:, 0:SL],
                                     lhsT=wtT[:, dcp:dcp + 2, D:D + 1],
                                     rhs=ctx[:, dcp:dcp + 2, :],
                                     start=(k == 0), stop=(k == 3),
                                     perf_mode=DR, skip_group_check=True)
                    k += 1

            # tgt2T [dout, s] = Wt @ relu(ctx) (raw): each oc group is
            # CONTIGUOUS (only one accumulation group may be open per bank)
            for oc in range(ND):
                k = 0
                for hv, ctx in ((0, ctx_hi), (1, ctx_lo)):
                    for dcp in range(0, ND, 2):
                        nc.tensor.matmul(
                            ps_t2T[:, oc, :],
                            lhsT=wtT[:, dcp:dcp + 2, oc * 128:(oc + 1) * 128],
                            rhs=ctx[:, dcp:dcp + 2, :],
                            start=(k == 0), stop=(k == 3),
                            perf_mode=DR,
                        )
                        k += 1

            # x1 = tgt2T*r^-1 + tgtb: ps_t2T is read ONCE (psum reads of one
            # bank serialize across engines); hi fp8 from x1s on DVE, f32 on
            # Pool (parallel), lo + Square(x1) after
            x1s = cst.tile([128, ND, SL], F32, tag="x1s")
            nc.vector.tensor_mul(x1s, ps_t2T, rb_bc)
            x1hi = cst.tile([128, ND, SL], FP8, tag="x1hi")
            nc.vector.tensor_add(x1hi.rearrange("p c s -> p (c s)"),
                                 x1s.rearrange("p c s -> p (c s)"),
                                 tgtbT.rearrange("p c s -> p (c s)"))
            x1Tf = cst.tile([128, ND, SL], F32, tag="x1Tf")
            nc.gpsimd.tensor_add(x1Tf.rearrange("p c s -> p (c s)"),
                                 x1s.rearrange("p c s -> p (c s)"),
                                 tgtbT.rearrange("p c s -> p (c s)"))
            x1lo = cst.tile([128, ND, SL], FP8, tag="x1lo")
            nc.vector.tensor_sub(x1lo.rearrange("p c s -> p (c s)"),
                                 x1Tf.rearrange("p c s -> p (c s)"),
                                 x1hi.rearrange("p c s -> p (c s)"))
            x1sq = cst.tile([128, ND, SL], BF16, tag="x1sq")
            nc.scalar.activation(out=x1sq.rearrange("p c s -> p (c s)"),
                                 in_=x1Tf.rearrange("p c s -> p (c s)"),
                                 func=mybir.ActivationFunctionType.Square)

            # mu algebra on Pool: S1 is RAW (unnormalized ctx), so
            # mu = (S1*r^-1)/D + tsum/D (host pre-divides rows[0] by D)
            s1n_row = cst.tile([1, SL], F32, tag="s1n_row")
            nc.vector.tensor_mul(s1n_row, ps_stat[:, 0:SL], rrec)
            mu_row = cst.tile([1, SL], F32, tag="mu_row")
            nc.vector.scalar_tensor_tensor(out=mu_row, in0=s1n_row,
                                           scalar=1.0 / D, in1=rows[0:1, 0, :],
                                           op0=AOP.mult, op1=AOP.add)
            cmb_row = cst.tile([1, SL], F32, tag="cmb_row")
            nc.vector.scalar_tensor_tensor(out=cmb_row, in0=mu_row,
                                           scalar=-float(D), in1=mu_row,
                                           op0=AOP.mult, op1=AOP.mult)
            # mustd fp8 row pair: [-mu ; std/8] (fix outer rhs)
            mustd = cst.tile([1, 2, SL], FP8, tag="mustd")
            nc.gpsimd.tensor_scalar_mul(mustd[0:1, 0, :], mu_row, -1.0)

            # variance chain: varD = sum x1^2 - D mu^2 (stat var matmuls gate)
            for dc in range(ND):
                nc.tensor.matmul(ps_stat[:, SL:2 * SL], lhsT=onesb,
                                 rhs=x1sq[:, dc, :],
                                 start=(dc == 0), stop=(dc == ND - 1),
                                 skip_group_check=True)
            varD_row = cst.tile([1, SL], F32, tag="varD_row")
            nc.vector.tensor_add(varD_row, ps_stat[:, SL:2 * SL], cmb_row)
            # std/8 = sqrt(varD/(64 D) + eps/64) straight into the fp8 pair
            nc.scalar.activation(out=mustd[0:1, 1, :], in_=varD_row,
                                 func=mybir.ActivationFunctionType.Sqrt,
                                 bias=epsc64, scale=1.0 / (64.0 * D))
            # off-chain: f32 std / rstd for the residual scaling
            std_row = cst.tile([1, SL], F32, tag="std_row")
            nc.scalar.activation(out=std_row, in_=varD_row,
                                 func=mybir.ActivationFunctionType.Sqrt,
                                 bias=epsc1, scale=1.0 / D)
            rstd_row = cst.tile([1, SL], F32, tag="rstd_row")
            nc.vector.reciprocal(out=rstd_row, in_=std_row)

            # ---- FFN1: h = W1q @ (x1hi + x1lo) + [w1sum;8b1] (x) [-mu;std/8]
            h_hi = cst.tile([128, NM, SL], FP8, tag="h_hi")
            h_lo = cst.tile([128, NM, SL], FP8, tag="h_lo")

            def ffn1_fc(fc):
                # fix FIRST (start=True): the in-order PE stream then stalls
                # on mustd only once, at the head, instead of between every
                # fc group's matmuls
                nc.tensor.matmul(ps_h[fc // 8][:, fc % 8, :],
                                 lhsT=wext[:, fc, :, :],
                                 rhs=mustd,
                                 start=True, stop=False,
                                 perf_mode=DR)
                k = 0
                for hv, x1q in ((0, x1hi), (1, x1lo)):
                    for dcp in range(0, ND, 2):
                        nc.tensor.matmul(
                            ps_h[fc // 8][:, fc % 8, :],
                            lhsT=w1T[:, dcp:dcp + 2, fc * 128:(fc + 1) * 128],
                            rhs=x1q[:, dcp:dcp + 2, :],
                            start=False, stop=(k == 3),
                            perf_mode=DR,
                        )
                        k += 1

            # h_hi = fp8 relu straight off psum (ACT) so FFN2-hi can start
            # immediately; h_lo = second psum read (DVE), overlapping the
            # hi DRs on PE
            def h_group8(g):
                sl8 = slice(8 * g, 8 * g + 8)
                nc.scalar.activation(
                    out=h_hi[:, sl8, :],
                    in_=ps_h[g],
                    func=mybir.ActivationFunctionType.Relu)
                nc.vector.scalar_tensor_tensor(
                    out=h_lo[:, sl8, :].rearrange("p c s -> p (c s)"),
                    in0=ps_h[g].rearrange("p c s -> p (c s)"),
                    scalar=0.0,
                    in1=h_hi[:, sl8, :].rearrange("p c s -> p (c s)"),
                    op0=AOP.max, op1=AOP.subtract)

            for fc in range(8):
                ffn1_fc(fc)
            h_group8(0)
            for fc in range(8, 16):
                ffn1_fc(fc)
            h_group8(1)

            # off-chain transposes fill the PE stall while w2 streams in
            ps_x1 = psp.tile([SL, D], F32, tag="scA", name="ps_x1")
            for dc in range(ND):
                nc.tensor.transpose(ps_x1[:, dc * 128:(dc + 1) * 128],
                                    x1Tf[:, dc, :], identf)
            ps_mr = psp.tile([SL, 2], F32, tag="sm", name="ps_mr")
            nc.tensor.transpose(ps_mr[:, 0:1], mu_row, identf1)
            nc.tensor.transpose(ps_mr[:, 1:2], rstd_row, identf1)

            # ---- FFN2: one accumulation group per dc, each in its OWN psum
            # bank so all four can be open at once; the hi-operand DRs for
            # dc0-2 run before h_lo is even ready, the lo DRs + closes follow.
            # dc3 is gated by the final w2 DMA and has the shortest tail.
            ps_o2dc = [
                psp.tile([128, SL], F32, tag="med", name="ps_o2d0"),
                psp.tile([128, SL], F32, tag="t2T", name="ps_o2d1"),
                psp.tile([128, SL], F32, tag="hA", name="ps_o2d2"),
                psp.tile([128, SL], F32, tag="hB", name="ps_o2d3"),
            ]
            ps_o2 = psp.tile([SL, D], BF16, tag="aux", name="ps_o2")
            ps_o2b = psp.tile([SL, D // 2], BF16, tag="sm", name="ps_o2b")
            mustd_s = bass.AP(tensor=mustd.tensor, offset=mustd[0:1, 1, :].offset,
                              ap=[list(mustd.ap[0]), [0, 2], [1, SL]])

            def ffn2_hi(dc):
                for fcp in range(0, NM, 2):
                    nc.tensor.matmul(
                        ps_o2dc[dc],
                        lhsT=w2T[:, dc, fcp:fcp + 2, :],
                        rhs=h_hi[:, fcp:fcp + 2, :],
                        start=(fcp == 0), stop=False,
                        perf_mode=DR,
                        skip_group_check=True,
                    )

            def ffn2_lo_close(dc):
                for fcp in range(0, NM, 2):
                    nc.tensor.matmul(
                        ps_o2dc[dc],
                        lhsT=w2T[:, dc, fcp:fcp + 2, :],
                        rhs=h_lo[:, fcp:fcp + 2, :],
                        start=False, stop=False,
                        perf_mode=DR,
                        skip_group_check=True,
                    )
                nc.tensor.matmul(ps_o2dc[dc],
                                 lhsT=wext[:, NM + dc, :, :],
                                 rhs=mustd_s,
                                 start=False, stop=True,
                                 perf_mode=DR,
                                 skip_group_check=True)

            # xhat = rstd * (x1 - mu) row-major f32 (early: overlaps FFN2)
            mr_col = cst.tile([SL, 2], F32, tag="mr_col")
            nc.vector.tensor_copy(out=mr_col, in_=ps_mr)
            xhat = cst.tile([SL, D], F32, tag="xhat")
            for qc in range(ND):
                cols = slice(qc * 128, (qc + 1) * 128)
                nc.vector.tensor_scalar(out=xhat[:, cols], in0=ps_x1[:, cols],
                                        scalar1=mr_col[:, 0:1],
                                        scalar2=mr_col[:, 1:2],
                                        op0=AOP.subtract, op1=AOP.mult)
            if apply_affine:
                nc.vector.tensor_mul(xhat, xhat, bvec["g2v"])
                nc.vector.tensor_add(xhat, xhat, bvec["be2v"])

            o2Ts = cst.tile([128, ND, SL], BF16, tag="o2Ts")
            x2 = cst.tile([SL, D], F32, tag="x2")
            SD = nc.vector.BN_STATS_DIM
            st2 = cst.tile([SL, 4 * SD], F32, tag="st2")

            def trans_dc(dc):
                tgt = ps_o2[:, dc * 128:(dc + 1) * 128] if dc < 2 else \
                    ps_o2b[:, (dc - 2) * 128:(dc - 1) * 128]
                nc.tensor.transpose(tgt, o2Ts[:, dc, :], identb)

            def x2_bn_dc(dc):
                src = ps_o2[:, dc * 128:(dc + 1) * 128] if dc < 2 else \
                    ps_o2b[:, (dc - 2) * 128:(dc - 1) * 128]
                cols = slice(dc * 128, (dc + 1) * 128)
                nc.vector.scalar_tensor_tensor(out=x2[:, cols], in0=src,
                                               scalar=mr_col[:, 1:2],
                                               in1=xhat[:, cols],
                                               op0=AOP.mult, op1=AOP.add)
                nc.vector.bn_stats(out=st2[:, dc * SD:(dc + 1) * SD],
                                   in_=x2[:, cols])

            for dc in range(3):
                ffn2_hi(dc)
            for dc in range(3):
                ffn2_lo_close(dc)
            ffn2_hi(3)
            nc.vector.tensor_copy(out=o2Ts[:, 0, :], in_=ps_o2dc[0])
            nc.vector.tensor_copy(out=o2Ts[:, 1, :], in_=ps_o2dc[1])
            nc.vector.tensor_copy(out=o2Ts[:, 2, :], in_=ps_o2dc[2])
            for dc in range(3):
                trans_dc(dc)
            for dc in range(3):
                x2_bn_dc(dc)
            ffn2_lo_close(3)
            nc.vector.tensor_copy(out=o2Ts[:, 3, :], in_=ps_o2dc[3])
            trans_dc(3)
            x2_bn_dc(3)
            mv2 = cst.tile([SL, nc.vector.BN_AGGR_DIM], F32, tag="mv2")
            nc.vector.bn_aggr(out=mv2, in_=st2)
            std2 = cst.tile([SL, 1], F32, tag="std2")
            nc.scalar.activation(out=std2, in_=mv2[:, 1:2],
                                 func=mybir.ActivationFunctionType.Sqrt,
                                 bias=epsc, scale=1.0)
            rstd2 = cst.tile([SL, 1], F32, tag="rstd2")
            nc.vector.reciprocal(out=rstd2, in_=std2)
            out_sb = cst.tile([SL, D], F32, tag="out")
            nc.vector.tensor_scalar(out=out_sb, in0=x2,
                                    scalar1=mv2[:, 0:1], scalar2=rstd2,
                                    op0=AOP.subtract, op1=AOP.mult)
            if apply_affine:
                nc.vector.tensor_mul(out_sb, out_sb, bvec["g3v"])
                nc.vector.tensor_add(out_sb, out_sb, bvec["be3v"])
            nc.sync.dma_start(out=out_h[:], in_=out_sb)
            if _dbg:
                nc.sync.dma_start(out=io_dbg["d_mu"][:], in_=mu_row)
                nc.sync.dma_start(out=io_dbg["d_std"][:], in_=std_row)
                nc.sync.dma_start(out=io_dbg["d_rrec"][:], in_=rrec)
                dx1 = cst.tile([128, ND, SL], F32, tag="dx1")
                nc.vector.tensor_add(dx1.rearrange("p c s -> p (c s)"),
                                     x1hi.rearrange("p c s -> p (c s)"),
                                     x1lo.rearrange("p c s -> p (c s)"))
                nc.sync.dma_start(out=io_dbg["d_x1"][:], in_=dx1)
                dh = cst.tile([128, NM, SL], F32, tag="dh")
                nc.vector.tensor_add(dh.rearrange("p c s -> p (c s)"),
                                     h_hi.rearrange("p c s -> p (c s)"),
                                     h_lo.rearrange("p c s -> p (c s)"))
                nc.sync.dma_start(out=io_dbg["d_h"][:], in_=dh)
                do2 = cst.tile([128, ND, SL], F32, tag="do2")
                nc.vector.tensor_copy(out=do2.rearrange("p c s -> p (c s)"),
                                      in_=o2Ts.rearrange("p c s -> p (c s)"))
                nc.sync.dma_start(out=io_dbg["d_o2"][:], in_=do2)
                nc.sync.dma_start(out=io_dbg["d_xhat"][:], in_=xhat)
                nc.sync.dma_start(out=io_dbg["d_x2"][:], in_=x2)

    _split_multi_waits(nc)
    return nc


_NC_CACHE = {}


def _f8(x):
    return np.asarray(x, np.float32).astype(F8)


def _prep_inputs(tgt, memory, pos, query_pos, action_idx,
                 W_tgt2, b_tgt2, W1, b1, W2, b2, g2, be2, g3, be3):
    inv = np.float32(1.0 / np.sqrt(D))
    tgt2d = np.ascontiguousarray(tgt[:, 0, :], np.float32)        # [S, D]
    qp2d = np.ascontiguousarray(query_pos[:, 0, :], np.float32)
    mem2d = np.ascontiguousarray(memory[:, 0, :], np.float32)     # [T, D]
    pos2d = np.ascontiguousarray(pos[:, 0, :], np.float32)

    k2d = mem2d + pos2d
    k_p = np.zeros((T + 2 * HALO, D), np.float32)
    k_p[HALO:HALO + T] = k2d
    mem_p = np.zeros((T + 2 * HALO, D), np.float32)
    mem_p[HALO:HALO + T] = mem2d
    q2d = (tgt2d + qp2d) * inv                                    # [S, D]

    # segment ids from action_idx change points (mirrors the reference mask)
    ai = np.asarray(action_idx)
    change = np.concatenate([[0], (ai[1:] != ai[:-1]).astype(np.int64)])
    seg_id = np.cumsum(change)

    aff = _needs_affine(g2, be2, g3, be3)
    W1f = np.asarray(W1, np.float32)
    b1f = np.asarray(b1, np.float32)
    if aff:
        # fold g2/be2 into FFN1: h1 = (x^)@ (W1*g2).T + (b1 + W1@be2)
        W1eff = W1f * np.asarray(g2, np.float32)[None, :]
        b1eff = b1f + W1f @ np.asarray(be2, np.float32)
    else:
        W1eff, b1eff = W1f, b1f

    w1T_h = np.ascontiguousarray(
        W1eff.T.reshape(ND, 128, DFF).transpose(1, 0, 2)).astype(F8)
    # w2 dc-major: w2T[p, dc, fc, j] = W2[dc*128+j, fc*128+p]
    w2T_h = np.ascontiguousarray(
        np.asarray(W2, np.float32).T.reshape(NM, 128, ND, 128)
        .transpose(1, 2, 0, 3)).astype(F8)
    wtT_q = np.ascontiguousarray(
        np.asarray(W_tgt2, np.float32).T.reshape(ND, 128, D)
        .transpose(1, 0, 2)).astype(F8)
    # 513th column per d-chunk: Wt column sums (of the quantized weights)
    wtcol = np.asarray(wtT_q, np.float32).sum(axis=2)              # [128, ND]
    wtT_h = np.zeros((128, ND, D + 32), np.float32)
    wtT_h[:, :, 0:D] = np.asarray(wtT_q, np.float32)
    wtT_h[:, :, D] = wtcol
    wtT_h = np.ascontiguousarray(wtT_h.astype(F8))
    # wext pairs: fc rows ([w1sum_fc],[8*b1eff_fc]); dc rows ([8*b2_dc],[0])
    w1sum = np.asarray(w1T_h, np.float32).sum(axis=0).sum(axis=0)  # [DFF]
    b2f = np.asarray(b2, np.float32)
    wext_h = np.zeros((1, NM + ND, 2, 128), np.float32)
    wext_h[0, 0:NM, 0, :] = w1sum.reshape(NM, 128)
    wext_h[0, 0:NM, 1, :] = 8.0 * b1eff.reshape(NM, 128)
    wext_h[0, NM:NM + ND, 0, :] = 8.0 * b2f.reshape(ND, 128)
    wext_h = np.ascontiguousarray(wext_h).astype(F8)

    in_maps = []
    for c in range(NCORES):
        sl = slice(c * SL, (c + 1) * SL)
        qc = q2d[sl].T.reshape(ND, 128, SL).transpose(1, 0, 2)     # [128,ND,SL]
        q_hi = qc.astype(F8)
        q_lo = (qc - q_hi.astype(np.float32)).astype(F8)
        qT2c = np.ascontiguousarray(
            np.stack([np.asarray(q_hi), np.asarray(q_lo)], axis=1))
        kslab = k_p[c * TSH:c * TSH + SLAB]                       # [2176, D]
        kTc = kslab.T.reshape(ND, 128, SLAB).transpose(1, 0, 2).astype(F8)
        v_h = mem_p[c * TSH:c * TSH + SLAB].reshape(NTC, 128, D).astype(F8)

        # additive band mask in T layout [128, NTC, SL]: 0 where query j
        # (global s=64c+j) attends slab frame t, else -60; pad rows stay -60.
        mk = np.full((SL, SLAB), -60.0, np.float32)
        g0 = c * TSH - HALO
        glo, ghi = max(0, g0), min(T, g0 + SLAB)
        if ghi > glo:
            seg = seg_id[glo:ghi]
            svec = np.arange(c * SL, (c + 1) * SL)
            ok = (np.abs(seg[None, :] - svec[:, None]) <= 1)
            mk[:, glo - g0:ghi - g0][ok] = 0.0
        mkT = np.ascontiguousarray(
            mk.T.reshape(NTC, 128, SL).transpose(1, 0, 2)).astype(F8)

        tgtb = (tgt2d[sl] + np.asarray(b_tgt2, np.float32)).astype(BF)
        tgtbT = np.ascontiguousarray(
            tgtb.T.reshape(ND, 128, SL).transpose(1, 0, 2))
        tgtbf = tgtb.astype(np.float32)
        # rows[0] = tsum/D (pre-divided for the fused device-side mu stt)
        rows_h = np.stack([tgtbf.sum(axis=1) / D,
                           (tgtbf * tgtbf).sum(axis=1)]).reshape(1, 2, SL)
        rows_h = np.ascontiguousarray(rows_h, np.float32)

        im = {
            "qT2": qT2c,
            "kT": np.ascontiguousarray(kTc),
            "v_r": np.ascontiguousarray(v_h),
            "maskT": mkT,
            "w1T": w1T_h,
            "w2T": w2T_h,
            "wtT": wtT_h,
            "wext": wext_h,
            "rows": rows_h,
            "tgtbT": tgtbT,
        }
        if aff:
            im.update({
                "g2v": np.asarray(g2, np.float32),
                "be2v": np.asarray(be2, np.float32),
                "g3v": np.asarray(g3, np.float32),
                "be3v": np.asarray(be3, np.float32),
            })
        in_maps.append(im)
    return in_maps


def _needs_affine(g2, be2, g3, be3):
    return not (np.all(np.asarray(g2) == 1) and np.all(np.asarray(g3) == 1)
                and np.all(np.asarray(be2) == 0) and np.all(np.asarray(be3) == 0))


_LAST = {}


def kernel(**inputs) -> np.ndarray:
    inputs = {k: np.asarray(v) for k, v in inputs.items()}
    aff = _needs_affine(inputs["g2"], inputs["be2"], inputs["g3"], inputs["be3"])
    if aff not in _NC_CACHE:
        _NC_CACHE[aff] = _build_nc(apply_affine=aff)
    nc = _NC_CACHE[aff]
    in_maps = _prep_inputs(**inputs)
    import os
    kw = {}
    if os.environ.get("BASS_TRACE"):
        kw = dict(trace=True, tmpdir=os.environ.get("BASS_TRACE_DIR") or None)
    res = run_bass_kernel_spmd(nc, in_maps, core_ids=list(range(NCORES)), **kw)
    _LAST["res"] = res
    out = np.concatenate([res.results[c]["out"] for c in range(NCORES)], axis=0)
    return np.ascontiguousarray(out.reshape(S, 1, D).astype(np.float32))


# revision 46
# speedup vs baseline: 1.0130x; 1.0130x over previous
# Trainium2 Bass kernel for nn_CrossAttention_6579889897579 (sparse segment-
# neighbor cross-attention + FFN block).
#
# Sharding: the S=512 queries map 1:1 onto 512 contiguous 32-frame segments of
# the T=16384 memory (action_idx encodes the segmentation; seg boundaries are
# recomputed from it on the host). Query s attends segments {s-1,s,s+1} =
# frames [32s-32, 32s+64). Sharding S across 8 cores (64 queries/core) makes
# attention block-local: core c only needs frames [2048c-64, 2048c+2112) (a
# 2176-frame slab, zero-padded at the global edges). No collectives.
#
# v4 design notes (cost-model-driven; v3 was 27437ns):
# - DMA is the serialized bottleneck (360 GB/s aggregate, one transfer at a
#   time), so the big streams (k, v, mask, weights) travel as fp8 e4m3.
# - All large matmuls use fp8 DoubleRow perf mode (two 128-deep fp8 matmuls
#   summed per instruction at 0.5 cycles/row = 4x bf16 throughput). Moving
#   operands (q, attn, relu(ctx), x1, h) are SPLIT fp8: hi = f8(x),
#   lo = f8(x - hi); hi+lo restores ~bf16 accuracy (verified: end-to-end rel
#   err 0.0149 == bf16 baseline), while each half streams at fp8 DR speed.
# - attn is stored as exp(scores - ln32) (fp8 range safety); the 1/32 scale
#   cancels exactly through the r = sum(attn) normalization.
# - FFN1 runs directly on quantized x1 (not x1-mu): h = W1@x1q + [w1sum;b1]
#   K-pair fix outer with rhs [-mu; std/8] (w1sum = quantized-W1 row sums), so
#   the x1->FFN1 chain does not wait for the mean/var statistics.
# - w2 is stored dc-major ([128, ND, NM, 128]) and DMAed in two dc-halves so
#   the final DMA only gates the last quarter of FFN2 + LN2 tail.
# - PSUM rules: (a) at most ONE matmul accumulation group open per 2KB PSUM
#   bank, (b) a group OVERWRITES its region when it closes, (c) pipeline
#   stages that overlap in time use separate tiles.
# - PE p-state ramps 0.65->1.2->2.4GHz with sustained-busy time and resets on
#   idle; warm-filler matmuls spin it up while the first k chunk streams in.
# - DMA issue order == consumption order (single HWDGE ring, 625ns per issue).
import sys

sys.path.insert(0, "/opt/trn_rl_repo")

import numpy as np
import ml_dtypes

import concourse.bass as bass
import concourse.mybir as mybir
import concourse.tile as tile
from concourse.bass_utils import run_bass_kernel_spmd
from concourse.masks import make_identity

# ---- Workaround: neuronxcc walrus rejects any instruction carrying more than
# one semaphore wait ("Too many sync wait commands"). Two pieces: (1) the Tile
# tail drain gets its waits split onto single-wait sync NOPs; (2) a post-pass
# splits multi-wait body instructions the same way.
import concourse.mybir as _mybir
from bass_rust import ScopedClock as _ScopedClock


def _drain_and_barrier(self, tick_clock, wait_clock):
    probe = self.nc.sync.nop(nofuse=True, hint="tail_wait_probe")
    wait_clock.add_sem_waits(probe.ins, _ScopedClock({None: tick_clock.global_clock}))
    waits = list(probe.ins.sync_info.on_wait)
    if waits:
        probe.ins.sync_info.on_wait = [waits[0]]
        for w in waits[1:]:
            n2 = self.nc.sync.nop(nofuse=True, hint="tail_wait_split")
            n2.ins.sync_info = _mybir.SyncInfo(on_wait=[w], on_update=[])
    self.nc.sync.drain()
    self.nc.all_engine_barrier()
    assert self.sems is not None
    popped = self.nc._tile_sem_poison_stack.pop()
    assert popped is self._sem_poison
    self.nc.clear_and_free_semaphores(list(self.sems.allocated().values()))
    self.nc.all_engine_barrier()


tile.TileContext._drain_and_barrier = _drain_and_barrier


def _split_multi_waits(nc, max_waits=1):
    uid = [0]
    for f in nc.m.functions:
        for bb in f.blocks:
            out = []
            for inst in bb.instructions:
                si = getattr(inst, "sync_info", None)
                if si is not None and si.on_wait and len(si.on_wait) > max_waits:
                    waits = list(si.on_wait)
                    for w in waits[:-max_waits]:
                        uid[0] += 1
                        nop = _mybir.InstNoOp(
                            name=f"I-waitsplit-{uid[0]}",
                            engine=inst.engine,
                            bass_nofuse=True,
                            ins=[], outs=[],
                            sync_info=_mybir.SyncInfo(on_wait=[w], on_update=[]),
                        )
                        out.append(nop)
                    inst.sync_info = _mybir.SyncInfo(
                        on_wait=waits[-max_waits:], on_update=list(si.on_update)
                    )
                out.append(inst)
            bb.instructions = out


S, T, D, DFF = 512, 16384, 512, 2048
NCORES = 8
SL = S // NCORES          # 64 queries per core
TSH = T // NCORES         # 2048 frames per core
HALO = 64
SLAB = TSH + 2 * HALO     # 2176 = 17 * 128
NTC = SLAB // 128         # 17 t-chunks
ND = D // 128             # 4 d-chunks
NM = DFF // 128           # 16 dff-chunks
F32 = mybir.dt.float32
BF16 = mybir.dt.bfloat16
FP8 = mybir.dt.float8e4
F8 = ml_dtypes.float8_e4m3fn
BF = ml_dtypes.bfloat16
AOP = mybir.AluOpType
DR = mybir.MatmulPerfMode.DoubleRow
LNA = float(np.log(256.0))   # attn = exp(s - ln256): max exp ~208 < fp8 448

# scores/AV chunk grouping over the 17 t-chunks, aligned to PSUM banks.
TGROUPS = [(0, 8), (8, 16), (16, 17)]
WARMN = 6     # warm-filler matmuls (512 cols each) before first scores


def _bcast(ap, n, axis_insert=1):
    """Insert a stride-0 dim of size n into an AP (middle broadcast)."""
    new_ap = list(ap.ap)
    new_ap.insert(axis_insert, [0, n])
    return bass.AP(tensor=ap.tensor, offset=ap.offset, ap=new_ap)


def _build_nc(apply_affine=True):
    nc = bass.Bass()
    io = {}
    io["qT2"] = nc.dram_tensor("qT2", [128, 2, ND, SL], FP8, kind="ExternalInput")
    io["kT"] = nc.dram_tensor("kT", [128, ND, SLAB], FP8, kind="ExternalInput")
    io["v_r"] = nc.dram_tensor("v_r", [NTC, 128, D], FP8, kind="ExternalInput")
    io["maskT"] = nc.dram_tensor("maskT", [128, NTC, SL], FP8, kind="ExternalInput")
    io["w1T"] = nc.dram_tensor("w1T", [128, ND, DFF], FP8, kind="ExternalInput")
    # w2 dc-major so dc-halves are contiguous 2KB-per-partition DMAs
    io["w2T"] = nc.dram_tensor("w2T", [128, ND, NM, 128], FP8, kind="ExternalInput")
    # wtT carries a 513th column per d-chunk: the Wt column sums (for the
    # early mean path  sum_d tgt2_raw = wtcol . ctxrT)
    io["wtT"] = nc.dram_tensor("wtT", [128, ND, D + 32], FP8,
                               kind="ExternalInput")
    # wext row pairs: [0:NM] = ([w1sum_fc],[8*b1eff_fc]); [NM:NM+ND] =
    # ([8*b2_dc],[0])
    io["wext"] = nc.dram_tensor("wext", [1, NM + ND, 2, 128], FP8,
                                kind="ExternalInput")
    # rows: [tgtb_rowsum ; tgtb_sq_rowsum] f32
    io["rows"] = nc.dram_tensor("rows", [1, 2, SL], F32, kind="ExternalInput")
    io["tgtbT"] = nc.dram_tensor("tgtbT", [128, ND, SL], BF16, kind="ExternalInput")
    if apply_affine:
        for nm in ("g2v", "be2v", "g3v", "be3v"):
            io[nm] = nc.dram_tensor(nm, [D], F32, kind="ExternalInput")
    out_h = nc.dram_tensor("out", [SL, D], F32, kind="ExternalOutput")
    import os as _os
    _dbg = bool(_os.environ.get("KDBG"))
    if _dbg:
        io_dbg = {
            "d_mu": nc.dram_tensor("d_mu", [1, SL], F32, kind="ExternalOutput"),
            "d_std": nc.dram_tensor("d_std", [1, SL], F32, kind="ExternalOutput"),
            "d_rrec": nc.dram_tensor("d_rrec", [1, SL], F32, kind="ExternalOutput"),
            "d_x1": nc.dram_tensor("d_x1", [128, ND, SL], F32, kind="ExternalOutput"),
            "d_h": nc.dram_tensor("d_h", [128, NM, SL], F32, kind="ExternalOutput"),
            "d_o2": nc.dram_tensor("d_o2", [128, ND, SL], F32, kind="ExternalOutput"),
            "d_xhat": nc.dram_tensor("d_xhat", [SL, D], F32, kind="ExternalOutput"),
            "d_x2": nc.dram_tensor("d_x2", [SL, D], F32, kind="ExternalOutput"),
        }

    with tile.TileContext(nc) as tc:
        with (
            tc.tile_pool(name="cst", bufs=1) as cst,
            tc.tile_pool(name="ps", bufs=1, space="PSUM") as psp,
        ):
            # ---- SBUF tiles
            qT2 = cst.tile([128, 2, ND, SL], FP8, tag="qT2")
            kT = cst.tile([128, ND, SLAB], FP8, tag="kT")
            v_sb = cst.tile([128, NTC, D], FP8, tag="v")
            maskT = cst.tile([128, NTC, SL], FP8, tag="maskT")
            wtT = cst.tile([128, ND, D + 32], FP8, tag="wt")
            w1T = cst.tile([128, ND, DFF], FP8, tag="w1")
            w2T = cst.tile([128, ND, NM, 128], FP8, tag="w2")
            wext = cst.tile([1, NM + ND, 2, 128], FP8, tag="wext")
            rows = cst.tile([1, 2, SL], F32, tag="rows")
            tgtbT = cst.tile([128, ND, SL], BF16, tag="tgtbT")

            # ---- DMA issue order == consumption order (single HWDGE ring).
            def kdma(gi):
                t0, t1 = TGROUPS[gi]
                nc.sync.dma_start(out=kT[:, :, t0 * 128:t1 * 128],
                                  in_=io["kT"][:][:, :, t0 * 128:t1 * 128])

            def vdma(gi):
                t0, t1 = TGROUPS[gi]
                nc.sync.dma_start(
                    out=v_sb[:, t0:t1, :],
                    in_=io["v_r"][t0:t1].rearrange("c p d -> p c d"))

            kdma(0)
            nc.sync.dma_start(out=qT2, in_=io["qT2"][:])
            nc.sync.dma_start(out=maskT, in_=io["maskT"][:])
            kdma(1)
            vdma(0)
            kdma(2)
            vdma(2)
            vdma(1)
            nc.sync.dma_start(out=wtT, in_=io["wtT"][:])
            nc.sync.dma_start(out=tgtbT, in_=io["tgtbT"][:])
            nc.sync.dma_start(out=rows, in_=io["rows"][:])
            nc.sync.dma_start(out=wext, in_=io["wext"][:])
            nc.sync.dma_start(out=w1T[:, :, 0:1024], in_=io["w1T"][:][:, :, 0:1024])
            nc.sync.dma_start(out=w1T[:, :, 1024:2048],
                              in_=io["w1T"][:][:, :, 1024:2048])
            # w2 dc-major: [dc0-2] then [dc3] so the final DMA gates only the
            # last quarter of FFN2 + the LN2 tail
            nc.sync.dma_start(out=w2T[:, 0:3], in_=io["w2T"][:][:, 0:3])
            nc.sync.dma_start(out=w2T[:, 3:4], in_=io["w2T"][:][:, 3:4])
            bvec = {}
            if apply_affine:
                for nm in ("g2v", "be2v", "g3v", "be3v"):
                    bvec[nm] = cst.tile([SL, D], F32, tag=nm, name=nm + "_b")
                    src = io[nm][:]
                    bcast = bass.AP(tensor=src.tensor, offset=src.offset,
                                    ap=[[0, SL]] + list(src.ap))
                    nc.gpsimd.dma_start(out=bvec[nm], in_=bcast)

            # ---- constants
            onesc = cst.tile([128, 1], FP8, tag="onesc")
            nc.vector.memset(onesc, 1.0)
            ones21 = cst.tile([128, 2, 32], FP8, tag="ones21")
            nc.vector.memset(ones21.rearrange("p a b -> p (a b)"), 1.0)
            onesb = cst.tile([128, 1], BF16, tag="onesb")
            nc.vector.memset(onesb, 1.0)
            twosb = cst.tile([128, 1], BF16, tag="twosb")
            nc.vector.memset(twosb, 2.0)
            ones_rf = cst.tile([1, 128], F32, tag="ones_rf")
            nc.vector.memset(ones_rf, 1.0)
            epsc1 = cst.tile([1, 1], F32, tag="epsc1")
            nc.vector.memset(epsc1, 1e-5)
            epsc64 = cst.tile([1, 1], F32, tag="epsc64")
            nc.vector.memset(epsc64, 1e-5 / 64.0)
            epsc = cst.tile([SL, 1], F32, tag="eps")
            nc.vector.memset(epsc, 1e-5)
            expb = cst.tile([128, 1], F32, tag="expb")
            nc.vector.memset(expb, -LNA)
            identf1 = cst.tile([1, 1], F32, tag="identf1")
            nc.vector.memset(identf1, 1.0)
            invD_row = cst.tile([1, SL], F32, tag="invD_row")
            nc.vector.memset(invD_row, 1.0 / D)
            negD_row = cst.tile([1, SL], F32, tag="negD_row")
            nc.vector.memset(negD_row, -float(D))
            identf = cst.tile([128, 128], F32, tag="identf")
            make_identity(nc, identf)
            identb = cst.tile([128, 128], BF16, tag="identb")
            make_identity(nc, identb)

            # ---- PSUM tiles (8 banks; see header notes)
            ps_sc = [
                psp.tile([128, 8, SL], F32, tag="scA", name="ps_scA"),
                psp.tile([128, 8, SL], F32, tag="scB", name="ps_scB"),
                psp.tile([128, 1, SL], F32, tag="sm", name="ps_scC"),
            ]
            ps_ctxT = psp.tile([128, ND, SL], F32, tag="med", name="ps_ctxT")
            ps_h = [
                psp.tile([128, 8, SL], F32, tag="hA", name="ps_hA"),
                psp.tile([128, 8, SL], F32, tag="hB", name="ps_hB"),
            ]
            ps_t2T = psp.tile([128, ND, SL], F32, tag="t2T", name="ps_t2T")
            ps_r = psp.tile([1, SL], F32, tag="aux", name="ps_r")
            ps_rb = psp.tile([128, SL], F32, tag="aux", name="ps_rb")
            ps_stat = psp.tile([1, 192], F32, tag="sm", name="ps_stat")

            # warm fillers: 512-col zero matmuls into the hA bank (untouched
            # until FFN1-A; groups closed immediately, WAW-safe).
            wzero = cst.tile([SL, 512], BF16, tag="wzero")
            nc.vector.memset(wzero.rearrange("p f -> p f"), 0.0)
            warm_out = ps_h[0].rearrange("p c s -> p (c s)")[0:SL, :]

            def warm(n):
                for _ in range(n):
                    nc.tensor.matmul(warm_out, lhsT=wzero[:, 0:SL], rhs=wzero,
                                     start=True, stop=True,
                                     skip_group_check=True)

            warm(3)

            # ---- attention: scoresT (kT chunks stationary, q hi/lo moving,
            # DoubleRow over dc pairs) -> +mask (DVE) -> exp hi fp8 + exp bf16
            # (ACT, bias -ln32) -> lo = bf - hi (DVE) -> AV (DoubleRow over tc
            # pairs, v stationary) with attn row sums via ones DR matmuls.
            # attn is SINGLE fp8 (exp writes fp8 directly): r is computed from
            # the same quantized attn, so the softmax normalization stays
            # exact and only the weighting carries the fp8 noise (modeled
            # end-to-end rel err 0.0157 < 2e-2 gate).
            attn_hi = cst.tile([128, NTC, SL], FP8, tag="attn_hi")

            def sc_group(gi):
                t0, t1 = TGROUPS[gi]
                ps = ps_sc[gi]
                for tcn in range(t0, t1):
                    k = 0
                    for hv in range(2):
                        for dcp in range(0, ND, 2):
                            nc.tensor.matmul(
                                ps[:, tcn - t0, :],
                                lhsT=kT[:, dcp:dcp + 2, tcn * 128:(tcn + 1) * 128],
                                rhs=qT2[:, hv, dcp:dcp + 2, :],
                                start=(k == 0), stop=(k == 3),
                                perf_mode=DR,
                            )
                            k += 1
                nc.vector.tensor_add(ps[:, 0:t1 - t0, :], ps[:, 0:t1 - t0, :],
                                     maskT[:, t0:t1, :])
                nc.scalar.activation(out=attn_hi[:, t0:t1, :],
                                     in_=ps[:, 0:t1 - t0, :],
                                     func=mybir.ActivationFunctionType.Exp,
                                     bias=expb, scale=1.0)

            def av_group(gi, first, last):
                t0, t1 = TGROUPS[gi]
                if t1 - t0 == 8:
                    for tcp in range(t0, t1, 2):
                        for dc in range(ND):
                            nc.tensor.matmul(
                                ps_ctxT[:, dc, :],
                                lhsT=v_sb[:, tcp:tcp + 2,
                                          dc * 128:(dc + 1) * 128],
                                rhs=attn_hi[:, tcp:tcp + 2, :],
                                start=(first and tcp == t0 and dc == 0),
                                stop=(last and tcp == t1 - 2 and dc == ND - 1),
                                perf_mode=DR,
                                skip_group_check=True,
                            )
                        nc.tensor.matmul(
                            ps_r, lhsT=ones21[:, :, 0:1],
                            rhs=attn_hi[:, tcp:tcp + 2, :],
                            start=(first and tcp == t0),
                            stop=(last and tcp == t1 - 2),
                            perf_mode=DR,
                            skip_group_check=True,
                        )
                else:  # single chunk: plain fp8 matmuls
                    for dc in range(ND):
                        nc.tensor.matmul(
                            ps_ctxT[:, dc, :],
                            lhsT=v_sb[:, t0, dc * 128:(dc + 1) * 128],
                            rhs=attn_hi[:, t0, :],
                            start=(first and dc == 0),
                            stop=(last and dc == ND - 1),
                            skip_group_check=True,
                        )
                    nc.tensor.matmul(
                        ps_r, lhsT=onesc, rhs=attn_hi[:, t0, :],
                        start=first, stop=last,
                        skip_group_check=True,
                    )

            # av order [0, 2, 1]: v1 is the LAST v transfer, so av_group(1)
            # closes the ctx/r accumulation; the small g2 tail (mask2/exp2/
            # av2) hides under the v1 transfer.
            warm(4)
            sc_group(0)
            sc_group(1)
            sc_group(2)
            av_group(0, True, False)
            av_group(2, False, False)
            av_group(1, False, True)

            # r^-1 row first on DVE (only needs ps_r), then ctx lo
            rrec = cst.tile([1, SL], F32, tag="rrec")
            nc.vector.reciprocal(out=rrec, in_=ps_r)
            # partition broadcast of r^-1 (K=1 fp32 outer) + sbuf copy
            nc.tensor.matmul(ps_rb, lhsT=ones_rf, rhs=rrec,
                             start=True, stop=True, skip_group_check=True)
            rb_sb = cst.tile([128, SL], F32, tag="rb_sb")
            nc.vector.tensor_copy(out=rb_sb, in_=ps_rb)
            rb_bc = _bcast(rb_sb[:], ND)

            # ctx stays UN-normalized (r^-1 column scaling commutes through
            # Wt and folds into x1): ctx_hi = fp8 relu straight off psum on
            # ACT; ctx_lo = second psum read on DVE, overlapping the hi DRs
            ctx_hi = cst.tile([128, ND, SL], FP8, tag="ctx_hi")
            ctx_lo = cst.tile([128, ND, SL], FP8, tag="ctx_lo")
            nc.vector.tensor_scalar_max(
                ctx_hi.rearrange("p c s -> p (c s)"),
                ps_ctxT.rearrange("p c s -> p (c s)"), 0.0)
            nc.vector.scalar_tensor_tensor(
                out=ctx_lo.rearrange("p c s -> p (c s)"),
                in0=ps_ctxT.rearrange("p c s -> p (c s)"),
                scalar=0.0,
                in1=ctx_hi.rearrange("p c s -> p (c s)"),
                op0=AOP.max, op1=AOP.subtract)

            # early mean path: S1 = wtcol . ctxn = sum_d tgt2T
            k = 0
            for hv, ctx in ((0, ctx_hi), (1, ctx_lo)):
                for dcp in range(0, ND, 2):
                    nc.tensor.matmul(ps_stat[:, 0:SL],
                                     lhsT=wtT[:, dcp:dcp + 2, D:D + 1],
                                     rhs=ctx[:, dcp:dcp + 2, :],
                                     start=(k == 0), stop=(k == 3),
                                     perf_mode=DR, skip_group_check=True)
                    k += 1

            # tgt2T [dout, s] = Wt @ relu(ctx) (raw): each oc group is
            # CONTIGUOUS (only one accumulation group may be open per bank)
            for oc in range(ND):
                k = 0
                for hv, ctx in ((0, ctx_hi), (1, ctx_lo)):
                    for dcp in range(0, ND, 2):
                        nc.tensor.matmul(
                            ps_t2T[:, oc, :],
                            lhsT=wtT[:, dcp:dcp + 2, oc * 128:(oc + 1) * 128],
                            rhs=ctx[:, dcp:dcp + 2, :],
                            start=(k == 0), stop=(k == 3),
                            perf_mode=DR,
                        )
                        k += 1

            # x1 = tgt2T*r^-1 + tgtb: ps_t2T is read ONCE (psum reads of one
            # bank serialize across engines); hi fp8 from x1s on DVE, f32 on
            # Pool (parallel), lo + Square(x1) after
            x1s = cst.tile([128, ND, SL], F32, tag="x1s")
            nc.vector.tensor_mul(x1s, ps_t2T, rb_bc)
            x1hi = cst.tile([128, ND, SL], FP8, tag="x1hi")
            nc.vector.tensor_add(x1hi.rearrange("p c s -> p (c s)"),
                                 x1s.rearrange("p c s -> p (c s)"),
                                 tgtbT.rearrange("p c s -> p (c s)"))
            x1Tf = cst.tile([128, ND, SL], F32, tag="x1Tf")
            nc.gpsimd.tensor_add(x1Tf.rearrange("p c s -> p (c s)"),
                                 x1s.rearrange("p c s -> p (c s)"),
                                 tgtbT.rearrange("p c s -> p (c s)"))
            x1lo = cst.tile([128, ND, SL], FP8, tag="x1lo")
            nc.vector.tensor_sub(x1lo.rearrange("p c s -> p (c s)"),
                                 x1Tf.rearrange("p c s -> p (c s)"),
                                 x1hi.rearrange("p c s -> p (c s)"))
            x1sq = cst.tile([128, ND, SL], BF16, tag="x1sq")
            nc.scalar.activation(out=x1sq.rearrange("p c s -> p (c s)"),
                                 in_=x1s.rearrange("p c s -> p (c s)"),
                                 func=mybir.ActivationFunctionType.Square)
            x1cr = cst.tile([128, ND, SL], BF16, tag="x1cr")
            nc.vector.tensor_mul(x1cr.rearrange("p c s -> p (c s)"),
                                 x1s.rearrange("p c s -> p (c s)"),
                                 tgtbT.rearrange("p c s -> p (c s)"))

            # mu algebra on Pool: S1 is RAW (unnormalized ctx), so
            # mu = (S1*r^-1)/D + tsum/D (host pre-divides rows[0] by D)
            s1n_row = cst.tile([1, SL], F32, tag="s1n_row")
            nc.vector.tensor_mul(s1n_row, ps_stat[:, 0:SL], rrec)
            mu_row = cst.tile([1, SL], F32, tag="mu_row")
            nc.vector.scalar_tensor_tensor(out=mu_row, in0=s1n_row,
                                           scalar=1.0 / D, in1=rows[0:1, 0, :],
                                           op0=AOP.mult, op1=AOP.add)
            musqD = cst.tile([1, SL], F32, tag="musqD")
            nc.vector.scalar_tensor_tensor(out=musqD, in0=mu_row,
                                           scalar=-float(D), in1=mu_row,
                                           op0=AOP.mult, op1=AOP.mult)
            cmb_row = cst.tile([1, SL], F32, tag="cmb_row")
            nc.vector.tensor_add(cmb_row, musqD, rows[0:1, 1, :])
            # mustd fp8 row pair: [-mu ; std/8] (fix outer rhs)
            mustd = cst.tile([1, 2, SL], FP8, tag="mustd")
            nc.gpsimd.tensor_scalar_mul(mustd[0:1, 0, :], mu_row, -1.0)

            # variance chain: varD = sum x1^2 - D mu^2 (stat var matmuls gate)
            for dc in range(ND):
                nc.tensor.matmul(ps_stat[:, SL:2 * SL], lhsT=onesb,
                                 rhs=x1sq[:, dc, :],
                                 start=(dc == 0), stop=False,
                                 skip_group_check=True)
            for dc in range(ND):
                nc.tensor.matmul(ps_stat[:, SL:2 * SL], lhsT=twosb,
                                 rhs=x1cr[:, dc, :],
                                 start=False, stop=(dc == ND - 1),
                                 skip_group_check=True)
            varD_row = cst.tile([1, SL], F32, tag="varD_row")
            nc.vector.tensor_add(varD_row, ps_stat[:, SL:2 * SL], cmb_row)
            # std/8 = sqrt(varD/(64 D) + eps/64) straight into the fp8 pair
            nc.scalar.activation(out=mustd[0:1, 1, :], in_=varD_row,
                                 func=mybir.ActivationFunctionType.Sqrt,
                                 bias=epsc64, scale=1.0 / (64.0 * D))
            # off-chain: f32 std / rstd for the residual scaling
            std_row = cst.tile([1, SL], F32, tag="std_row")
            nc.scalar.activation(out=std_row, in_=varD_row,
                                 func=mybir.ActivationFunctionType.Sqrt,
                                 bias=epsc1, scale=1.0 / D)
            rstd_row = cst.tile([1, SL], F32, tag="rstd_row")
            nc.vector.reciprocal(out=rstd_row, in_=std_row)

            # ---- FFN1: h = W1q @ (x1hi + x1lo) + [w1sum;8b1] (x) [-mu;std/8]
            h_hi = cst.tile([128, NM, SL], FP8, tag="h_hi")
            h_lo = cst.tile([128, NM, SL], FP8, tag="h_lo")

            def ffn1_fc(fc):
                # fix FIRST (start=True): the in-order PE stream then stalls
                # on mustd only once, at the head, instead of between every
                # fc group's matmuls
                nc.tensor.matmul(ps_h[fc // 8][:, fc % 8, :],
                                 lhsT=wext[:, fc, :, :],
                                 rhs=mustd,
                                 start=True, stop=False,
                                 perf_mode=DR)
                k = 0
                for hv, x1q in ((0, x1hi), (1, x1lo)):
                    for dcp in range(0, ND, 2):
                        nc.tensor.matmul(
                            ps_h[fc // 8][:, fc % 8, :],
                            lhsT=w1T[:, dcp:dcp + 2, fc * 128:(fc + 1) * 128],
                            rhs=x1q[:, dcp:dcp + 2, :],
                            start=False, stop=(k == 3),
                            perf_mode=DR,
                        )
                        k += 1

            # h_hi = fp8 relu straight off psum (ACT) so FFN2-hi can start
            # immediately; h_lo = second psum read (DVE), overlapping the
            # hi DRs on PE
            def h_group8(g):
                sl8 = slice(8 * g, 8 * g + 8)
                nc.scalar.activation(
                    out=h_hi[:, sl8, :],
                    in_=ps_h[g],
                    func=mybir.ActivationFunctionType.Relu)
                nc.vector.scalar_tensor_tensor(
                    out=h_lo[:, sl8, :].rearrange("p c s -> p (c s)"),
                    in0=ps_h[g].rearrange("p c s -> p (c s)"),
                    scalar=0.0,
                    in1=h_hi[:, sl8, :].rearrange("p c s -> p (c s)"),
                    op0=AOP.max, op1=AOP.subtract)

            for fc in range(8):
                ffn1_fc(fc)
            h_group8(0)
            for fc in range(8, 16):
                ffn1_fc(fc)
            h_group8(1)

            # off-chain transposes fill the PE stall while w2 streams in
            ps_x1 = psp.tile([SL, D], F32, tag="scA", name="ps_x1")
            for dc in range(ND):
                nc.tensor.transpose(ps_x1[:, dc * 128:(dc + 1) * 128],
                                    x1Tf[:, dc, :], identf)
            ps_mr = psp.tile([SL, 2], F32, tag="sm", name="ps_mr")
            nc.tensor.transpose(ps_mr[:, 0:1], mu_row, identf1)
            nc.tensor.transpose(ps_mr[:, 1:2], rstd_row, identf1)

            # ---- FFN2: one accumulation group per dc, each in its OWN psum
            # bank so all four can be open at once; the hi-operand DRs for
            # dc0-2 run before h_lo is even ready, the lo DRs + closes follow.
            # dc3 is gated by the final w2 DMA and has the shortest tail.
            ps_o2dc = [
                psp.tile([128, SL], F32, tag="med", name="ps_o2d0"),
                psp.tile([128, SL], F32, tag="t2T", name="ps_o2d1"),
                psp.tile([128, SL], F32, tag="hA", name="ps_o2d2"),
                psp.tile([128, SL], F32, tag="hB", name="ps_o2d3"),
            ]
            ps_o2 = psp.tile([SL, D], BF16, tag="aux", name="ps_o2")
            ps_o2b = psp.tile([SL, D // 2], BF16, tag="sm", name="ps_o2b")
            mustd_s = bass.AP(tensor=mustd.tensor, offset=mustd[0:1, 1, :].offset,
                              ap=[list(mustd.ap[0]), [0, 2], [1, SL]])

            def ffn2_hi(dc):
                for fcp in range(0, NM, 2):
                    nc.tensor.matmul(
                        ps_o2dc[dc],
                        lhsT=w2T[:, dc, fcp:fcp + 2, :],
                        rhs=h_hi[:, fcp:fcp + 2, :],
                        start=(fcp == 0), stop=False,
                        perf_mode=DR,
                        skip_group_check=True,
                    )

            def ffn2_lo_close(dc):
                for fcp in range(0, NM, 2):
                    nc.tensor.matmul(
                        ps_o2dc[dc],
                        lhsT=w2T[:, dc, fcp:fcp + 2, :],
                        rhs=h_lo[:, fcp:fcp + 2, :],
                        start=False, stop=False,
                        perf_mode=DR,
                        skip_group_check=True,
                    )
                nc.tensor.matmul(ps_o2dc[dc],
                                 lhsT=wext[:, NM + dc, :, :],
                                 rhs=mustd_s,
                                 start=False, stop=True,
                                 perf_mode=DR,
                                 skip_group_check=True)

            # xhat = rstd * (x1 - mu) row-major f32 (early: overlaps FFN2)
            mr_col = cst.tile([SL, 2], F32, tag="mr_col")
            nc.vector.tensor_copy(out=mr_col, in_=ps_mr)
            xhat = cst.tile([SL, D], F32, tag="xhat")
            for qc in range(ND):
                cols = slice(qc * 128, (qc + 1) * 128)
                nc.vector.tensor_scalar(out=xhat[:, cols], in0=ps_x1[:, cols],
                                        scalar1=mr_col[:, 0:1],
                                        scalar2=mr_col[:, 1:2],
                                        op0=AOP.subtract, op1=AOP.mult)
            if apply_affine:
                nc.vector.tensor_mul(xhat, xhat, bvec["g2v"])
                nc.vector.tensor_add(xhat, xhat, bvec["be2v"])

            o2Ts = cst.tile([128, ND, SL], BF16, tag="o2Ts")
            x2 = cst.tile([SL, D], F32, tag="x2")
            SD = nc.vector.BN_STATS_DIM
            st2 = cst.tile([SL, 4 * SD], F32, tag="st2")

            def trans_dc(dc):
                tgt = ps_o2[:, dc * 128:(dc + 1) * 128] if dc < 2 else \
                    ps_o2b[:, (dc - 2) * 128:(dc - 1) * 128]
                nc.tensor.transpose(tgt, o2Ts[:, dc, :], identb)

            def x2_bn_dc(dc):
                src = ps_o2[:, dc * 128:(dc + 1) * 128] if dc < 2 else \
                    ps_o2b[:, (dc - 2) * 128:(dc - 1) * 128]
                cols = slice(dc * 128, (dc + 1) * 128)
                nc.vector.scalar_tensor_tensor(out=x2[:, cols], in0=src,
                                               scalar=mr_col[:, 1:2],
                                               in1=xhat[:, cols],
                                               op0=AOP.mult, op1=AOP.add)
                nc.vector.bn_stats(out=st2[:, dc * SD:(dc + 1) * SD],
                                   in_=x2[:, cols])

            for dc in range(3):
                ffn2_hi(dc)
            for dc in range(3):
                ffn2_lo_close(dc)
            ffn2_hi(3)
            nc.vector.tensor_copy(out=o2Ts[:, 0, :], in_=ps_o2dc[0])
            nc.vector.tensor_copy(out=o2Ts[:, 1, :], in_=ps_o2dc[1])
            nc.vector.tensor_copy(out=o2Ts[:, 2, :], in_=ps_o2dc[2])
            for dc in range(3):
                trans_dc(dc)
            for dc in range(3):
                x2_bn_dc(dc)
            ffn2_lo_close(3)
            nc.vector.tensor_copy(out=o2Ts[:, 3, :], in_=ps_o2dc[3])
            trans_dc(3)
            x2_bn_dc(3)
            mv2 = cst.tile([SL, nc.vector.BN_AGGR_DIM], F32, tag="mv2")
            nc.vector.bn_aggr(out=mv2, in_=st2)
            std2 = cst.tile([SL, 1], F32, tag="std2")
            nc.scalar.activation(out=std2, in_=mv2[:, 1:2],
                                 func=mybir.ActivationFunctionType.Sqrt,
                                 bias=epsc, scale=1.0)
            rstd2 = cst.tile([SL, 1], F32, tag="rstd2")
            nc.vector.reciprocal(out=rstd2, in_=std2)
            out_sb = cst.tile([SL, D], F32, tag="out")
            nc.vector.tensor_scalar(out=out_sb, in0=x2,
                                    scalar1=mv2[:, 0:1], scalar2=rstd2,
                                    op0=AOP.subtract, op1=AOP.mult)
            if apply_affine:
                nc.vector.tensor_mul(out_sb, out_sb, bvec["g3v"])
                nc.vector.tensor_add(out_sb, out_sb, bvec["be3v"])
            nc.sync.dma_start(out=out_h[:], in_=out_sb)
            if _dbg:
                nc.sync.dma_start(out=io_dbg["d_mu"][:], in_=mu_row)
                nc.sync.dma_start(out=io_dbg["d_std"][:], in_=std_row)
                nc.sync.dma_start(out=io_dbg["d_rrec"][:], in_=rrec)
                dx1 = cst.tile([128, ND, SL], F32, tag="dx1")
                nc.vector.tensor_add(dx1.rearrange("p c s -> p (c s)"),
                                     x1hi.rearrange("p c s -> p (c s)"),
                                     x1lo.rearrange("p c s -> p (c s)"))
                nc.sync.dma_start(out=io_dbg["d_x1"][:], in_=dx1)
                dh = cst.tile([128, NM, SL], F32, tag="dh")
                nc.vector.tensor_add(dh.rearrange("p c s -> p (c s)"),
                                     h_hi.rearrange("p c s -> p (c s)"),
                                     h_lo.rearrange("p c s -> p (c s)"))
                nc.sync.dma_start(out=io_dbg["d_h"][:], in_=dh)
                do2 = cst.tile([128, ND, SL], F32, tag="do2")
                nc.vector.tensor_copy(out=do2.rearrange("p c s -> p (c s)"),
                                      in_=o2Ts.rearrange("p c s -> p (c s)"))
                nc.sync.dma_start(out=io_dbg["d_o2"][:], in_=do2)
                nc.sync.dma_start(out=io_dbg["d_xhat"][:], in_=xhat)
                nc.sync.dma_start(out=io_dbg["d_x2"][:], in_=x2)

    _split_multi_waits(nc)
    return nc


_NC_CACHE = {}


def _f8(x):
    return np.asarray(x, np.float32).astype(F8)


def _prep_inputs(tgt, memory, pos, query_pos, action_idx,
                 W_tgt2, b_tgt2, W1, b1, W2, b2, g2, be2, g3, be3):
    inv = np.float32(1.0 / np.sqrt(D))
    tgt2d = np.ascontiguousarray(tgt[:, 0, :], np.float32)        # [S, D]
    qp2d = np.ascontiguousarray(query_pos[:, 0, :], np.float32)
    mem2d = np.ascontiguousarray(memory[:, 0, :], np.float32)     # [T, D]
    pos2d = np.ascontiguousarray(pos[:, 0, :], np.float32)

    k2d = mem2d + pos2d
    k_p = np.zeros((T + 2 * HALO, D), np.float32)
    k_p[HALO:HALO + T] = k2d
    mem_p = np.zeros((T + 2 * HALO, D), np.float32)
    mem_p[HALO:HALO + T] = mem2d
    q2d = (tgt2d + qp2d) * inv                                    # [S, D]

    # segment ids from action_idx change points (mirrors the reference mask)
    ai = np.asarray(action_idx)
    change = np.concatenate([[0], (ai[1:] != ai[:-1]).astype(np.int64)])
    seg_id = np.cumsum(change)

    aff = _needs_affine(g2, be2, g3, be3)
    W1f = np.asarray(W1, np.float32)
    b1f = np.asarray(b1, np.float32)
    if aff:
        # fold g2/be2 into FFN1: h1 = (x^)@ (W1*g2).T + (b1 + W1@be2)
        W1eff = W1f * np.asarray(g2, np.float32)[None, :]
        b1eff = b1f + W1f @ np.asarray(be2, np.float32)
    else:
        W1eff, b1eff = W1f, b1f

    w1T_h = np.ascontiguousarray(
        W1eff.T.reshape(ND, 128, DFF).transpose(1, 0, 2)).astype(F8)
    # w2 dc-major: w2T[p, dc, fc, j] = W2[dc*128+j, fc*128+p]
    w2T_h = np.ascontiguousarray(
        np.asarray(W2, np.float32).T.reshape(NM, 128, ND, 128)
        .transpose(1, 2, 0, 3)).astype(F8)
    wtT_q = np.ascontiguousarray(
        np.asarray(W_tgt2, np.float32).T.reshape(ND, 128, D)
        .transpose(1, 0, 2)).astype(F8)
    # 513th column per d-chunk: Wt column sums (of the quantized weights)
    wtcol = np.asarray(wtT_q, np.float32).sum(axis=2)              # [128, ND]
    wtT_h = np.zeros((128, ND, D + 32), np.float32)
    wtT_h[:, :, 0:D] = np.asarray(wtT_q, np.float32)
    wtT_h[:, :, D] = wtcol
    wtT_h = np.ascontiguousarray(wtT_h.astype(F8))
    # wext pairs: fc rows ([w1sum_fc],[8*b1eff_fc]); dc rows ([8*b2_dc],[0])
    w1sum = np.asarray(w1T_h, np.float32).sum(axis=0).sum(axis=0)  # [DFF]
    b2f = np.asarray(b2, np.float32)
    wext_h = np.zeros((1, NM + ND, 2, 128), np.float32)
    wext_h[0, 0:NM, 0, :] = w1sum.reshape(NM, 128)
    wext_h[0, 0:NM, 1, :] = 8.0 * b1eff.reshape(NM, 128)
    wext_h[0, NM:NM + ND, 0, :] = 8.0 * b2f.reshape(ND, 128)
    wext_h = np.ascontiguousarray(wext_h).astype(F8)

    in_maps = []
    for c in range(NCORES):
        sl = slice(c * SL, (c + 1) * SL)
        qc = q2d[sl].T.reshape(ND, 128, SL).transpose(1, 0, 2)     # [128,ND,SL]
        q_hi = qc.astype(F8)
        q_lo = (qc - q_hi.astype(np.float32)).astype(F8)
        qT2c = np.ascontiguousarray(
            np.stack([np.asarray(q_hi), np.asarray(q_lo)], axis=1))
        kslab = k_p[c * TSH:c * TSH + SLAB]                       # [2176, D]
        kTc = kslab.T.reshape(ND, 128, SLAB).transpose(1, 0, 2).astype(F8)
        v_h = mem_p[c * TSH:c * TSH + SLAB].reshape(NTC, 128, D).astype(F8)

        # additive band mask in T layout [128, NTC, SL]: 0 where query j
        # (global s=64c+j) attends slab frame t, else -60; pad rows stay -60.
        mk = np.full((SL, SLAB), -60.0, np.float32)
        g0 = c * TSH - HALO
        glo, ghi = max(0, g0), min(T, g0 + SLAB)
        if ghi > glo:
            seg = seg_id[glo:ghi]
            svec = np.arange(c * SL, (c + 1) * SL)
            ok = (np.abs(seg[None, :] - svec[:, None]) <= 1)
            mk[:, glo - g0:ghi - g0][ok] = 0.0
        mkT = np.ascontiguousarray(
            mk.T.reshape(NTC, 128, SL).transpose(1, 0, 2)).astype(F8)

        tgtb = (tgt2d[sl] + np.asarray(b_tgt2, np.float32)).astype(BF)
        tgtbT = np.ascontiguousarray(
            tgtb.T.reshape(ND, 128, SL).transpose(1, 0, 2))
        tgtbf = tgtb.astype(np.float32)
        # rows[0] = tsum/D (pre-divided for the fused device-side mu stt)
        rows_h = np.stack([tgtbf.sum(axis=1) / D,
                           (tgtbf * tgtbf).sum(axis=1)]).reshape(1, 2, SL)
        rows_h = np.ascontiguousarray(rows_h, np.float32)

        im = {
            "qT2": qT2c,
            "kT": np.ascontiguousarray(kTc),
            "v_r": np.ascontiguousarray(v_h),
            "maskT": mkT,
            "w1T": w1T_h,
            "w2T": w2T_h,
            "wtT": wtT_h,
            "wext": wext_h,
            "rows": rows_h,
            "tgtbT": tgtbT,
        }
        if aff:
            im.update({
                "g2v": np.asarray(g2, np.float32),
                "be2v": np.asarray(be2, np.float32),
                "g3v": np.asarray(g3, np.float32),
                "be3v": np.asarray(be3, np.float32),
            })
        in_maps.append(im)
    return in_maps


def _needs_affine(g2, be2, g3, be3):
    return not (np.all(np.asarray(g2) == 1) and np.all(np.asarray(g3) == 1)
                and np.all(np.asarray(be2) == 0) and np.all(np.asarray(be3) == 0))


_LAST = {}


def kernel(**inputs) -> np.ndarray:
    inputs = {k: np.asarray(v) for k, v in inputs.items()}
    aff = _needs_affine(inputs["g2"], inputs["be2"], inputs["g3"], inputs["be3"])
    if aff not in _NC_CACHE:
        _NC_CACHE[aff] = _build_nc(apply_affine=aff)
    nc = _NC_CACHE[aff]
    in_maps = _prep_inputs(**inputs)
    import os
    kw = {}
    if os.environ.get("BASS_TRACE"):
        kw = dict(trace=True, tmpdir=os.environ.get("BASS_TRACE_DIR") or None)
    res = run_bass_kernel_spmd(nc, in_maps, core_ids=list(range(NCORES)), **kw)
    _LAST["res"] = res
    out = np.concatenate([res.results[c]["out"] for c in range(NCORES)], axis=0)
    return np.ascontiguousarray(out.reshape(S, 1, D).astype(np.float32))


# revision 48
# speedup vs baseline: 1.0356x; 1.0223x over previous
# Trainium2 Bass kernel for nn_CrossAttention_6579889897579 (sparse segment-
# neighbor cross-attention + FFN block).
#
# Sharding: the S=512 queries map 1:1 onto 512 contiguous 32-frame segments of
# the T=16384 memory (action_idx encodes the segmentation; seg boundaries are
# recomputed from it on the host). Query s attends segments {s-1,s,s+1} =
# frames [32s-32, 32s+64). Sharding S across 8 cores (64 queries/core) makes
# attention block-local: core c only needs frames [2048c-64, 2048c+2112) (a
# 2176-frame slab, zero-padded at the global edges). No collectives.
#
# v4 design notes (cost-model-driven; v3 was 27437ns):
# - DMA is the serialized bottleneck (360 GB/s aggregate, one transfer at a
#   time), so the big streams (k, v, mask, weights) travel as fp8 e4m3.
# - All large matmuls use fp8 DoubleRow perf mode (two 128-deep fp8 matmuls
#   summed per instruction at 0.5 cycles/row = 4x bf16 throughput). Moving
#   operands (q, attn, relu(ctx), x1, h) are SPLIT fp8: hi = f8(x),
#   lo = f8(x - hi); hi+lo restores ~bf16 accuracy (verified: end-to-end rel
#   err 0.0149 == bf16 baseline), while each half streams at fp8 DR speed.
# - attn is stored as exp(scores - ln32) (fp8 range safety); the 1/32 scale
#   cancels exactly through the r = sum(attn) normalization.
# - FFN1 runs directly on quantized x1 (not x1-mu): h = W1@x1q + [w1sum;b1]
#   K-pair fix outer with rhs [-mu; std/8] (w1sum = quantized-W1 row sums), so
#   the x1->FFN1 chain does not wait for the mean/var statistics.
# - w2 is stored dc-major ([128, ND, NM, 128]) and DMAed in two dc-halves so
#   the final DMA only gates the last quarter of FFN2 + LN2 tail.
# - PSUM rules: (a) at most ONE matmul accumulation group open per 2KB PSUM
#   bank, (b) a group OVERWRITES its region when it closes, (c) pipeline
#   stages that overlap in time use separate tiles.
# - PE p-state ramps 0.65->1.2->2.4GHz with sustained-busy time and resets on
#   idle; warm-filler matmuls spin it up while the first k chunk streams in.
# - DMA issue order == consumption order (single HWDGE ring, 625ns per issue).
import sys

sys.path.insert(0, "/opt/trn_rl_repo")

import numpy as np
import ml_dtypes

import concourse.bass as bass
import concourse.mybir as mybir
import concourse.tile as tile
from concourse.bass_utils import run_bass_kernel_spmd
from concourse.masks import make_identity

# ---- Workaround: neuronxcc walrus rejects any instruction carrying more than
# one semaphore wait ("Too many sync wait commands"). Two pieces: (1) the Tile
# tail drain gets its waits split onto single-wait sync NOPs; (2) a post-pass
# splits multi-wait body instructions the same way.
import concourse.mybir as _mybir
from bass_rust import ScopedClock as _ScopedClock


def _drain_and_barrier(self, tick_clock, wait_clock):
    probe = self.nc.sync.nop(nofuse=True, hint="tail_wait_probe")
    wait_clock.add_sem_waits(probe.ins, _ScopedClock({None: tick_clock.global_clock}))
    waits = list(probe.ins.sync_info.on_wait)
    if waits:
        probe.ins.sync_info.on_wait = [waits[0]]
        for w in waits[1:]:
            n2 = self.nc.sync.nop(nofuse=True, hint="tail_wait_split")
            n2.ins.sync_info = _mybir.SyncInfo(on_wait=[w], on_update=[])
    self.nc.sync.drain()
    self.nc.all_engine_barrier()
    assert self.sems is not None
    popped = self.nc._tile_sem_poison_stack.pop()
    assert popped is self._sem_poison
    self.nc.clear_and_free_semaphores(list(self.sems.allocated().values()))
    self.nc.all_engine_barrier()


tile.TileContext._drain_and_barrier = _drain_and_barrier


def _split_multi_waits(nc, max_waits=1):
    uid = [0]
    for f in nc.m.functions:
        for bb in f.blocks:
            out = []
            for inst in bb.instructions:
                si = getattr(inst, "sync_info", None)
                if si is not None and si.on_wait and len(si.on_wait) > max_waits:
                    waits = list(si.on_wait)
                    for w in waits[:-max_waits]:
                        uid[0] += 1
                        nop = _mybir.InstNoOp(
                            name=f"I-waitsplit-{uid[0]}",
                            engine=inst.engine,
                            bass_nofuse=True,
                            ins=[], outs=[],
                            sync_info=_mybir.SyncInfo(on_wait=[w], on_update=[]),
                        )
                        out.append(nop)
                    inst.sync_info = _mybir.SyncInfo(
                        on_wait=waits[-max_waits:], on_update=list(si.on_update)
                    )
                out.append(inst)
            bb.instructions = out


S, T, D, DFF = 512, 16384, 512, 2048
NCORES = 8
SL = S // NCORES          # 64 queries per core
TSH = T // NCORES         # 2048 frames per core
HALO = 64
SLAB = TSH + 2 * HALO     # 2176 = 17 * 128
NTC = SLAB // 128         # 17 t-chunks
ND = D // 128             # 4 d-chunks
NM = DFF // 128           # 16 dff-chunks
F32 = mybir.dt.float32
BF16 = mybir.dt.bfloat16
FP8 = mybir.dt.float8e4
F8 = ml_dtypes.float8_e4m3fn
BF = ml_dtypes.bfloat16
AOP = mybir.AluOpType
DR = mybir.MatmulPerfMode.DoubleRow
LNA = float(np.log(256.0))   # attn = exp(s - ln256): max exp ~208 < fp8 448

# scores/AV chunk grouping over the 17 t-chunks, aligned to PSUM banks.
TGROUPS = [(0, 8), (8, 16), (16, 17)]
WARMN = 6     # warm-filler matmuls (512 cols each) before first scores


def _bcast(ap, n, axis_insert=1):
    """Insert a stride-0 dim of size n into an AP (middle broadcast)."""
    new_ap = list(ap.ap)
    new_ap.insert(axis_insert, [0, n])
    return bass.AP(tensor=ap.tensor, offset=ap.offset, ap=new_ap)


def _build_nc(apply_affine=True):
    nc = bass.Bass()
    io = {}
    io["qT2"] = nc.dram_tensor("qT2", [128, 2, ND, SL], FP8, kind="ExternalInput")
    io["kT"] = nc.dram_tensor("kT", [128, ND, SLAB], FP8, kind="ExternalInput")
    io["v_r"] = nc.dram_tensor("v_r", [NTC, 128, D], FP8, kind="ExternalInput")
    io["maskT"] = nc.dram_tensor("maskT", [128, NTC, SL], FP8, kind="ExternalInput")
    io["w1T"] = nc.dram_tensor("w1T", [128, ND, DFF], FP8, kind="ExternalInput")
    # w2 dc-major so dc-halves are contiguous 2KB-per-partition DMAs
    io["w2T"] = nc.dram_tensor("w2T", [128, ND, NM, 128], FP8, kind="ExternalInput")
    # wtT carries a 513th column per d-chunk: the Wt column sums (for the
    # early mean path  sum_d tgt2_raw = wtcol . ctxrT)
    io["wtT"] = nc.dram_tensor("wtT", [128, ND, D + 32], FP8,
                               kind="ExternalInput")
    # wext row pairs: [0:NM] = ([w1sum_fc],[8*b1eff_fc]); [NM:NM+ND] =
    # ([8*b2_dc],[0])
    io["wext"] = nc.dram_tensor("wext", [1, NM + ND, 2, 128], FP8,
                                kind="ExternalInput")
    # rows: [tgtb_rowsum ; tgtb_sq_rowsum] f32
    io["rows"] = nc.dram_tensor("rows", [1, 2, SL], F32, kind="ExternalInput")
    io["tgtbT"] = nc.dram_tensor("tgtbT", [128, ND, SL], BF16, kind="ExternalInput")
    if apply_affine:
        for nm in ("g2v", "be2v", "g3v", "be3v"):
            io[nm] = nc.dram_tensor(nm, [D], F32, kind="ExternalInput")
    out_h = nc.dram_tensor("out", [SL, D], F32, kind="ExternalOutput")
    import os as _os
    _dbg = bool(_os.environ.get("KDBG"))
    if _dbg:
        io_dbg = {
            "d_mu": nc.dram_tensor("d_mu", [1, SL], F32, kind="ExternalOutput"),
            "d_std": nc.dram_tensor("d_std", [1, SL], F32, kind="ExternalOutput"),
            "d_rrec": nc.dram_tensor("d_rrec", [1, SL], F32, kind="ExternalOutput"),
            "d_x1": nc.dram_tensor("d_x1", [128, ND, SL], F32, kind="ExternalOutput"),
            "d_h": nc.dram_tensor("d_h", [128, NM, SL], F32, kind="ExternalOutput"),
            "d_o2": nc.dram_tensor("d_o2", [128, ND, SL], F32, kind="ExternalOutput"),
            "d_xhat": nc.dram_tensor("d_xhat", [SL, D], F32, kind="ExternalOutput"),
            "d_x2": nc.dram_tensor("d_x2", [SL, D], F32, kind="ExternalOutput"),
        }

    with tile.TileContext(nc) as tc:
        with (
            tc.tile_pool(name="cst", bufs=1) as cst,
            tc.tile_pool(name="ps", bufs=1, space="PSUM") as psp,
        ):
            # ---- SBUF tiles
            qT2 = cst.tile([128, 2, ND, SL], FP8, tag="qT2")
            kT = cst.tile([128, ND, SLAB], FP8, tag="kT")
            v_sb = cst.tile([128, NTC, D], FP8, tag="v")
            maskT = cst.tile([128, NTC, SL], FP8, tag="maskT")
            wtT = cst.tile([128, ND, D + 32], FP8, tag="wt")
            w1T = cst.tile([128, ND, DFF], FP8, tag="w1")
            w2T = cst.tile([128, ND, NM, 128], FP8, tag="w2")
            wext = cst.tile([1, NM + ND, 2, 128], FP8, tag="wext")
            rows = cst.tile([1, 2, SL], F32, tag="rows")
            tgtbT = cst.tile([128, ND, SL], BF16, tag="tgtbT")

            # ---- DMA issue order == consumption order (single HWDGE ring).
            def kdma(gi):
                t0, t1 = TGROUPS[gi]
                nc.sync.dma_start(out=kT[:, :, t0 * 128:t1 * 128],
                                  in_=io["kT"][:][:, :, t0 * 128:t1 * 128])

            def vdma(gi):
                t0, t1 = TGROUPS[gi]
                nc.sync.dma_start(
                    out=v_sb[:, t0:t1, :],
                    in_=io["v_r"][t0:t1].rearrange("c p d -> p c d"))

            kdma(0)
            nc.sync.dma_start(out=qT2, in_=io["qT2"][:])
            nc.sync.dma_start(out=maskT, in_=io["maskT"][:])
            kdma(1)
            vdma(0)
            kdma(2)
            vdma(2)
            vdma(1)
            nc.sync.dma_start(out=wtT, in_=io["wtT"][:])
            nc.sync.dma_start(out=tgtbT, in_=io["tgtbT"][:])
            nc.sync.dma_start(out=rows, in_=io["rows"][:])
            nc.sync.dma_start(out=wext, in_=io["wext"][:])
            nc.sync.dma_start(out=w1T[:, :, 0:1024], in_=io["w1T"][:][:, :, 0:1024])
            nc.sync.dma_start(out=w1T[:, :, 1024:2048],
                              in_=io["w1T"][:][:, :, 1024:2048])
            # w2 dc-major: [dc0-2] then [dc3] so the final DMA gates only the
            # last quarter of FFN2 + the LN2 tail
            nc.sync.dma_start(out=w2T[:, 0:3], in_=io["w2T"][:][:, 0:3])
            nc.sync.dma_start(out=w2T[:, 3:4], in_=io["w2T"][:][:, 3:4])
            bvec = {}
            if apply_affine:
                for nm in ("g2v", "be2v", "g3v", "be3v"):
                    bvec[nm] = cst.tile([SL, D], F32, tag=nm, name=nm + "_b")
                    src = io[nm][:]
                    bcast = bass.AP(tensor=src.tensor, offset=src.offset,
                                    ap=[[0, SL]] + list(src.ap))
                    nc.gpsimd.dma_start(out=bvec[nm], in_=bcast)

            # ---- constants
            onesc = cst.tile([128, 1], FP8, tag="onesc")
            nc.vector.memset(onesc, 1.0)
            ones21 = cst.tile([128, 2, 32], FP8, tag="ones21")
            nc.vector.memset(ones21.rearrange("p a b -> p (a b)"), 1.0)
            onesb = cst.tile([128, 1], BF16, tag="onesb")
            nc.vector.memset(onesb, 1.0)
            twosb = cst.tile([128, 1], BF16, tag="twosb")
            nc.vector.memset(twosb, 2.0)
            ones_rf = cst.tile([1, 128], F32, tag="ones_rf")
            nc.vector.memset(ones_rf, 1.0)
            epsc1 = cst.tile([1, 1], F32, tag="epsc1")
            nc.vector.memset(epsc1, 1e-5)
            epsc64 = cst.tile([1, 1], F32, tag="epsc64")
            nc.vector.memset(epsc64, 1e-5 / 64.0)
            epsc = cst.tile([SL, 1], F32, tag="eps")
            nc.vector.memset(epsc, 1e-5)
            expb = cst.tile([128, 1], F32, tag="expb")
            nc.vector.memset(expb, -LNA)
            identf1 = cst.tile([1, 1], F32, tag="identf1")
            nc.vector.memset(identf1, 1.0)
            invD_row = cst.tile([1, SL], F32, tag="invD_row")
            nc.vector.memset(invD_row, 1.0 / D)
            negD_row = cst.tile([1, SL], F32, tag="negD_row")
            nc.vector.memset(negD_row, -float(D))
            identf = cst.tile([128, 128], F32, tag="identf")
            make_identity(nc, identf)
            identb = cst.tile([128, 128], BF16, tag="identb")
            make_identity(nc, identb)

            # ---- PSUM tiles (8 banks; see header notes)
            ps_sc = [
                psp.tile([128, 8, SL], F32, tag="scA", name="ps_scA"),
                psp.tile([128, 8, SL], F32, tag="scB", name="ps_scB"),
                psp.tile([128, 1, SL], F32, tag="sm", name="ps_scC"),
            ]
            ps_ctxT = psp.tile([128, ND, SL], F32, tag="med", name="ps_ctxT")
            ps_h = [
                psp.tile([128, 8, SL], F32, tag="hA", name="ps_hA"),
                psp.tile([128, 8, SL], F32, tag="hB", name="ps_hB"),
            ]
            ps_t2T = psp.tile([128, ND, SL], F32, tag="t2T", name="ps_t2T")
            ps_r = psp.tile([1, SL], F32, tag="aux", name="ps_r")
            ps_rb = psp.tile([128, SL], F32, tag="aux", name="ps_rb")
            ps_stat = psp.tile([1, 192], F32, tag="sm", name="ps_stat")

            # warm fillers: 512-col zero matmuls into the hA bank (untouched
            # until FFN1-A; groups closed immediately, WAW-safe).
            wzero = cst.tile([SL, 512], BF16, tag="wzero")
            nc.vector.memset(wzero.rearrange("p f -> p f"), 0.0)
            warm_out = ps_h[0].rearrange("p c s -> p (c s)")[0:SL, :]

            def warm(n):
                for _ in range(n):
                    nc.tensor.matmul(warm_out, lhsT=wzero[:, 0:SL], rhs=wzero,
                                     start=True, stop=True,
                                     skip_group_check=True)

            warm(3)

            # ---- attention: scoresT (kT chunks stationary, q hi/lo moving,
            # DoubleRow over dc pairs) -> +mask (DVE) -> exp hi fp8 + exp bf16
            # (ACT, bias -ln32) -> lo = bf - hi (DVE) -> AV (DoubleRow over tc
            # pairs, v stationary) with attn row sums via ones DR matmuls.
            # attn is SINGLE fp8 (exp writes fp8 directly): r is computed from
            # the same quantized attn, so the softmax normalization stays
            # exact and only the weighting carries the fp8 noise (modeled
            # end-to-end rel err 0.0157 < 2e-2 gate).
            attn_hi = cst.tile([128, NTC, SL], FP8, tag="attn_hi")

            def sc_group(gi):
                t0, t1 = TGROUPS[gi]
                ps = ps_sc[gi]
                for tcn in range(t0, t1):
                    k = 0
                    for hv in range(2):
                        for dcp in range(0, ND, 2):
                            nc.tensor.matmul(
                                ps[:, tcn - t0, :],
                                lhsT=kT[:, dcp:dcp + 2, tcn * 128:(tcn + 1) * 128],
                                rhs=qT2[:, hv, dcp:dcp + 2, :],
                                start=(k == 0), stop=(k == 3),
                                perf_mode=DR,
                            )
                            k += 1
                nc.vector.tensor_add(ps[:, 0:t1 - t0, :], ps[:, 0:t1 - t0, :],
                                     maskT[:, t0:t1, :])
                nc.scalar.activation(out=attn_hi[:, t0:t1, :],
                                     in_=ps[:, 0:t1 - t0, :],
                                     func=mybir.ActivationFunctionType.Exp,
                                     bias=expb, scale=1.0)

            def av_group(gi, first, last):
                t0, t1 = TGROUPS[gi]
                if t1 - t0 == 8:
                    for tcp in range(t0, t1, 2):
                        for dc in range(ND):
                            nc.tensor.matmul(
                                ps_ctxT[:, dc, :],
                                lhsT=v_sb[:, tcp:tcp + 2,
                                          dc * 128:(dc + 1) * 128],
                                rhs=attn_hi[:, tcp:tcp + 2, :],
                                start=(first and tcp == t0 and dc == 0),
                                stop=(last and tcp == t1 - 2 and dc == ND - 1),
                                perf_mode=DR,
                                skip_group_check=True,
                            )
                        nc.tensor.matmul(
                            ps_r, lhsT=ones21[:, :, 0:1],
                            rhs=attn_hi[:, tcp:tcp + 2, :],
                            start=(first and tcp == t0),
                            stop=(last and tcp == t1 - 2),
                            perf_mode=DR,
                            skip_group_check=True,
                        )
                else:  # single chunk: plain fp8 matmuls
                    for dc in range(ND):
                        nc.tensor.matmul(
                            ps_ctxT[:, dc, :],
                            lhsT=v_sb[:, t0, dc * 128:(dc + 1) * 128],
                            rhs=attn_hi[:, t0, :],
                            start=(first and dc == 0),
                            stop=(last and dc == ND - 1),
                            skip_group_check=True,
                        )
                    nc.tensor.matmul(
                        ps_r, lhsT=onesc, rhs=attn_hi[:, t0, :],
                        start=first, stop=last,
                        skip_group_check=True,
                    )

            # av order [0, 2, 1]: v1 is the LAST v transfer, so av_group(1)
            # closes the ctx/r accumulation; the small g2 tail (mask2/exp2/
            # av2) hides under the v1 transfer.
            warm(4)
            sc_group(0)
            sc_group(1)
            sc_group(2)
            av_group(0, True, False)
            av_group(2, False, False)
            av_group(1, False, True)

            # r^-1 row first on DVE (only needs ps_r), then ctx lo
            rrec = cst.tile([1, SL], F32, tag="rrec")
            nc.vector.reciprocal(out=rrec, in_=ps_r)
            # partition broadcast of r^-1 (K=1 fp32 outer) + sbuf copy
            nc.tensor.matmul(ps_rb, lhsT=ones_rf, rhs=rrec,
                             start=True, stop=True, skip_group_check=True)
            rb_sb = cst.tile([128, SL], F32, tag="rb_sb")
            nc.vector.tensor_copy(out=rb_sb, in_=ps_rb)
            rb_bc = _bcast(rb_sb[:], ND)

            # ctx stays UN-normalized (r^-1 column scaling commutes through
            # Wt and folds into x1): ctx_hi = fp8 relu straight off psum on
            # ACT; ctx_lo = second psum read on DVE, overlapping the hi DRs
            ctx_hi = cst.tile([128, ND, SL], FP8, tag="ctx_hi")
            ctx_lo = cst.tile([128, ND, SL], FP8, tag="ctx_lo")
            nc.vector.tensor_scalar_max(
                ctx_hi.rearrange("p c s -> p (c s)"),
                ps_ctxT.rearrange("p c s -> p (c s)"), 0.0)
            nc.vector.scalar_tensor_tensor(
                out=ctx_lo.rearrange("p c s -> p (c s)"),
                in0=ps_ctxT.rearrange("p c s -> p (c s)"),
                scalar=0.0,
                in1=ctx_hi.rearrange("p c s -> p (c s)"),
                op0=AOP.max, op1=AOP.subtract)

            # early mean path: S1 = wtcol . ctxn = sum_d tgt2T
            k = 0
            for hv, ctx in ((0, ctx_hi), (1, ctx_lo)):
                for dcp in range(0, ND, 2):
                    nc.tensor.matmul(ps_stat[:, 0:SL],
                                     lhsT=wtT[:, dcp:dcp + 2, D:D + 1],
                                     rhs=ctx[:, dcp:dcp + 2, :],
                                     start=(k == 0), stop=(k == 3),
                                     perf_mode=DR, skip_group_check=True)
                    k += 1

            # tgt2T [dout, s] = Wt @ relu(ctx) (raw): each oc group is
            # CONTIGUOUS (only one accumulation group may be open per bank)
            for oc in range(ND):
                k = 0
                for hv, ctx in ((0, ctx_hi), (1, ctx_lo)):
                    for dcp in range(0, ND, 2):
                        nc.tensor.matmul(
                            ps_t2T[:, oc, :],
                            lhsT=wtT[:, dcp:dcp + 2, oc * 128:(oc + 1) * 128],
                            rhs=ctx[:, dcp:dcp + 2, :],
                            start=(k == 0), stop=(k == 3),
                            perf_mode=DR,
                        )
                        k += 1

            # x1 = tgt2T*r^-1 + tgtb: ps_t2T is read ONCE (psum reads of one
            # bank serialize across engines); hi fp8 from x1s on DVE, f32 on
            # Pool (parallel), lo + Square(x1) after
            x1s = cst.tile([128, ND, SL], F32, tag="x1s")
            nc.vector.tensor_mul(x1s, ps_t2T, rb_bc)
            x1hi = cst.tile([128, ND, SL], FP8, tag="x1hi")
            nc.vector.tensor_add(x1hi.rearrange("p c s -> p (c s)"),
                                 x1s.rearrange("p c s -> p (c s)"),
                                 tgtbT.rearrange("p c s -> p (c s)"))
            x1Tf = cst.tile([128, ND, SL], F32, tag="x1Tf")
            nc.gpsimd.tensor_add(x1Tf.rearrange("p c s -> p (c s)"),
                                 x1s.rearrange("p c s -> p (c s)"),
                                 tgtbT.rearrange("p c s -> p (c s)"))
            x1lo = cst.tile([128, ND, SL], FP8, tag="x1lo")
            nc.vector.tensor_sub(x1lo.rearrange("p c s -> p (c s)"),
                                 x1Tf.rearrange("p c s -> p (c s)"),
                                 x1hi.rearrange("p c s -> p (c s)"))
            x1sq = cst.tile([128, ND, SL], BF16, tag="x1sq")
            nc.scalar.activation(out=x1sq.rearrange("p c s -> p (c s)"),
                                 in_=x1s.rearrange("p c s -> p (c s)"),
                                 func=mybir.ActivationFunctionType.Square)
            x1cr = cst.tile([128, ND, SL], BF16, tag="x1cr")
            nc.vector.tensor_mul(x1cr.rearrange("p c s -> p (c s)"),
                                 x1s.rearrange("p c s -> p (c s)"),
                                 tgtbT.rearrange("p c s -> p (c s)"))

            # mu algebra on Pool: S1 is RAW (unnormalized ctx), so
            # mu = (S1*r^-1)/D + tsum/D (host pre-divides rows[0] by D)
            s1n_row = cst.tile([1, SL], F32, tag="s1n_row")
            nc.vector.tensor_mul(s1n_row, ps_stat[:, 0:SL], rrec)
            mu_row = cst.tile([1, SL], F32, tag="mu_row")
            nc.vector.scalar_tensor_tensor(out=mu_row, in0=s1n_row,
                                           scalar=1.0 / D, in1=rows[0:1, 0, :],
                                           op0=AOP.mult, op1=AOP.add)
            musqD = cst.tile([1, SL], F32, tag="musqD")
            nc.vector.scalar_tensor_tensor(out=musqD, in0=mu_row,
                                           scalar=-float(D), in1=mu_row,
                                           op0=AOP.mult, op1=AOP.mult)
            cmb_row = cst.tile([1, SL], F32, tag="cmb_row")
            nc.vector.tensor_add(cmb_row, musqD, rows[0:1, 1, :])
            # mustd fp8 row pair: [-mu ; std/8] (fix outer rhs)
            mustd = cst.tile([1, 2, SL], FP8, tag="mustd")
            nc.gpsimd.tensor_scalar_mul(mustd[0:1, 0, :], mu_row, -1.0)

            # variance chain: varD = sum x1^2 - D mu^2 (stat var matmuls gate)
            for dc in range(ND):
                nc.tensor.matmul(ps_stat[:, SL:2 * SL], lhsT=onesb,
                                 rhs=x1sq[:, dc, :],
                                 start=(dc == 0), stop=False,
                                 skip_group_check=True)
            for dc in range(ND):
                nc.tensor.matmul(ps_stat[:, SL:2 * SL], lhsT=twosb,
                                 rhs=x1cr[:, dc, :],
                                 start=False, stop=(dc == ND - 1),
                                 skip_group_check=True)
            varD_row = cst.tile([1, SL], F32, tag="varD_row")
            nc.vector.tensor_add(varD_row, ps_stat[:, SL:2 * SL], cmb_row)
            # std/8 = sqrt(varD/(64 D) + eps/64) straight into the fp8 pair
            nc.scalar.activation(out=mustd[0:1, 1, :], in_=varD_row,
                                 func=mybir.ActivationFunctionType.Sqrt,
                                 bias=epsc64, scale=1.0 / (64.0 * D))
            # off-chain: f32 std / rstd for the residual scaling
            std_row = cst.tile([1, SL], F32, tag="std_row")
            nc.scalar.activation(out=std_row, in_=varD_row,
                                 func=mybir.ActivationFunctionType.Sqrt,
                                 bias=epsc1, scale=1.0 / D)
            rstd_row = cst.tile([1, SL], F32, tag="rstd_row")
            nc.vector.reciprocal(out=rstd_row, in_=std_row)

            # ---- FFN1: h = W1q @ (x1hi + x1lo) + [w1sum;8b1] (x) [-mu;std/8]
            h_hi = cst.tile([128, NM, SL], FP8, tag="h_hi")
            h_lo = cst.tile([128, NM, SL], FP8, tag="h_lo")

            def ffn1_fc(fc):
                # fix FIRST (start=True): the in-order PE stream then stalls
                # on mustd only once, at the head, instead of between every
                # fc group's matmuls
                nc.tensor.matmul(ps_h[fc // 8][:, fc % 8, :],
                                 lhsT=wext[:, fc, :, :],
                                 rhs=mustd,
                                 start=True, stop=False,
                                 perf_mode=DR)
                k = 0
                for hv, x1q in ((0, x1hi), (1, x1lo)):
                    for dcp in range(0, ND, 2):
                        nc.tensor.matmul(
                            ps_h[fc // 8][:, fc % 8, :],
                            lhsT=w1T[:, dcp:dcp + 2, fc * 128:(fc + 1) * 128],
                            rhs=x1q[:, dcp:dcp + 2, :],
                            start=False, stop=(k == 3),
                            perf_mode=DR,
                        )
                        k += 1

            # h_hi = fp8 relu straight off psum (ACT) so FFN2-hi can start
            # immediately; h_lo = second psum read (DVE), overlapping the
            # hi DRs on PE
            def h_group8(g):
                sl8 = slice(8 * g, 8 * g + 8)
                nc.scalar.activation(
                    out=h_hi[:, sl8, :],
                    in_=ps_h[g],
                    func=mybir.ActivationFunctionType.Relu)
                nc.vector.scalar_tensor_tensor(
                    out=h_lo[:, sl8, :].rearrange("p c s -> p (c s)"),
                    in0=ps_h[g].rearrange("p c s -> p (c s)"),
                    scalar=0.0,
                    in1=h_hi[:, sl8, :].rearrange("p c s -> p (c s)"),
                    op0=AOP.max, op1=AOP.subtract)

            for fc in range(8):
                ffn1_fc(fc)
            h_group8(0)
            for fc in range(8, 16):
                ffn1_fc(fc)
            h_group8(1)

            # off-chain transposes fill the PE stall while w2 streams in
            ps_x1 = psp.tile([SL, D], F32, tag="scA", name="ps_x1")
            for dc in range(ND):
                nc.tensor.transpose(ps_x1[:, dc * 128:(dc + 1) * 128],
                                    x1Tf[:, dc, :], identf)
            ps_mr = psp.tile([SL, 2], F32, tag="sm", name="ps_mr")
            nc.tensor.transpose(ps_mr[:, 0:1], mu_row, identf1)
            nc.tensor.transpose(ps_mr[:, 1:2], rstd_row, identf1)

            # ---- FFN2: one accumulation group per dc, each in its OWN psum
            # bank so all four can be open at once; the hi-operand DRs for
            # dc0-2 run before h_lo is even ready, the lo DRs + closes follow.
            # dc3 is gated by the final w2 DMA and has the shortest tail.
            ps_o2dc = [
                psp.tile([128, SL], F32, tag="med", name="ps_o2d0"),
                psp.tile([128, SL], F32, tag="t2T", name="ps_o2d1"),
                psp.tile([128, SL], F32, tag="hA", name="ps_o2d2"),
                psp.tile([128, SL], F32, tag="hB", name="ps_o2d3"),
            ]
            ps_o2 = psp.tile([SL, D], BF16, tag="aux", name="ps_o2")
            ps_o2b = psp.tile([SL, D // 2], BF16, tag="sm", name="ps_o2b")
            mustd_s = bass.AP(tensor=mustd.tensor, offset=mustd[0:1, 1, :].offset,
                              ap=[list(mustd.ap[0]), [0, 2], [1, SL]])

            def ffn2_hi(dc):
                for fcp in range(0, NM, 2):
                    nc.tensor.matmul(
                        ps_o2dc[dc],
                        lhsT=w2T[:, dc, fcp:fcp + 2, :],
                        rhs=h_hi[:, fcp:fcp + 2, :],
                        start=(fcp == 0), stop=False,
                        perf_mode=DR,
                        skip_group_check=True,
                    )

            def ffn2_lo_close(dc):
                for fcp in range(0, NM, 2):
                    nc.tensor.matmul(
                        ps_o2dc[dc],
                        lhsT=w2T[:, dc, fcp:fcp + 2, :],
                        rhs=h_lo[:, fcp:fcp + 2, :],
                        start=False, stop=False,
                        perf_mode=DR,
                        skip_group_check=True,
                    )
                nc.tensor.matmul(ps_o2dc[dc],
                                 lhsT=wext[:, NM + dc, :, :],
                                 rhs=mustd_s,
                                 start=False, stop=True,
                                 perf_mode=DR,
                                 skip_group_check=True)

            # xhat = rstd * (x1 - mu) row-major f32 (early: overlaps FFN2)
            mr_col = cst.tile([SL, 2], F32, tag="mr_col")
            nc.vector.tensor_copy(out=mr_col, in_=ps_mr)
            xhat = cst.tile([SL, D], F32, tag="xhat")
            for qc in range(1):
                cols = slice(0, D)
                nc.vector.tensor_scalar(out=xhat[:, cols], in0=ps_x1[:, cols],
                                        scalar1=mr_col[:, 0:1],
                                        scalar2=mr_col[:, 1:2],
                                        op0=AOP.subtract, op1=AOP.mult)
            if apply_affine:
                nc.vector.tensor_mul(xhat, xhat, bvec["g2v"])
                nc.vector.tensor_add(xhat, xhat, bvec["be2v"])

            o2Ts = cst.tile([128, ND, SL], BF16, tag="o2Ts")
            x2 = cst.tile([SL, D], F32, tag="x2")
            SD = nc.vector.BN_STATS_DIM
            st2 = cst.tile([SL, 4 * SD], F32, tag="st2")

            def trans_dc(dc):
                tgt = ps_o2[:, dc * 128:(dc + 1) * 128] if dc < 2 else \
                    ps_o2b[:, (dc - 2) * 128:(dc - 1) * 128]
                nc.tensor.transpose(tgt, o2Ts[:, dc, :], identb)

            def x2_bn_dc(dc):
                src = ps_o2[:, dc * 128:(dc + 1) * 128] if dc < 2 else \
                    ps_o2b[:, (dc - 2) * 128:(dc - 1) * 128]
                cols = slice(dc * 128, (dc + 1) * 128)
                nc.vector.scalar_tensor_tensor(out=x2[:, cols], in0=src,
                                               scalar=mr_col[:, 1:2],
                                               in1=xhat[:, cols],
                                               op0=AOP.mult, op1=AOP.add)
                nc.vector.bn_stats(out=st2[:, dc * SD:(dc + 1) * SD],
                                   in_=x2[:, cols])

            for dc in range(3):
                ffn2_hi(dc)
            for dc in range(3):
                ffn2_lo_close(dc)
            ffn2_hi(3)
            nc.vector.tensor_copy(out=o2Ts[:, 0, :], in_=ps_o2dc[0])
            nc.vector.tensor_copy(out=o2Ts[:, 1, :], in_=ps_o2dc[1])
            nc.vector.tensor_copy(out=o2Ts[:, 2, :], in_=ps_o2dc[2])
            for dc in range(3):
                trans_dc(dc)
            for dc in range(3):
                x2_bn_dc(dc)
            ffn2_lo_close(3)
            nc.vector.tensor_copy(out=o2Ts[:, 3, :], in_=ps_o2dc[3])
            trans_dc(3)
            x2_bn_dc(3)
            mv2 = cst.tile([SL, nc.vector.BN_AGGR_DIM], F32, tag="mv2")
            nc.vector.bn_aggr(out=mv2, in_=st2)
            std2 = cst.tile([SL, 1], F32, tag="std2")
            nc.scalar.activation(out=std2, in_=mv2[:, 1:2],
                                 func=mybir.ActivationFunctionType.Sqrt,
                                 bias=epsc, scale=1.0)
            rstd2 = cst.tile([SL, 1], F32, tag="rstd2")
            nc.vector.reciprocal(out=rstd2, in_=std2)
            out_sb = cst.tile([SL, D], F32, tag="out")
            nc.vector.tensor_scalar(out=out_sb, in0=x2,
                                    scalar1=mv2[:, 0:1], scalar2=rstd2,
                                    op0=AOP.subtract, op1=AOP.mult)
            if apply_affine:
                nc.vector.tensor_mul(out_sb, out_sb, bvec["g3v"])
                nc.vector.tensor_add(out_sb, out_sb, bvec["be3v"])
            nc.sync.dma_start(out=out_h[:], in_=out_sb)
            if _dbg:
                nc.sync.dma_start(out=io_dbg["d_mu"][:], in_=mu_row)
                nc.sync.dma_start(out=io_dbg["d_std"][:], in_=std_row)
                nc.sync.dma_start(out=io_dbg["d_rrec"][:], in_=rrec)
                dx1 = cst.tile([128, ND, SL], F32, tag="dx1")
                nc.vector.tensor_add(dx1.rearrange("p c s -> p (c s)"),
                                     x1hi.rearrange("p c s -> p (c s)"),
                                     x1lo.rearrange("p c s -> p (c s)"))
                nc.sync.dma_start(out=io_dbg["d_x1"][:], in_=dx1)
                dh = cst.tile([128, NM, SL], F32, tag="dh")
                nc.vector.tensor_add(dh.rearrange("p c s -> p (c s)"),
                                     h_hi.rearrange("p c s -> p (c s)"),
                                     h_lo.rearrange("p c s -> p (c s)"))
                nc.sync.dma_start(out=io_dbg["d_h"][:], in_=dh)
                do2 = cst.tile([128, ND, SL], F32, tag="do2")
                nc.vector.tensor_copy(out=do2.rearrange("p c s -> p (c s)"),
                                      in_=o2Ts.rearrange("p c s -> p (c s)"))
                nc.sync.dma_start(out=io_dbg["d_o2"][:], in_=do2)
                nc.sync.dma_start(out=io_dbg["d_xhat"][:], in_=xhat)
                nc.sync.dma_start(out=io_dbg["d_x2"][:], in_=x2)

    _split_multi_waits(nc)
    return nc


_NC_CACHE = {}


def _f8(x):
    return np.asarray(x, np.float32).astype(F8)


def _prep_inputs(tgt, memory, pos, query_pos, action_idx,
                 W_tgt2, b_tgt2, W1, b1, W2, b2, g2, be2, g3, be3):
    inv = np.float32(1.0 / np.sqrt(D))
    tgt2d = np.ascontiguousarray(tgt[:, 0, :], np.float32)        # [S, D]
    qp2d = np.ascontiguousarray(query_pos[:, 0, :], np.float32)
    mem2d = np.ascontiguousarray(memory[:, 0, :], np.float32)     # [T, D]
    pos2d = np.ascontiguousarray(pos[:, 0, :], np.float32)

    k2d = mem2d + pos2d
    k_p = np.zeros((T + 2 * HALO, D), np.float32)
    k_p[HALO:HALO + T] = k2d
    mem_p = np.zeros((T + 2 * HALO, D), np.float32)
    mem_p[HALO:HALO + T] = mem2d
    q2d = (tgt2d + qp2d) * inv                                    # [S, D]

    # segment ids from action_idx change points (mirrors the reference mask)
    ai = np.asarray(action_idx)
    change = np.concatenate([[0], (ai[1:] != ai[:-1]).astype(np.int64)])
    seg_id = np.cumsum(change)

    aff = _needs_affine(g2, be2, g3, be3)
    W1f = np.asarray(W1, np.float32)
    b1f = np.asarray(b1, np.float32)
    if aff:
        # fold g2/be2 into FFN1: h1 = (x^)@ (W1*g2).T + (b1 + W1@be2)
        W1eff = W1f * np.asarray(g2, np.float32)[None, :]
        b1eff = b1f + W1f @ np.asarray(be2, np.float32)
    else:
        W1eff, b1eff = W1f, b1f

    w1T_h = np.ascontiguousarray(
        W1eff.T.reshape(ND, 128, DFF).transpose(1, 0, 2)).astype(F8)
    # w2 dc-major: w2T[p, dc, fc, j] = W2[dc*128+j, fc*128+p]
    w2T_h = np.ascontiguousarray(
        np.asarray(W2, np.float32).T.reshape(NM, 128, ND, 128)
        .transpose(1, 2, 0, 3)).astype(F8)
    wtT_q = np.ascontiguousarray(
        np.asarray(W_tgt2, np.float32).T.reshape(ND, 128, D)
        .transpose(1, 0, 2)).astype(F8)
    # 513th column per d-chunk: Wt column sums (of the quantized weights)
    wtcol = np.asarray(wtT_q, np.float32).sum(axis=2)              # [128, ND]
    wtT_h = np.zeros((128, ND, D + 32), np.float32)
    wtT_h[:, :, 0:D] = np.asarray(wtT_q, np.float32)
    wtT_h[:, :, D] = wtcol
    wtT_h = np.ascontiguousarray(wtT_h.astype(F8))
    # wext pairs: fc rows ([w1sum_fc],[8*b1eff_fc]); dc rows ([8*b2_dc],[0])
    w1sum = np.asarray(w1T_h, np.float32).sum(axis=0).sum(axis=0)  # [DFF]
    b2f = np.asarray(b2, np.float32)
    wext_h = np.zeros((1, NM + ND, 2, 128), np.float32)
    wext_h[0, 0:NM, 0, :] = w1sum.reshape(NM, 128)
    wext_h[0, 0:NM, 1, :] = 8.0 * b1eff.reshape(NM, 128)
    wext_h[0, NM:NM + ND, 0, :] = 8.0 * b2f.reshape(ND, 128)
    wext_h = np.ascontiguousarray(wext_h).astype(F8)

    in_maps = []
    for c in range(NCORES):
        sl = slice(c * SL, (c + 1) * SL)
        qc = q2d[sl].T.reshape(ND, 128, SL).transpose(1, 0, 2)     # [128,ND,SL]
        q_hi = qc.astype(F8)
        q_lo = (qc - q_hi.astype(np.float32)).astype(F8)
        qT2c = np.ascontiguousarray(
            np.stack([np.asarray(q_hi), np.asarray(q_lo)], axis=1))
        kslab = k_p[c * TSH:c * TSH + SLAB]                       # [2176, D]
        kTc = kslab.T.reshape(ND, 128, SLAB).transpose(1, 0, 2).astype(F8)
        v_h = mem_p[c * TSH:c * TSH + SLAB].reshape(NTC, 128, D).astype(F8)

        # additive band mask in T layout [128, NTC, SL]: 0 where query j
        # (global s=64c+j) attends slab frame t, else -60; pad rows stay -60.
        mk = np.full((SL, SLAB), -60.0, np.float32)
        g0 = c * TSH - HALO
        glo, ghi = max(0, g0), min(T, g0 + SLAB)
        if ghi > glo:
            seg = seg_id[glo:ghi]
            svec = np.arange(c * SL, (c + 1) * SL)
            ok = (np.abs(seg[None, :] - svec[:, None]) <= 1)
            mk[:, glo - g0:ghi - g0][ok] = 0.0
        mkT = np.ascontiguousarray(
            mk.T.reshape(NTC, 128, SL).transpose(1, 0, 2)).astype(F8)

        tgtb = (tgt2d[sl] + np.asarray(b_tgt2, np.float32)).astype(BF)
        tgtbT = np.ascontiguousarray(
            tgtb.T.reshape(ND, 128, SL).transpose(1, 0, 2))
        tgtbf = tgtb.astype(np.float32)
        # rows[0] = tsum/D (pre-divided for the fused device-side mu stt)
        rows_h = np.stack([tgtbf.sum(axis=1) / D,
                           (tgtbf * tgtbf).sum(axis=1)]).reshape(1, 2, SL)
        rows_h = np.ascontiguousarray(rows_h, np.float32)

        im = {
            "qT2": qT2c,
            "kT": np.ascontiguousarray(kTc),
            "v_r": np.ascontiguousarray(v_h),
            "maskT": mkT,
            "w1T": w1T_h,
            "w2T": w2T_h,
            "wtT": wtT_h,
            "wext": wext_h,
            "rows": rows_h,
            "tgtbT": tgtbT,
        }
        if aff:
            im.update({
                "g2v": np.asarray(g2, np.float32),
                "be2v": np.asarray(be2, np.float32),
                "g3v": np.asarray(g3, np.float32),
                "be3v": np.asarray(be3, np.float32),
            })
        in_maps.append(im)
    return in_maps


def _needs_affine(g2, be2, g3, be3):
    return not (np.all(np.asarray(g2) == 1) and np.all(np.asarray(g3) == 1)
                and np.all(np.asarray(be2) == 0) and np.all(np.asarray(be3) == 0))


_LAST = {}


def kernel(**inputs) -> np.ndarray:
    inputs = {k: np.asarray(v) for k, v in inputs.items()}
    aff = _needs_affine(inputs["g2"], inputs["be2"], inputs["g3"], inputs["be3"])
    if aff not in _NC_CACHE:
        _NC_CACHE[aff] = _build_nc(apply_affine=aff)
    nc = _NC_CACHE[aff]
    in_maps = _prep_inputs(**inputs)
    import os
    kw = {}
    if os.environ.get("BASS_TRACE"):
        kw = dict(trace=True, tmpdir=os.environ.get("BASS_TRACE_DIR") or None)
    res = run_bass_kernel_spmd(nc, in_maps, core_ids=list(range(NCORES)), **kw)
    _LAST["res"] = res
    out = np.concatenate([res.results[c]["out"] for c in range(NCORES)], axis=0)
    return np.ascontiguousarray(out.reshape(S, 1, D).astype(np.float32))


# revision 51
# speedup vs baseline: 1.0528x; 1.0166x over previous
# Trainium2 Bass kernel for nn_CrossAttention_6579889897579 (sparse segment-
# neighbor cross-attention + FFN block).
#
# Sharding: the S=512 queries map 1:1 onto 512 contiguous 32-frame segments of
# the T=16384 memory (action_idx encodes the segmentation; seg boundaries are
# recomputed from it on the host). Query s attends segments {s-1,s,s+1} =
# frames [32s-32, 32s+64). Sharding S across 8 cores (64 queries/core) makes
# attention block-local: core c only needs frames [2048c-64, 2048c+2112) (a
# 2176-frame slab, zero-padded at the global edges). No collectives.
#
# v4 design notes (cost-model-driven; v3 was 27437ns):
# - DMA is the serialized bottleneck (360 GB/s aggregate, one transfer at a
#   time), so the big streams (k, v, mask, weights) travel as fp8 e4m3.
# - All large matmuls use fp8 DoubleRow perf mode (two 128-deep fp8 matmuls
#   summed per instruction at 0.5 cycles/row = 4x bf16 throughput). Moving
#   operands (q, attn, relu(ctx), x1, h) are SPLIT fp8: hi = f8(x),
#   lo = f8(x - hi); hi+lo restores ~bf16 accuracy (verified: end-to-end rel
#   err 0.0149 == bf16 baseline), while each half streams at fp8 DR speed.
# - attn is stored as exp(scores - ln32) (fp8 range safety); the 1/32 scale
#   cancels exactly through the r = sum(attn) normalization.
# - FFN1 runs directly on quantized x1 (not x1-mu): h = W1@x1q + [w1sum;b1]
#   K-pair fix outer with rhs [-mu; std/8] (w1sum = quantized-W1 row sums), so
#   the x1->FFN1 chain does not wait for the mean/var statistics.
# - w2 is stored dc-major ([128, ND, NM, 128]) and DMAed in two dc-halves so
#   the final DMA only gates the last quarter of FFN2 + LN2 tail.
# - PSUM rules: (a) at most ONE matmul accumulation group open per 2KB PSUM
#   bank, (b) a group OVERWRITES its region when it closes, (c) pipeline
#   stages that overlap in time use separate tiles.
# - PE p-state ramps 0.65->1.2->2.4GHz with sustained-busy time and resets on
#   idle; warm-filler matmuls spin it up while the first k chunk streams in.
# - DMA issue order == consumption order (single HWDGE ring, 625ns per issue).
import sys

sys.path.insert(0, "/opt/trn_rl_repo")

import numpy as np
import ml_dtypes

import concourse.bass as bass
import concourse.mybir as mybir
import concourse.tile as tile
from concourse.bass_utils import run_bass_kernel_spmd
from concourse.masks import make_identity

# ---- Workaround: neuronxcc walrus rejects any instruction carrying more than
# one semaphore wait ("Too many sync wait commands"). Two pieces: (1) the Tile
# tail drain gets its waits split onto single-wait sync NOPs; (2) a post-pass
# splits multi-wait body instructions the same way.
import concourse.mybir as _mybir
from bass_rust import ScopedClock as _ScopedClock


def _drain_and_barrier(self, tick_clock, wait_clock):
    probe = self.nc.sync.nop(nofuse=True, hint="tail_wait_probe")
    wait_clock.add_sem_waits(probe.ins, _ScopedClock({None: tick_clock.global_clock}))
    waits = list(probe.ins.sync_info.on_wait)
    if waits:
        probe.ins.sync_info.on_wait = [waits[0]]
        for w in waits[1:]:
            n2 = self.nc.sync.nop(nofuse=True, hint="tail_wait_split")
            n2.ins.sync_info = _mybir.SyncInfo(on_wait=[w], on_update=[])
    self.nc.sync.drain()
    self.nc.all_engine_barrier()
    assert self.sems is not None
    popped = self.nc._tile_sem_poison_stack.pop()
    assert popped is self._sem_poison
    self.nc.clear_and_free_semaphores(list(self.sems.allocated().values()))
    self.nc.all_engine_barrier()


tile.TileContext._drain_and_barrier = _drain_and_barrier


def _split_multi_waits(nc, max_waits=1):
    uid = [0]
    for f in nc.m.functions:
        for bb in f.blocks:
            out = []
            for inst in bb.instructions:
                si = getattr(inst, "sync_info", None)
                if si is not None and si.on_wait and len(si.on_wait) > max_waits:
                    waits = list(si.on_wait)
                    for w in waits[:-max_waits]:
                        uid[0] += 1
                        nop = _mybir.InstNoOp(
                            name=f"I-waitsplit-{uid[0]}",
                            engine=inst.engine,
                            bass_nofuse=True,
                            ins=[], outs=[],
                            sync_info=_mybir.SyncInfo(on_wait=[w], on_update=[]),
                        )
                        out.append(nop)
                    inst.sync_info = _mybir.SyncInfo(
                        on_wait=waits[-max_waits:], on_update=list(si.on_update)
                    )
                out.append(inst)
            bb.instructions = out


S, T, D, DFF = 512, 16384, 512, 2048
NCORES = 8
SL = S // NCORES          # 64 queries per core
TSH = T // NCORES         # 2048 frames per core
HALO = 64
SLAB = TSH + 2 * HALO     # 2176 = 17 * 128
NTC = SLAB // 128         # 17 t-chunks
ND = D // 128             # 4 d-chunks
NM = DFF // 128           # 16 dff-chunks
F32 = mybir.dt.float32
BF16 = mybir.dt.bfloat16
FP8 = mybir.dt.float8e4
F8 = ml_dtypes.float8_e4m3fn
BF = ml_dtypes.bfloat16
AOP = mybir.AluOpType
DR = mybir.MatmulPerfMode.DoubleRow
LNA = float(np.log(256.0))   # attn = exp(s - ln256): max exp ~208 < fp8 448

# scores/AV chunk grouping over the 17 t-chunks, aligned to PSUM banks.
TGROUPS = [(0, 8), (8, 16), (16, 17)]
WARMN = 6     # warm-filler matmuls (512 cols each) before first scores


def _bcast(ap, n, axis_insert=1):
    """Insert a stride-0 dim of size n into an AP (middle broadcast)."""
    new_ap = list(ap.ap)
    new_ap.insert(axis_insert, [0, n])
    return bass.AP(tensor=ap.tensor, offset=ap.offset, ap=new_ap)


def _build_nc(apply_affine=True):
    nc = bass.Bass()
    io = {}
    io["qT2"] = nc.dram_tensor("qT2", [128, 2, ND, SL], FP8, kind="ExternalInput")
    io["kT"] = nc.dram_tensor("kT", [128, ND, SLAB], FP8, kind="ExternalInput")
    io["v_r"] = nc.dram_tensor("v_r", [NTC, 128, D], FP8, kind="ExternalInput")
    io["maskT"] = nc.dram_tensor("maskT", [128, NTC, SL], FP8, kind="ExternalInput")
    io["w1T"] = nc.dram_tensor("w1T", [128, ND, DFF], FP8, kind="ExternalInput")
    # w2 dc-major so dc-halves are contiguous 2KB-per-partition DMAs
    io["w2T"] = nc.dram_tensor("w2T", [128, ND, NM, 128], FP8, kind="ExternalInput")
    # wtT carries a 513th column per d-chunk: the Wt column sums (for the
    # early mean path  sum_d tgt2_raw = wtcol . ctxrT)
    io["wtT"] = nc.dram_tensor("wtT", [128, ND, D + 32], FP8,
                               kind="ExternalInput")
    # wext row pairs: [0:NM] = ([w1sum_fc],[8*b1eff_fc]); [NM:NM+ND] =
    # ([8*b2_dc],[0])
    io["wext"] = nc.dram_tensor("wext", [1, NM + ND, 2, 128], FP8,
                                kind="ExternalInput")
    # rows: [tgtb_rowsum ; tgtb_sq_rowsum] f32
    io["rows"] = nc.dram_tensor("rows", [1, 2, SL], F32, kind="ExternalInput")
    io["tgtbT"] = nc.dram_tensor("tgtbT", [128, ND, SL], BF16, kind="ExternalInput")
    if apply_affine:
        for nm in ("g2v", "be2v", "g3v", "be3v"):
            io[nm] = nc.dram_tensor(nm, [D], F32, kind="ExternalInput")
    out_h = nc.dram_tensor("out", [SL, D], F32, kind="ExternalOutput")
    import os as _os
    _dbg = bool(_os.environ.get("KDBG"))
    if _dbg:
        io_dbg = {
            "d_mu": nc.dram_tensor("d_mu", [1, SL], F32, kind="ExternalOutput"),
            "d_std": nc.dram_tensor("d_std", [1, SL], F32, kind="ExternalOutput"),
            "d_rrec": nc.dram_tensor("d_rrec", [1, SL], F32, kind="ExternalOutput"),
            "d_x1": nc.dram_tensor("d_x1", [128, ND, SL], F32, kind="ExternalOutput"),
            "d_h": nc.dram_tensor("d_h", [128, NM, SL], F32, kind="ExternalOutput"),
            "d_o2": nc.dram_tensor("d_o2", [128, ND, SL], F32, kind="ExternalOutput"),
            "d_xhat": nc.dram_tensor("d_xhat", [SL, D], F32, kind="ExternalOutput"),
            "d_x2": nc.dram_tensor("d_x2", [SL, D], F32, kind="ExternalOutput"),
        }

    with tile.TileContext(nc) as tc:
        with (
            tc.tile_pool(name="cst", bufs=1) as cst,
            tc.tile_pool(name="ps", bufs=1, space="PSUM") as psp,
        ):
            # ---- SBUF tiles
            qT2 = cst.tile([128, 2, ND, SL], FP8, tag="qT2")
            kT = cst.tile([128, ND, SLAB], FP8, tag="kT")
            v_sb = cst.tile([128, NTC, D], FP8, tag="v")
            maskT = cst.tile([128, NTC, SL], FP8, tag="maskT")
            wtT = cst.tile([128, ND, D + 32], FP8, tag="wt")
            w1T = cst.tile([128, ND, DFF], FP8, tag="w1")
            w2T = cst.tile([128, ND, NM, 128], FP8, tag="w2")
            wext = cst.tile([1, NM + ND, 2, 128], FP8, tag="wext")
            rows = cst.tile([1, 2, SL], F32, tag="rows")
            tgtbT = cst.tile([128, ND, SL], BF16, tag="tgtbT")

            # ---- DMA issue order == consumption order (single HWDGE ring).
            def kdma(gi):
                t0, t1 = TGROUPS[gi]
                nc.sync.dma_start(out=kT[:, :, t0 * 128:t1 * 128],
                                  in_=io["kT"][:][:, :, t0 * 128:t1 * 128])

            def vdma(gi):
                t0, t1 = TGROUPS[gi]
                nc.sync.dma_start(
                    out=v_sb[:, t0:t1, :],
                    in_=io["v_r"][t0:t1].rearrange("c p d -> p c d"))

            kdma(0)
            nc.sync.dma_start(out=qT2, in_=io["qT2"][:])
            nc.sync.dma_start(out=maskT, in_=io["maskT"][:])
            kdma(1)
            vdma(0)
            kdma(2)
            vdma(2)
            vdma(1)
            nc.sync.dma_start(out=wtT, in_=io["wtT"][:])
            nc.sync.dma_start(out=tgtbT, in_=io["tgtbT"][:])
            nc.sync.dma_start(out=rows, in_=io["rows"][:])
            nc.sync.dma_start(out=wext, in_=io["wext"][:])
            nc.sync.dma_start(out=w1T[:, :, 0:1024], in_=io["w1T"][:][:, :, 0:1024])
            nc.sync.dma_start(out=w1T[:, :, 1024:2048],
                              in_=io["w1T"][:][:, :, 1024:2048])
            # w2 dc-major: [dc0-2] then [dc3] so the final DMA gates only the
            # last quarter of FFN2 + the LN2 tail
            nc.sync.dma_start(out=w2T[:, 0:3], in_=io["w2T"][:][:, 0:3])
            nc.sync.dma_start(out=w2T[:, 3:4], in_=io["w2T"][:][:, 3:4])
            bvec = {}
            if apply_affine:
                for nm in ("g2v", "be2v", "g3v", "be3v"):
                    bvec[nm] = cst.tile([SL, D], F32, tag=nm, name=nm + "_b")
                    src = io[nm][:]
                    bcast = bass.AP(tensor=src.tensor, offset=src.offset,
                                    ap=[[0, SL]] + list(src.ap))
                    nc.gpsimd.dma_start(out=bvec[nm], in_=bcast)

            # ---- constants
            onesc = cst.tile([128, 1], FP8, tag="onesc")
            nc.vector.memset(onesc, 1.0)
            ones21 = cst.tile([128, 2, 32], FP8, tag="ones21")
            nc.vector.memset(ones21.rearrange("p a b -> p (a b)"), 1.0)
            onesb = cst.tile([128, 1], BF16, tag="onesb")
            nc.vector.memset(onesb, 1.0)
            twosb = cst.tile([128, 1], BF16, tag="twosb")
            nc.vector.memset(twosb, 2.0)
            ones_rf = cst.tile([1, 128], F32, tag="ones_rf")
            nc.vector.memset(ones_rf, 1.0)
            epsc1 = cst.tile([1, 1], F32, tag="epsc1")
            nc.vector.memset(epsc1, 1e-5)
            epsc64 = cst.tile([1, 1], F32, tag="epsc64")
            nc.vector.memset(epsc64, 1e-5 / 64.0)
            epsc = cst.tile([SL, 1], F32, tag="eps")
            nc.vector.memset(epsc, 1e-5)
            expb = cst.tile([128, 1], F32, tag="expb")
            nc.vector.memset(expb, -LNA)
            identf1 = cst.tile([1, 1], F32, tag="identf1")
            nc.vector.memset(identf1, 1.0)
            invD_row = cst.tile([1, SL], F32, tag="invD_row")
            nc.vector.memset(invD_row, 1.0 / D)
            negD_row = cst.tile([1, SL], F32, tag="negD_row")
            nc.vector.memset(negD_row, -float(D))
            identf = cst.tile([128, 128], F32, tag="identf")
            make_identity(nc, identf)
            identb = cst.tile([128, 128], BF16, tag="identb")
            make_identity(nc, identb)

            # ---- PSUM tiles (8 banks; see header notes)
            ps_sc = [
                psp.tile([128, 8, SL], F32, tag="scA", name="ps_scA"),
                psp.tile([128, 8, SL], F32, tag="scB", name="ps_scB"),
                psp.tile([128, 1, SL], F32, tag="sm", name="ps_scC"),
            ]
            ps_ctxT = psp.tile([128, ND, SL], F32, tag="med", name="ps_ctxT")
            ps_h = [
                psp.tile([128, 8, SL], F32, tag="hA", name="ps_hA"),
                psp.tile([128, 8, SL], F32, tag="hB", name="ps_hB"),
            ]
            ps_t2T = psp.tile([128, ND, SL], F32, tag="t2T", name="ps_t2T")
            ps_r = psp.tile([1, SL], F32, tag="aux", name="ps_r")
            ps_rb = psp.tile([128, SL], F32, tag="aux", name="ps_rb")
            ps_stat = psp.tile([1, 192], F32, tag="sm", name="ps_stat")

            # warm fillers: 512-col zero matmuls into the hA bank (untouched
            # until FFN1-A; groups closed immediately, WAW-safe).
            wzero = cst.tile([SL, 512], BF16, tag="wzero")
            nc.vector.memset(wzero.rearrange("p f -> p f"), 0.0)
            warm_out = ps_h[0].rearrange("p c s -> p (c s)")[0:SL, :]

            def warm(n):
                for _ in range(n):
                    nc.tensor.matmul(warm_out, lhsT=wzero[:, 0:SL], rhs=wzero,
                                     start=True, stop=True,
                                     skip_group_check=True)

            warm(3)

            # ---- attention: scoresT (kT chunks stationary, q hi/lo moving,
            # DoubleRow over dc pairs) -> +mask (DVE) -> exp hi fp8 + exp bf16
            # (ACT, bias -ln32) -> lo = bf - hi (DVE) -> AV (DoubleRow over tc
            # pairs, v stationary) with attn row sums via ones DR matmuls.
            # attn is SINGLE fp8 (exp writes fp8 directly): r is computed from
            # the same quantized attn, so the softmax normalization stays
            # exact and only the weighting carries the fp8 noise (modeled
            # end-to-end rel err 0.0157 < 2e-2 gate).
            attn_hi = cst.tile([128, NTC, SL], FP8, tag="attn_hi")

            def sc_group(gi):
                t0, t1 = TGROUPS[gi]
                ps = ps_sc[gi]
                for tcn in range(t0, t1):
                    k = 0
                    for hv in range(2):
                        for dcp in range(0, ND, 2):
                            nc.tensor.matmul(
                                ps[:, tcn - t0, :],
                                lhsT=kT[:, dcp:dcp + 2, tcn * 128:(tcn + 1) * 128],
                                rhs=qT2[:, hv, dcp:dcp + 2, :],
                                start=(k == 0), stop=(k == 3),
                                perf_mode=DR,
                            )
                            k += 1
                nc.vector.tensor_add(ps[:, 0:t1 - t0, :], ps[:, 0:t1 - t0, :],
                                     maskT[:, t0:t1, :])
                nc.scalar.activation(out=attn_hi[:, t0:t1, :],
                                     in_=ps[:, 0:t1 - t0, :],
                                     func=mybir.ActivationFunctionType.Exp,
                                     bias=expb, scale=1.0)

            def av_group(gi, first, last):
                t0, t1 = TGROUPS[gi]
                if t1 - t0 == 8:
                    for tcp in range(t0, t1, 2):
                        for dc in range(ND):
                            nc.tensor.matmul(
                                ps_ctxT[:, dc, :],
                                lhsT=v_sb[:, tcp:tcp + 2,
                                          dc * 128:(dc + 1) * 128],
                                rhs=attn_hi[:, tcp:tcp + 2, :],
                                start=(first and tcp == t0 and dc == 0),
                                stop=(last and tcp == t1 - 2 and dc == ND - 1),
                                perf_mode=DR,
                                skip_group_check=True,
                            )
                        nc.tensor.matmul(
                            ps_r, lhsT=ones21[:, :, 0:1],
                            rhs=attn_hi[:, tcp:tcp + 2, :],
                            start=(first and tcp == t0),
                            stop=(last and tcp == t1 - 2),
                            perf_mode=DR,
                            skip_group_check=True,
                        )
                else:  # single chunk: plain fp8 matmuls
                    for dc in range(ND):
                        nc.tensor.matmul(
                            ps_ctxT[:, dc, :],
                            lhsT=v_sb[:, t0, dc * 128:(dc + 1) * 128],
                            rhs=attn_hi[:, t0, :],
                            start=(first and dc == 0),
                            stop=(last and dc == ND - 1),
                            skip_group_check=True,
                        )
                    nc.tensor.matmul(
                        ps_r, lhsT=onesc, rhs=attn_hi[:, t0, :],
                        start=first, stop=last,
                        skip_group_check=True,
                    )

            # av order [0, 2, 1]: v1 is the LAST v transfer, so av_group(1)
            # closes the ctx/r accumulation; the small g2 tail (mask2/exp2/
            # av2) hides under the v1 transfer.
            warm(4)
            sc_group(0)
            sc_group(1)
            sc_group(2)
            av_group(0, True, False)
            av_group(2, False, False)
            av_group(1, False, True)

            # r^-1 row first on DVE (only needs ps_r), then ctx lo
            rrec = cst.tile([1, SL], F32, tag="rrec")
            nc.vector.reciprocal(out=rrec, in_=ps_r)
            # partition broadcast of r^-1 (K=1 fp32 outer) + sbuf copy
            nc.tensor.matmul(ps_rb, lhsT=ones_rf, rhs=rrec,
                             start=True, stop=True, skip_group_check=True)
            rb_sb = cst.tile([128, SL], F32, tag="rb_sb")
            nc.vector.tensor_copy(out=rb_sb, in_=ps_rb)
            rb_bc = _bcast(rb_sb[:], ND)

            # ctx stays UN-normalized (r^-1 column scaling commutes through
            # Wt and folds into x1): ctx_hi = fp8 relu straight off psum on
            # ACT; ctx_lo = second psum read on DVE, overlapping the hi DRs
            ctx_hi = cst.tile([128, ND, SL], FP8, tag="ctx_hi")
            ctx_lo = cst.tile([128, ND, SL], FP8, tag="ctx_lo")
            nc.vector.tensor_scalar_max(
                ctx_hi.rearrange("p c s -> p (c s)"),
                ps_ctxT.rearrange("p c s -> p (c s)"), 0.0)
            nc.vector.scalar_tensor_tensor(
                out=ctx_lo.rearrange("p c s -> p (c s)"),
                in0=ps_ctxT.rearrange("p c s -> p (c s)"),
                scalar=0.0,
                in1=ctx_hi.rearrange("p c s -> p (c s)"),
                op0=AOP.max, op1=AOP.subtract)

            # early mean path: S1 = wtcol . ctxn = sum_d tgt2T
            k = 0
            for hv, ctx in ((0, ctx_hi), (1, ctx_lo)):
                for dcp in range(0, ND, 2):
                    nc.tensor.matmul(ps_stat[:, 0:SL],
                                     lhsT=wtT[:, dcp:dcp + 2, D:D + 1],
                                     rhs=ctx[:, dcp:dcp + 2, :],
                                     start=(k == 0), stop=(k == 3),
                                     perf_mode=DR, skip_group_check=True)
                    k += 1

            # tgt2T [dout, s] = Wt @ relu(ctx) (raw): each oc group is
            # CONTIGUOUS (only one accumulation group may be open per bank)
            for oc in range(ND):
                k = 0
                for hv, ctx in ((0, ctx_hi), (1, ctx_lo)):
                    for dcp in range(0, ND, 2):
                        nc.tensor.matmul(
                            ps_t2T[:, oc, :],
                            lhsT=wtT[:, dcp:dcp + 2, oc * 128:(oc + 1) * 128],
                            rhs=ctx[:, dcp:dcp + 2, :],
                            start=(k == 0), stop=(k == 3),
                            perf_mode=DR,
                        )
                        k += 1

            # x1 = tgt2T*r^-1 + tgtb: ps_t2T is read ONCE (psum reads of one
            # bank serialize across engines); hi fp8 from x1s on DVE, f32 on
            # Pool (parallel), lo + Square(x1) after
            x1s = cst.tile([128, ND, SL], F32, tag="x1s")
            nc.vector.tensor_mul(x1s, ps_t2T, rb_bc)
            x1hi = cst.tile([128, ND, SL], FP8, tag="x1hi")
            nc.vector.tensor_add(x1hi.rearrange("p c s -> p (c s)"),
                                 x1s.rearrange("p c s -> p (c s)"),
                                 tgtbT.rearrange("p c s -> p (c s)"))
            x1Tf = cst.tile([128, ND, SL], F32, tag="x1Tf")
            nc.gpsimd.tensor_add(x1Tf.rearrange("p c s -> p (c s)"),
                                 x1s.rearrange("p c s -> p (c s)"),
                                 tgtbT.rearrange("p c s -> p (c s)"))
            x1lo = cst.tile([128, ND, SL], FP8, tag="x1lo")
            nc.vector.tensor_sub(x1lo.rearrange("p c s -> p (c s)"),
                                 x1Tf.rearrange("p c s -> p (c s)"),
                                 x1hi.rearrange("p c s -> p (c s)"))
            x1sq = cst.tile([128, ND, SL], BF16, tag="x1sq")
            nc.scalar.activation(out=x1sq.rearrange("p c s -> p (c s)"),
                                 in_=x1s.rearrange("p c s -> p (c s)"),
                                 func=mybir.ActivationFunctionType.Square)
            x1cr = cst.tile([128, ND, SL], BF16, tag="x1cr")
            nc.vector.tensor_mul(x1cr.rearrange("p c s -> p (c s)"),
                                 x1s.rearrange("p c s -> p (c s)"),
                                 tgtbT.rearrange("p c s -> p (c s)"))

            # mu algebra on Pool: S1 is RAW (unnormalized ctx), so
            # mu = (S1*r^-1)/D + tsum/D (host pre-divides rows[0] by D)
            s1n_row = cst.tile([1, SL], F32, tag="s1n_row")
            nc.vector.tensor_mul(s1n_row, ps_stat[:, 0:SL], rrec)
            mu_row = cst.tile([1, SL], F32, tag="mu_row")
            nc.vector.scalar_tensor_tensor(out=mu_row, in0=s1n_row,
                                           scalar=1.0 / D, in1=rows[0:1, 0, :],
                                           op0=AOP.mult, op1=AOP.add)
            musqD = cst.tile([1, SL], F32, tag="musqD")
            nc.vector.scalar_tensor_tensor(out=musqD, in0=mu_row,
                                           scalar=-float(D), in1=mu_row,
                                           op0=AOP.mult, op1=AOP.mult)
            cmb_row = cst.tile([1, SL], F32, tag="cmb_row")
            nc.vector.tensor_add(cmb_row, musqD, rows[0:1, 1, :])
            # mustd fp8 row pair: [-mu ; std/8] (fix outer rhs)
            mustd = cst.tile([1, 2, SL], FP8, tag="mustd")
            nc.gpsimd.tensor_scalar_mul(mustd[0:1, 0, :], mu_row, -1.0)

            # variance chain: varD = sum x1^2 - D mu^2 (stat var matmuls gate)
            for dc in range(ND):
                nc.tensor.matmul(ps_stat[:, SL:2 * SL], lhsT=onesb,
                                 rhs=x1sq[:, dc, :],
                                 start=(dc == 0), stop=False,
                                 skip_group_check=True)
            for dc in range(ND):
                nc.tensor.matmul(ps_stat[:, SL:2 * SL], lhsT=twosb,
                                 rhs=x1cr[:, dc, :],
                                 start=False, stop=(dc == ND - 1),
                                 skip_group_check=True)
            varD_row = cst.tile([1, SL], F32, tag="varD_row")
            nc.vector.tensor_add(varD_row, ps_stat[:, SL:2 * SL], cmb_row)
            # std/8 = sqrt(varD/(64 D) + eps/64) straight into the fp8 pair
            nc.scalar.activation(out=mustd[0:1, 1, :], in_=varD_row,
                                 func=mybir.ActivationFunctionType.Sqrt,
                                 bias=epsc64, scale=1.0 / (64.0 * D))
            # off-chain: f32 std / rstd for the residual scaling
            std_row = cst.tile([1, SL], F32, tag="std_row")
            nc.scalar.activation(out=std_row, in_=varD_row,
                                 func=mybir.ActivationFunctionType.Sqrt,
                                 bias=epsc1, scale=1.0 / D)
            rstd_row = cst.tile([1, SL], F32, tag="rstd_row")
            nc.vector.reciprocal(out=rstd_row, in_=std_row)

            # ---- FFN1: h = W1q @ (x1hi + x1lo) + [w1sum;8b1] (x) [-mu;std/8]
            h_hi = cst.tile([128, NM, SL], FP8, tag="h_hi")
            h_lo = cst.tile([128, NM, SL], FP8, tag="h_lo")

            def ffn1_fc(fc):
                # fix FIRST (start=True): the in-order PE stream then stalls
                # on mustd only once, at the head, instead of between every
                # fc group's matmuls
                nc.tensor.matmul(ps_h[fc // 8][:, fc % 8, :],
                                 lhsT=wext[:, fc, :, :],
                                 rhs=mustd,
                                 start=True, stop=False,
                                 perf_mode=DR)
                k = 0
                for hv, x1q in ((0, x1hi), (1, x1lo)):
                    for dcp in range(0, ND, 2):
                        nc.tensor.matmul(
                            ps_h[fc // 8][:, fc % 8, :],
                            lhsT=w1T[:, dcp:dcp + 2, fc * 128:(fc + 1) * 128],
                            rhs=x1q[:, dcp:dcp + 2, :],
                            start=False, stop=(k == 3),
                            perf_mode=DR,
                        )
                        k += 1

            # h_hi = fp8 relu straight off psum (ACT) so FFN2-hi can start
            # immediately; h_lo = second psum read (DVE), overlapping the
            # hi DRs on PE
            def h_group8(g):
                sl8 = slice(8 * g, 8 * g + 8)
                nc.scalar.activation(
                    out=h_hi[:, sl8, :],
                    in_=ps_h[g],
                    func=mybir.ActivationFunctionType.Relu)
                nc.vector.scalar_tensor_tensor(
                    out=h_lo[:, sl8, :].rearrange("p c s -> p (c s)"),
                    in0=ps_h[g].rearrange("p c s -> p (c s)"),
                    scalar=0.0,
                    in1=h_hi[:, sl8, :].rearrange("p c s -> p (c s)"),
                    op0=AOP.max, op1=AOP.subtract)

            for fc in range(8):
                ffn1_fc(fc)
            h_group8(0)
            for fc in range(8, 16):
                ffn1_fc(fc)
            h_group8(1)

            # off-chain transposes fill the PE stall while w2 streams in
            ps_x1 = psp.tile([SL, D], F32, tag="scA", name="ps_x1")
            for dc in range(ND):
                nc.tensor.transpose(ps_x1[:, dc * 128:(dc + 1) * 128],
                                    x1Tf[:, dc, :], identf)
            ps_mr = psp.tile([SL, 2], F32, tag="sm", name="ps_mr")
            nc.tensor.transpose(ps_mr[:, 0:1], mu_row, identf1)
            nc.tensor.transpose(ps_mr[:, 1:2], rstd_row, identf1)

            # ---- FFN2: one accumulation group per dc, each in its OWN psum
            # bank so all four can be open at once; the hi-operand DRs for
            # dc0-2 run before h_lo is even ready, the lo DRs + closes follow.
            # dc3 is gated by the final w2 DMA and has the shortest tail.
            ps_o2dc = [
                psp.tile([128, SL], F32, tag="med", name="ps_o2d0"),
                psp.tile([128, SL], F32, tag="t2T", name="ps_o2d1"),
                psp.tile([128, SL], F32, tag="hA", name="ps_o2d2"),
                psp.tile([128, SL], F32, tag="hB", name="ps_o2d3"),
            ]
            ps_o2 = psp.tile([SL, D], BF16, tag="aux", name="ps_o2")
            ps_o2b = psp.tile([SL, D // 2], BF16, tag="sm", name="ps_o2b")
            mustd_s = bass.AP(tensor=mustd.tensor, offset=mustd[0:1, 1, :].offset,
                              ap=[list(mustd.ap[0]), [0, 2], [1, SL]])

            def ffn2_hi(dc):
                for fcp in range(0, NM, 2):
                    nc.tensor.matmul(
                        ps_o2dc[dc],
                        lhsT=w2T[:, dc, fcp:fcp + 2, :],
                        rhs=h_hi[:, fcp:fcp + 2, :],
                        start=(fcp == 0), stop=False,
                        perf_mode=DR,
                        skip_group_check=True,
                    )

            def ffn2_lo_close(dc):
                for fcp in range(0, NM, 2):
                    nc.tensor.matmul(
                        ps_o2dc[dc],
                        lhsT=w2T[:, dc, fcp:fcp + 2, :],
                        rhs=h_lo[:, fcp:fcp + 2, :],
                        start=False, stop=False,
                        perf_mode=DR,
                        skip_group_check=True,
                    )
                nc.tensor.matmul(ps_o2dc[dc],
                                 lhsT=wext[:, NM + dc, :, :],
                                 rhs=mustd_s,
                                 start=False, stop=True,
                                 perf_mode=DR,
                                 skip_group_check=True)

            # xhat = rstd * (x1 - mu) row-major f32 (early: overlaps FFN2)
            mr_col = cst.tile([SL, 2], F32, tag="mr_col")
            nc.vector.tensor_copy(out=mr_col, in_=ps_mr)
            xhat = cst.tile([SL, D], F32, tag="xhat")
            nc.vector.tensor_scalar(out=xhat, in0=ps_x1,
                                    scalar1=mr_col[:, 0:1],
                                    scalar2=mr_col[:, 1:2],
                                    op0=AOP.subtract, op1=AOP.mult)
            if apply_affine:
                nc.vector.tensor_mul(xhat, xhat, bvec["g2v"])
                nc.vector.tensor_add(xhat, xhat, bvec["be2v"])

            o2Ts = cst.tile([128, ND, SL], BF16, tag="o2Ts")
            x2 = cst.tile([SL, D], F32, tag="x2")
            SD = nc.vector.BN_STATS_DIM
            st2 = cst.tile([SL, 4 * SD], F32, tag="st2")

            def trans_dc(dc):
                tgt = ps_o2[:, dc * 128:(dc + 1) * 128] if dc < 2 else \
                    ps_o2b[:, (dc - 2) * 128:(dc - 1) * 128]
                nc.tensor.transpose(tgt, o2Ts[:, dc, :], identb)

            def x2_bn_dc(dc):
                src = ps_o2[:, dc * 128:(dc + 1) * 128] if dc < 2 else \
                    ps_o2b[:, (dc - 2) * 128:(dc - 1) * 128]
                cols = slice(dc * 128, (dc + 1) * 128)
                nc.vector.scalar_tensor_tensor(out=x2[:, cols], in0=src,
                                               scalar=mr_col[:, 1:2],
                                               in1=xhat[:, cols],
                                               op0=AOP.mult, op1=AOP.add)
                nc.vector.bn_stats(out=st2[:, dc * SD:(dc + 1) * SD],
                                   in_=x2[:, cols])

            for dc in range(3):
                ffn2_hi(dc)
            for dc in range(3):
                ffn2_lo_close(dc)
            ffn2_hi(3)
            for dc in range(3):
                nc.scalar.activation(out=o2Ts[:, dc, :], in_=ps_o2dc[dc],
                                     func=mybir.ActivationFunctionType.Copy)
            for dc in range(3):
                trans_dc(dc)
            for dc in range(3):
                x2_bn_dc(dc)
            ffn2_lo_close(3)
            nc.scalar.activation(out=o2Ts[:, 3, :], in_=ps_o2dc[3],
                                 func=mybir.ActivationFunctionType.Copy)
            trans_dc(3)
            x2_bn_dc(3)
            mv2 = cst.tile([SL, nc.vector.BN_AGGR_DIM], F32, tag="mv2")
            nc.vector.bn_aggr(out=mv2, in_=st2)
            std2 = cst.tile([SL, 1], F32, tag="std2")
            nc.scalar.activation(out=std2, in_=mv2[:, 1:2],
                                 func=mybir.ActivationFunctionType.Sqrt,
                                 bias=epsc, scale=1.0)
            rstd2 = cst.tile([SL, 1], F32, tag="rstd2")
            nc.vector.reciprocal(out=rstd2, in_=std2)
            out_sb = cst.tile([SL, D], F32, tag="out")
            nc.vector.tensor_scalar(out=out_sb, in0=x2,
                                    scalar1=mv2[:, 0:1], scalar2=rstd2,
                                    op0=AOP.subtract, op1=AOP.mult)
            if apply_affine:
                nc.vector.tensor_mul(out_sb, out_sb, bvec["g3v"])
                nc.vector.tensor_add(out_sb, out_sb, bvec["be3v"])
            nc.sync.dma_start(out=out_h[:], in_=out_sb)
            if _dbg:
                nc.sync.dma_start(out=io_dbg["d_mu"][:], in_=mu_row)
                nc.sync.dma_start(out=io_dbg["d_std"][:], in_=std_row)
                nc.sync.dma_start(out=io_dbg["d_rrec"][:], in_=rrec)
                dx1 = cst.tile([128, ND, SL], F32, tag="dx1")
                nc.vector.tensor_add(dx1.rearrange("p c s -> p (c s)"),
                                     x1hi.rearrange("p c s -> p (c s)"),
                                     x1lo.rearrange("p c s -> p (c s)"))
                nc.sync.dma_start(out=io_dbg["d_x1"][:], in_=dx1)
                dh = cst.tile([128, NM, SL], F32, tag="dh")
                nc.vector.tensor_add(dh.rearrange("p c s -> p (c s)"),
                                     h_hi.rearrange("p c s -> p (c s)"),
                                     h_lo.rearrange("p c s -> p (c s)"))
                nc.sync.dma_start(out=io_dbg["d_h"][:], in_=dh)
                do2 = cst.tile([128, ND, SL], F32, tag="do2")
                nc.vector.tensor_copy(out=do2.rearrange("p c s -> p (c s)"),
                                      in_=o2Ts.rearrange("p c s -> p (c s)"))
                nc.sync.dma_start(out=io_dbg["d_o2"][:], in_=do2)
                nc.sync.dma_start(out=io_dbg["d_xhat"][:], in_=xhat)
                nc.sync.dma_start(out=io_dbg["d_x2"][:], in_=x2)

    _split_multi_waits(nc)
    return nc


_NC_CACHE = {}


def _f8(x):
    return np.asarray(x, np.float32).astype(F8)


def _prep_inputs(tgt, memory, pos, query_pos, action_idx,
                 W_tgt2, b_tgt2, W1, b1, W2, b2, g2, be2, g3, be3):
    inv = np.float32(1.0 / np.sqrt(D))
    tgt2d = np.ascontiguousarray(tgt[:, 0, :], np.float32)        # [S, D]
    qp2d = np.ascontiguousarray(query_pos[:, 0, :], np.float32)
    mem2d = np.ascontiguousarray(memory[:, 0, :], np.float32)     # [T, D]
    pos2d = np.ascontiguousarray(pos[:, 0, :], np.float32)

    k2d = mem2d + pos2d
    k_p = np.zeros((T + 2 * HALO, D), np.float32)
    k_p[HALO:HALO + T] = k2d
    mem_p = np.zeros((T + 2 * HALO, D), np.float32)
    mem_p[HALO:HALO + T] = mem2d
    q2d = (tgt2d + qp2d) * inv                                    # [S, D]

    # segment ids from action_idx change points (mirrors the reference mask)
    ai = np.asarray(action_idx)
    change = np.concatenate([[0], (ai[1:] != ai[:-1]).astype(np.int64)])
    seg_id = np.cumsum(change)

    aff = _needs_affine(g2, be2, g3, be3)
    W1f = np.asarray(W1, np.float32)
    b1f = np.asarray(b1, np.float32)
    if aff:
        # fold g2/be2 into FFN1: h1 = (x^)@ (W1*g2).T + (b1 + W1@be2)
        W1eff = W1f * np.asarray(g2, np.float32)[None, :]
        b1eff = b1f + W1f @ np.asarray(be2, np.float32)
    else:
        W1eff, b1eff = W1f, b1f

    w1T_h = np.ascontiguousarray(
        W1eff.T.reshape(ND, 128, DFF).transpose(1, 0, 2)).astype(F8)
    # w2 dc-major: w2T[p, dc, fc, j] = W2[dc*128+j, fc*128+p]
    w2T_h = np.ascontiguousarray(
        np.asarray(W2, np.float32).T.reshape(NM, 128, ND, 128)
        .transpose(1, 2, 0, 3)).astype(F8)
    wtT_q = np.ascontiguousarray(
        np.asarray(W_tgt2, np.float32).T.reshape(ND, 128, D)
        .transpose(1, 0, 2)).astype(F8)
    # 513th column per d-chunk: Wt column sums (of the quantized weights)
    wtcol = np.asarray(wtT_q, np.float32).sum(axis=2)              # [128, ND]
    wtT_h = np.zeros((128, ND, D + 32), np.float32)
    wtT_h[:, :, 0:D] = np.asarray(wtT_q, np.float32)
    wtT_h[:, :, D] = wtcol
    wtT_h = np.ascontiguousarray(wtT_h.astype(F8))
    # wext pairs: fc rows ([w1sum_fc],[8*b1eff_fc]); dc rows ([8*b2_dc],[0])
    w1sum = np.asarray(w1T_h, np.float32).sum(axis=0).sum(axis=0)  # [DFF]
    b2f = np.asarray(b2, np.float32)
    wext_h = np.zeros((1, NM + ND, 2, 128), np.float32)
    wext_h[0, 0:NM, 0, :] = w1sum.reshape(NM, 128)
    wext_h[0, 0:NM, 1, :] = 8.0 * b1eff.reshape(NM, 128)
    wext_h[0, NM:NM + ND, 0, :] = 8.0 * b2f.reshape(ND, 128)
    wext_h = np.ascontiguousarray(wext_h).astype(F8)

    in_maps = []
    for c in range(NCORES):
        sl = slice(c * SL, (c + 1) * SL)
        qc = q2d[sl].T.reshape(ND, 128, SL).transpose(1, 0, 2)     # [128,ND,SL]
        q_hi = qc.astype(F8)
        q_lo = (qc - q_hi.astype(np.float32)).astype(F8)
        qT2c = np.ascontiguousarray(
            np.stack([np.asarray(q_hi), np.asarray(q_lo)], axis=1))
        kslab = k_p[c * TSH:c * TSH + SLAB]                       # [2176, D]
        kTc = kslab.T.reshape(ND, 128, SLAB).transpose(1, 0, 2).astype(F8)
        v_h = mem_p[c * TSH:c * TSH + SLAB].reshape(NTC, 128, D).astype(F8)

        # additive band mask in T layout [128, NTC, SL]: 0 where query j
        # (global s=64c+j) attends slab frame t, else -60; pad rows stay -60.
        mk = np.full((SL, SLAB), -60.0, np.float32)
        g0 = c * TSH - HALO
        glo, ghi = max(0, g0), min(T, g0 + SLAB)
        if ghi > glo:
            seg = seg_id[glo:ghi]
            svec = np.arange(c * SL, (c + 1) * SL)
            ok = (np.abs(seg[None, :] - svec[:, None]) <= 1)
            mk[:, glo - g0:ghi - g0][ok] = 0.0
        mkT = np.ascontiguousarray(
            mk.T.reshape(NTC, 128, SL).transpose(1, 0, 2)).astype(F8)

        tgtb = (tgt2d[sl] + np.asarray(b_tgt2, np.float32)).astype(BF)
        tgtbT = np.ascontiguousarray(
            tgtb.T.reshape(ND, 128, SL).transpose(1, 0, 2))
        tgtbf = tgtb.astype(np.float32)
        # rows[0] = tsum/D (pre-divided for the fused device-side mu stt)
        rows_h = np.stack([tgtbf.sum(axis=1) / D,
                           (tgtbf * tgtbf).sum(axis=1)]).reshape(1, 2, SL)
        rows_h = np.ascontiguousarray(rows_h, np.float32)

        im = {
            "qT2": qT2c,
            "kT": np.ascontiguousarray(kTc),
            "v_r": np.ascontiguousarray(v_h),
            "maskT": mkT,
            "w1T": w1T_h,
            "w2T": w2T_h,
            "wtT": wtT_h,
            "wext": wext_h,
            "rows": rows_h,
            "tgtbT": tgtbT,
        }
        if aff:
            im.update({
                "g2v": np.asarray(g2, np.float32),
                "be2v": np.asarray(be2, np.float32),
                "g3v": np.asarray(g3, np.float32),
                "be3v": np.asarray(be3, np.float32),
            })
        in_maps.append(im)
    return in_maps


def _needs_affine(g2, be2, g3, be3):
    return not (np.all(np.asarray(g2) == 1) and np.all(np.asarray(g3) == 1)
                and np.all(np.asarray(be2) == 0) and np.all(np.asarray(be3) == 0))


_LAST = {}


def kernel(**inputs) -> np.ndarray:
    inputs = {k: np.asarray(v) for k, v in inputs.items()}
    aff = _needs_affine(inputs["g2"], inputs["be2"], inputs["g3"], inputs["be3"])
    if aff not in _NC_CACHE:
        _NC_CACHE[aff] = _build_nc(apply_affine=aff)
    nc = _NC_CACHE[aff]
    in_maps = _prep_inputs(**inputs)
    import os
    kw = {}
    if os.environ.get("BASS_TRACE"):
        kw = dict(trace=True, tmpdir=os.environ.get("BASS_TRACE_DIR") or None)
    res = run_bass_kernel_spmd(nc, in_maps, core_ids=list(range(NCORES)), **kw)
    _LAST["res"] = res
    out = np.concatenate([res.results[c]["out"] for c in range(NCORES)], axis=0)
    return np.ascontiguousarray(out.reshape(S, 1, D).astype(np.float32))


# revision 52
# speedup vs baseline: 1.0615x; 1.0082x over previous
# Trainium2 Bass kernel for nn_CrossAttention_6579889897579 (sparse segment-
# neighbor cross-attention + FFN block).
#
# Sharding: the S=512 queries map 1:1 onto 512 contiguous 32-frame segments of
# the T=16384 memory (action_idx encodes the segmentation; seg boundaries are
# recomputed from it on the host). Query s attends segments {s-1,s,s+1} =
# frames [32s-32, 32s+64). Sharding S across 8 cores (64 queries/core) makes
# attention block-local: core c only needs frames [2048c-64, 2048c+2112) (a
# 2176-frame slab, zero-padded at the global edges). No collectives.
#
# v4 design notes (cost-model-driven; v3 was 27437ns):
# - DMA is the serialized bottleneck (360 GB/s aggregate, one transfer at a
#   time), so the big streams (k, v, mask, weights) travel as fp8 e4m3.
# - All large matmuls use fp8 DoubleRow perf mode (two 128-deep fp8 matmuls
#   summed per instruction at 0.5 cycles/row = 4x bf16 throughput). Moving
#   operands (q, attn, relu(ctx), x1, h) are SPLIT fp8: hi = f8(x),
#   lo = f8(x - hi); hi+lo restores ~bf16 accuracy (verified: end-to-end rel
#   err 0.0149 == bf16 baseline), while each half streams at fp8 DR speed.
# - attn is stored as exp(scores - ln32) (fp8 range safety); the 1/32 scale
#   cancels exactly through the r = sum(attn) normalization.
# - FFN1 runs directly on quantized x1 (not x1-mu): h = W1@x1q + [w1sum;b1]
#   K-pair fix outer with rhs [-mu; std/8] (w1sum = quantized-W1 row sums), so
#   the x1->FFN1 chain does not wait for the mean/var statistics.
# - w2 is stored dc-major ([128, ND, NM, 128]) and DMAed in two dc-halves so
#   the final DMA only gates the last quarter of FFN2 + LN2 tail.
# - PSUM rules: (a) at most ONE matmul accumulation group open per 2KB PSUM
#   bank, (b) a group OVERWRITES its region when it closes, (c) pipeline
#   stages that overlap in time use separate tiles.
# - PE p-state ramps 0.65->1.2->2.4GHz with sustained-busy time and resets on
#   idle; warm-filler matmuls spin it up while the first k chunk streams in.
# - DMA issue order == consumption order (single HWDGE ring, 625ns per issue).
import sys

sys.path.insert(0, "/opt/trn_rl_repo")

import numpy as np
import ml_dtypes

import concourse.bass as bass
import concourse.mybir as mybir
import concourse.tile as tile
from concourse.bass_utils import run_bass_kernel_spmd
from concourse.masks import make_identity

# ---- Workaround: neuronxcc walrus rejects any instruction carrying more than
# one semaphore wait ("Too many sync wait commands"). Two pieces: (1) the Tile
# tail drain gets its waits split onto single-wait sync NOPs; (2) a post-pass
# splits multi-wait body instructions the same way.
import concourse.mybir as _mybir
from bass_rust import ScopedClock as _ScopedClock


def _drain_and_barrier(self, tick_clock, wait_clock):
    probe = self.nc.sync.nop(nofuse=True, hint="tail_wait_probe")
    wait_clock.add_sem_waits(probe.ins, _ScopedClock({None: tick_clock.global_clock}))
    waits = list(probe.ins.sync_info.on_wait)
    if waits:
        probe.ins.sync_info.on_wait = [waits[0]]
        for w in waits[1:]:
            n2 = self.nc.sync.nop(nofuse=True, hint="tail_wait_split")
            n2.ins.sync_info = _mybir.SyncInfo(on_wait=[w], on_update=[])
    self.nc.sync.drain()
    self.nc.all_engine_barrier()
    assert self.sems is not None
    popped = self.nc._tile_sem_poison_stack.pop()
    assert popped is self._sem_poison
    self.nc.clear_and_free_semaphores(list(self.sems.allocated().values()))
    self.nc.all_engine_barrier()


tile.TileContext._drain_and_barrier = _drain_and_barrier


def _split_multi_waits(nc, max_waits=1):
    uid = [0]
    for f in nc.m.functions:
        for bb in f.blocks:
            out = []
            for inst in bb.instructions:
                si = getattr(inst, "sync_info", None)
                if si is not None and si.on_wait and len(si.on_wait) > max_waits:
                    waits = list(si.on_wait)
                    for w in waits[:-max_waits]:
                        uid[0] += 1
                        nop = _mybir.InstNoOp(
                            name=f"I-waitsplit-{uid[0]}",
                            engine=inst.engine,
                            bass_nofuse=True,
                            ins=[], outs=[],
                            sync_info=_mybir.SyncInfo(on_wait=[w], on_update=[]),
                        )
                        out.append(nop)
                    inst.sync_info = _mybir.SyncInfo(
                        on_wait=waits[-max_waits:], on_update=list(si.on_update)
                    )
                out.append(inst)
            bb.instructions = out


S, T, D, DFF = 512, 16384, 512, 2048
NCORES = 8
SL = S // NCORES          # 64 queries per core
TSH = T // NCORES         # 2048 frames per core
HALO = 64
SLAB = TSH + 2 * HALO     # 2176 = 17 * 128
NTC = SLAB // 128         # 17 t-chunks
ND = D // 128             # 4 d-chunks
NM = DFF // 128           # 16 dff-chunks
F32 = mybir.dt.float32
BF16 = mybir.dt.bfloat16
FP8 = mybir.dt.float8e4
F8 = ml_dtypes.float8_e4m3fn
BF = ml_dtypes.bfloat16
AOP = mybir.AluOpType
DR = mybir.MatmulPerfMode.DoubleRow
LNA = float(np.log(256.0))   # attn = exp(s - ln256): max exp ~208 < fp8 448

# scores/AV chunk grouping over the 17 t-chunks, aligned to PSUM banks.
TGROUPS = [(0, 8), (8, 16), (16, 17)]
WARMN = 6     # warm-filler matmuls (512 cols each) before first scores


def _bcast(ap, n, axis_insert=1):
    """Insert a stride-0 dim of size n into an AP (middle broadcast)."""
    new_ap = list(ap.ap)
    new_ap.insert(axis_insert, [0, n])
    return bass.AP(tensor=ap.tensor, offset=ap.offset, ap=new_ap)


def _build_nc(apply_affine=True):
    nc = bass.Bass()
    io = {}
    io["qT2"] = nc.dram_tensor("qT2", [128, 2, ND, SL], FP8, kind="ExternalInput")
    io["kT"] = nc.dram_tensor("kT", [128, ND, SLAB], FP8, kind="ExternalInput")
    io["v_r"] = nc.dram_tensor("v_r", [NTC, 128, D], FP8, kind="ExternalInput")
    io["maskT"] = nc.dram_tensor("maskT", [128, NTC, SL], FP8, kind="ExternalInput")
    io["w1T"] = nc.dram_tensor("w1T", [128, ND, DFF], FP8, kind="ExternalInput")
    # w2 dc-major so dc-halves are contiguous 2KB-per-partition DMAs
    io["w2T"] = nc.dram_tensor("w2T", [128, ND, NM, 128], FP8, kind="ExternalInput")
    # wtT carries a 513th column per d-chunk: the Wt column sums (for the
    # early mean path  sum_d tgt2_raw = wtcol . ctxrT)
    io["wtT"] = nc.dram_tensor("wtT", [128, ND, D + 32], FP8,
                               kind="ExternalInput")
    # wext row pairs: [0:NM] = ([w1sum_fc],[8*b1eff_fc]); [NM:NM+ND] =
    # ([8*b2_dc],[0])
    io["wext"] = nc.dram_tensor("wext", [1, NM + ND, 2, 128], FP8,
                                kind="ExternalInput")
    # rows: [tgtb_rowsum ; tgtb_sq_rowsum] f32
    io["rows"] = nc.dram_tensor("rows", [1, 2, SL], F32, kind="ExternalInput")
    io["tgtbT"] = nc.dram_tensor("tgtbT", [128, ND, SL], BF16, kind="ExternalInput")
    if apply_affine:
        for nm in ("g2v", "be2v", "g3v", "be3v"):
            io[nm] = nc.dram_tensor(nm, [D], F32, kind="ExternalInput")
    out_h = nc.dram_tensor("out", [SL, D], F32, kind="ExternalOutput")
    import os as _os
    _dbg = bool(_os.environ.get("KDBG"))
    if _dbg:
        io_dbg = {
            "d_mu": nc.dram_tensor("d_mu", [1, SL], F32, kind="ExternalOutput"),
            "d_std": nc.dram_tensor("d_std", [1, SL], F32, kind="ExternalOutput"),
            "d_rrec": nc.dram_tensor("d_rrec", [1, SL], F32, kind="ExternalOutput"),
            "d_x1": nc.dram_tensor("d_x1", [128, ND, SL], F32, kind="ExternalOutput"),
            "d_h": nc.dram_tensor("d_h", [128, NM, SL], F32, kind="ExternalOutput"),
            "d_o2": nc.dram_tensor("d_o2", [128, ND, SL], F32, kind="ExternalOutput"),
            "d_xhat": nc.dram_tensor("d_xhat", [SL, D], F32, kind="ExternalOutput"),
            "d_x2": nc.dram_tensor("d_x2", [SL, D], F32, kind="ExternalOutput"),
        }

    with tile.TileContext(nc) as tc:
        with (
            tc.tile_pool(name="cst", bufs=1) as cst,
            tc.tile_pool(name="ps", bufs=1, space="PSUM") as psp,
        ):
            # ---- SBUF tiles
            qT2 = cst.tile([128, 2, ND, SL], FP8, tag="qT2")
            kT = cst.tile([128, ND, SLAB], FP8, tag="kT")
            v_sb = cst.tile([128, NTC, D], FP8, tag="v")
            maskT = cst.tile([128, NTC, SL], FP8, tag="maskT")
            wtT = cst.tile([128, ND, D + 32], FP8, tag="wt")
            w1T = cst.tile([128, ND, DFF], FP8, tag="w1")
            w2T = cst.tile([128, ND, NM, 128], FP8, tag="w2")
            wext = cst.tile([1, NM + ND, 2, 128], FP8, tag="wext")
            rows = cst.tile([1, 2, SL], F32, tag="rows")
            tgtbT = cst.tile([128, ND, SL], BF16, tag="tgtbT")

            # ---- DMA issue order == consumption order (single HWDGE ring).
            def kdma(gi):
                t0, t1 = TGROUPS[gi]
                nc.sync.dma_start(out=kT[:, :, t0 * 128:t1 * 128],
                                  in_=io["kT"][:][:, :, t0 * 128:t1 * 128])

            def vdma(gi):
                t0, t1 = TGROUPS[gi]
                nc.sync.dma_start(
                    out=v_sb[:, t0:t1, :],
                    in_=io["v_r"][t0:t1].rearrange("c p d -> p c d"))

            kdma(0)
            nc.sync.dma_start(out=qT2, in_=io["qT2"][:])
            nc.sync.dma_start(out=maskT, in_=io["maskT"][:])
            kdma(1)
            vdma(0)
            kdma(2)
            vdma(2)
            vdma(1)
            nc.sync.dma_start(out=wtT, in_=io["wtT"][:])
            nc.sync.dma_start(out=tgtbT, in_=io["tgtbT"][:])
            nc.sync.dma_start(out=rows, in_=io["rows"][:])
            nc.sync.dma_start(out=wext, in_=io["wext"][:])
            nc.sync.dma_start(out=w1T[:, :, 0:1024], in_=io["w1T"][:][:, :, 0:1024])
            nc.sync.dma_start(out=w1T[:, :, 1024:2048],
                              in_=io["w1T"][:][:, :, 1024:2048])
            # w2 dc-major: [dc0-2] then [dc3] so the final DMA gates only the
            # last quarter of FFN2 + the LN2 tail
            nc.sync.dma_start(out=w2T[:, 0:3], in_=io["w2T"][:][:, 0:3])
            nc.sync.dma_start(out=w2T[:, 3:4], in_=io["w2T"][:][:, 3:4])
            bvec = {}
            if apply_affine:
                for nm in ("g2v", "be2v", "g3v", "be3v"):
                    bvec[nm] = cst.tile([SL, D], F32, tag=nm, name=nm + "_b")
                    src = io[nm][:]
                    bcast = bass.AP(tensor=src.tensor, offset=src.offset,
                                    ap=[[0, SL]] + list(src.ap))
                    nc.gpsimd.dma_start(out=bvec[nm], in_=bcast)

            # ---- constants
            onesc = cst.tile([128, 1], FP8, tag="onesc")
            nc.vector.memset(onesc, 1.0)
            ones21 = cst.tile([128, 2, 32], FP8, tag="ones21")
            nc.vector.memset(ones21.rearrange("p a b -> p (a b)"), 1.0)
            onesb = cst.tile([128, 1], BF16, tag="onesb")
            nc.vector.memset(onesb, 1.0)
            twosb = cst.tile([128, 1], BF16, tag="twosb")
            nc.vector.memset(twosb, 2.0)
            ones_rf = cst.tile([1, 128], F32, tag="ones_rf")
            nc.vector.memset(ones_rf, 1.0)
            epsc1 = cst.tile([1, 1], F32, tag="epsc1")
            nc.vector.memset(epsc1, 1e-5)
            epsc64 = cst.tile([1, 1], F32, tag="epsc64")
            nc.vector.memset(epsc64, 1e-5 / 64.0)
            epsc = cst.tile([SL, 1], F32, tag="eps")
            nc.vector.memset(epsc, 1e-5)
            expb = cst.tile([128, 1], F32, tag="expb")
            nc.vector.memset(expb, -LNA)
            identf1 = cst.tile([1, 1], F32, tag="identf1")
            nc.vector.memset(identf1, 1.0)
            invD_row = cst.tile([1, SL], F32, tag="invD_row")
            nc.vector.memset(invD_row, 1.0 / D)
            negD_row = cst.tile([1, SL], F32, tag="negD_row")
            nc.vector.memset(negD_row, -float(D))
            identf = cst.tile([128, 128], F32, tag="identf")
            make_identity(nc, identf)
            identb = cst.tile([128, 128], BF16, tag="identb")
            make_identity(nc, identb)

            # ---- PSUM tiles (8 banks; see header notes)
            ps_sc = [
                psp.tile([128, 8, SL], F32, tag="scA", name="ps_scA"),
                psp.tile([128, 8, SL], F32, tag="scB", name="ps_scB"),
                psp.tile([128, 1, SL], F32, tag="sm", name="ps_scC"),
            ]
            ps_ctxT = psp.tile([128, ND, SL], F32, tag="med", name="ps_ctxT")
            ps_h = [
                psp.tile([128, 8, SL], F32, tag="hA", name="ps_hA"),
                psp.tile([128, 8, SL], F32, tag="hB", name="ps_hB"),
            ]
            ps_t2T = psp.tile([128, ND, SL], F32, tag="t2T", name="ps_t2T")
            ps_r = psp.tile([1, SL], F32, tag="aux", name="ps_r")
            ps_rb = psp.tile([128, SL], F32, tag="aux", name="ps_rb")
            ps_stat = psp.tile([1, 192], F32, tag="sm", name="ps_stat")

            # warm fillers: 512-col zero matmuls into the hA bank (untouched
            # until FFN1-A; groups closed immediately, WAW-safe).
            wzero = cst.tile([SL, 512], BF16, tag="wzero")
            nc.vector.memset(wzero.rearrange("p f -> p f"), 0.0)
            warm_out = ps_h[0].rearrange("p c s -> p (c s)")[0:SL, :]

            def warm(n):
                for _ in range(n):
                    nc.tensor.matmul(warm_out, lhsT=wzero[:, 0:SL], rhs=wzero,
                                     start=True, stop=True,
                                     skip_group_check=True)

            warm(3)

            # ---- attention: scoresT (kT chunks stationary, q hi/lo moving,
            # DoubleRow over dc pairs) -> +mask (DVE) -> exp hi fp8 + exp bf16
            # (ACT, bias -ln32) -> lo = bf - hi (DVE) -> AV (DoubleRow over tc
            # pairs, v stationary) with attn row sums via ones DR matmuls.
            # attn is SINGLE fp8 (exp writes fp8 directly): r is computed from
            # the same quantized attn, so the softmax normalization stays
            # exact and only the weighting carries the fp8 noise (modeled
            # end-to-end rel err 0.0157 < 2e-2 gate).
            attn_hi = cst.tile([128, NTC, SL], FP8, tag="attn_hi")

            def sc_group(gi):
                t0, t1 = TGROUPS[gi]
                ps = ps_sc[gi]
                for tcn in range(t0, t1):
                    k = 0
                    for hv in range(2):
                        for dcp in range(0, ND, 2):
                            nc.tensor.matmul(
                                ps[:, tcn - t0, :],
                                lhsT=kT[:, dcp:dcp + 2, tcn * 128:(tcn + 1) * 128],
                                rhs=qT2[:, hv, dcp:dcp + 2, :],
                                start=(k == 0), stop=(k == 3),
                                perf_mode=DR,
                            )
                            k += 1
                nc.vector.tensor_add(ps[:, 0:t1 - t0, :], ps[:, 0:t1 - t0, :],
                                     maskT[:, t0:t1, :])
                nc.scalar.activation(out=attn_hi[:, t0:t1, :],
                                     in_=ps[:, 0:t1 - t0, :],
                                     func=mybir.ActivationFunctionType.Exp,
                                     bias=expb, scale=1.0)

            def av_group(gi, first, last):
                t0, t1 = TGROUPS[gi]
                if t1 - t0 == 8:
                    for tcp in range(t0, t1, 2):
                        for dc in range(ND):
                            nc.tensor.matmul(
                                ps_ctxT[:, dc, :],
                                lhsT=v_sb[:, tcp:tcp + 2,
                                          dc * 128:(dc + 1) * 128],
                                rhs=attn_hi[:, tcp:tcp + 2, :],
                                start=(first and tcp == t0 and dc == 0),
                                stop=(last and tcp == t1 - 2 and dc == ND - 1),
                                perf_mode=DR,
                                skip_group_check=True,
                            )
                        nc.tensor.matmul(
                            ps_r, lhsT=ones21[:, :, 0:1],
                            rhs=attn_hi[:, tcp:tcp + 2, :],
                            start=(first and tcp == t0),
                            stop=(last and tcp == t1 - 2),
                            perf_mode=DR,
                            skip_group_check=True,
                        )
                else:  # single chunk: plain fp8 matmuls
                    for dc in range(ND):
                        nc.tensor.matmul(
                            ps_ctxT[:, dc, :],
                            lhsT=v_sb[:, t0, dc * 128:(dc + 1) * 128],
                            rhs=attn_hi[:, t0, :],
                            start=(first and dc == 0),
                            stop=(last and dc == ND - 1),
                            skip_group_check=True,
                        )
                    nc.tensor.matmul(
                        ps_r, lhsT=onesc, rhs=attn_hi[:, t0, :],
                        start=first, stop=last,
                        skip_group_check=True,
                    )

            # av order [0, 2, 1]: v1 is the LAST v transfer, so av_group(1)
            # closes the ctx/r accumulation; the small g2 tail (mask2/exp2/
            # av2) hides under the v1 transfer.
            warm(4)
            sc_group(0)
            sc_group(1)
            sc_group(2)
            av_group(0, True, False)
            av_group(2, False, False)
            av_group(1, False, True)

            # r^-1 row first on DVE (only needs ps_r), then ctx lo
            rrec = cst.tile([1, SL], F32, tag="rrec")
            nc.vector.reciprocal(out=rrec, in_=ps_r)
            # partition broadcast of r^-1 (K=1 fp32 outer) + sbuf copy
            nc.tensor.matmul(ps_rb, lhsT=ones_rf, rhs=rrec,
                             start=True, stop=True, skip_group_check=True)
            rb_sb = cst.tile([128, SL], F32, tag="rb_sb")
            nc.vector.tensor_copy(out=rb_sb, in_=ps_rb)
            rb_bc = _bcast(rb_sb[:], ND)

            # ctx stays UN-normalized (r^-1 column scaling commutes through
            # Wt and folds into x1): ctx_hi = fp8 relu straight off psum on
            # ACT; ctx_lo = second psum read on DVE, overlapping the hi DRs
            ctx_hi = cst.tile([128, ND, SL], FP8, tag="ctx_hi")
            ctx_lo = cst.tile([128, ND, SL], FP8, tag="ctx_lo")
            nc.vector.tensor_scalar_max(
                ctx_hi.rearrange("p c s -> p (c s)"),
                ps_ctxT.rearrange("p c s -> p (c s)"), 0.0)
            nc.vector.scalar_tensor_tensor(
                out=ctx_lo.rearrange("p c s -> p (c s)"),
                in0=ps_ctxT.rearrange("p c s -> p (c s)"),
                scalar=0.0,
                in1=ctx_hi.rearrange("p c s -> p (c s)"),
                op0=AOP.max, op1=AOP.subtract)

            # early mean path: S1 = wtcol . ctxn = sum_d tgt2T
            k = 0
            for hv, ctx in ((0, ctx_hi), (1, ctx_lo)):
                for dcp in range(0, ND, 2):
                    nc.tensor.matmul(ps_stat[:, 0:SL],
                                     lhsT=wtT[:, dcp:dcp + 2, D:D + 1],
                                     rhs=ctx[:, dcp:dcp + 2, :],
                                     start=(k == 0), stop=(k == 3),
                                     perf_mode=DR, skip_group_check=True)
                    k += 1

            # tgt2T [dout, s] = Wt @ relu(ctx) (raw): each oc group is
            # CONTIGUOUS (only one accumulation group may be open per bank)
            for oc in range(ND):
                k = 0
                for hv, ctx in ((0, ctx_hi), (1, ctx_lo)):
                    for dcp in range(0, ND, 2):
                        nc.tensor.matmul(
                            ps_t2T[:, oc, :],
                            lhsT=wtT[:, dcp:dcp + 2, oc * 128:(oc + 1) * 128],
                            rhs=ctx[:, dcp:dcp + 2, :],
                            start=(k == 0), stop=(k == 3),
                            perf_mode=DR,
                        )
                        k += 1

            # x1 = tgt2T*r^-1 + tgtb: ps_t2T is read ONCE (psum reads of one
            # bank serialize across engines); hi fp8 from x1s on DVE, f32 on
            # Pool (parallel), lo + Square(x1) after
            x1s = cst.tile([128, ND, SL], F32, tag="x1s")
            nc.vector.tensor_mul(x1s, ps_t2T, rb_bc)
            # variance cross term FIRST on DVE: the var chain gates FFN1
            # (via mustd/fix), while x1hi is only consumed once the fix lands
            x1cr = cst.tile([128, ND, SL], BF16, tag="x1cr")
            nc.vector.tensor_mul(x1cr.rearrange("p c s -> p (c s)"),
                                 x1s.rearrange("p c s -> p (c s)"),
                                 tgtbT.rearrange("p c s -> p (c s)"))
            x1sq = cst.tile([128, ND, SL], BF16, tag="x1sq")
            nc.scalar.activation(out=x1sq.rearrange("p c s -> p (c s)"),
                                 in_=x1s.rearrange("p c s -> p (c s)"),
                                 func=mybir.ActivationFunctionType.Square)
            x1hi = cst.tile([128, ND, SL], FP8, tag="x1hi")
            nc.vector.tensor_add(x1hi.rearrange("p c s -> p (c s)"),
                                 x1s.rearrange("p c s -> p (c s)"),
                                 tgtbT.rearrange("p c s -> p (c s)"))
            x1Tf = cst.tile([128, ND, SL], F32, tag="x1Tf")
            nc.gpsimd.tensor_add(x1Tf.rearrange("p c s -> p (c s)"),
                                 x1s.rearrange("p c s -> p (c s)"),
                                 tgtbT.rearrange("p c s -> p (c s)"))
            x1lo = cst.tile([128, ND, SL], FP8, tag="x1lo")
            nc.vector.tensor_sub(x1lo.rearrange("p c s -> p (c s)"),
                                 x1Tf.rearrange("p c s -> p (c s)"),
                                 x1hi.rearrange("p c s -> p (c s)"))

            # mu algebra on Pool: S1 is RAW (unnormalized ctx), so
            # mu = (S1*r^-1)/D + tsum/D (host pre-divides rows[0] by D)
            s1n_row = cst.tile([1, SL], F32, tag="s1n_row")
            nc.vector.tensor_mul(s1n_row, ps_stat[:, 0:SL], rrec)
            mu_row = cst.tile([1, SL], F32, tag="mu_row")
            nc.vector.scalar_tensor_tensor(out=mu_row, in0=s1n_row,
                                           scalar=1.0 / D, in1=rows[0:1, 0, :],
                                           op0=AOP.mult, op1=AOP.add)
            musqD = cst.tile([1, SL], F32, tag="musqD")
            nc.vector.scalar_tensor_tensor(out=musqD, in0=mu_row,
                                           scalar=-float(D), in1=mu_row,
                                           op0=AOP.mult, op1=AOP.mult)
            cmb_row = cst.tile([1, SL], F32, tag="cmb_row")
            nc.vector.tensor_add(cmb_row, musqD, rows[0:1, 1, :])
            # mustd fp8 row pair: [-mu ; std/8] (fix outer rhs)
            mustd = cst.tile([1, 2, SL], FP8, tag="mustd")
            nc.gpsimd.tensor_scalar_mul(mustd[0:1, 0, :], mu_row, -1.0)

            # variance chain: varD = sum x1^2 - D mu^2 (stat var matmuls gate)
            for dc in range(ND):
                nc.tensor.matmul(ps_stat[:, SL:2 * SL], lhsT=onesb,
                                 rhs=x1sq[:, dc, :],
                                 start=(dc == 0), stop=False,
                                 skip_group_check=True)
            for dc in range(ND):
                nc.tensor.matmul(ps_stat[:, SL:2 * SL], lhsT=twosb,
                                 rhs=x1cr[:, dc, :],
                                 start=False, stop=(dc == ND - 1),
                                 skip_group_check=True)
            varD_row = cst.tile([1, SL], F32, tag="varD_row")
            nc.vector.tensor_add(varD_row, ps_stat[:, SL:2 * SL], cmb_row)
            # std/8 = sqrt(varD/(64 D) + eps/64) straight into the fp8 pair
            nc.scalar.activation(out=mustd[0:1, 1, :], in_=varD_row,
                                 func=mybir.ActivationFunctionType.Sqrt,
                                 bias=epsc64, scale=1.0 / (64.0 * D))
            # off-chain: f32 std / rstd for the residual scaling
            std_row = cst.tile([1, SL], F32, tag="std_row")
            nc.scalar.activation(out=std_row, in_=varD_row,
                                 func=mybir.ActivationFunctionType.Sqrt,
                                 bias=epsc1, scale=1.0 / D)
            rstd_row = cst.tile([1, SL], F32, tag="rstd_row")
            nc.vector.reciprocal(out=rstd_row, in_=std_row)

            # ---- FFN1: h = W1q @ (x1hi + x1lo) + [w1sum;8b1] (x) [-mu;std/8]
            h_hi = cst.tile([128, NM, SL], FP8, tag="h_hi")
            h_lo = cst.tile([128, NM, SL], FP8, tag="h_lo")

            def ffn1_fc(fc):
                # fix FIRST (start=True): the in-order PE stream then stalls
                # on mustd only once, at the head, instead of between every
                # fc group's matmuls
                nc.tensor.matmul(ps_h[fc // 8][:, fc % 8, :],
                                 lhsT=wext[:, fc, :, :],
                                 rhs=mustd,
                                 start=True, stop=False,
                                 perf_mode=DR)
                k = 0
                for hv, x1q in ((0, x1hi), (1, x1lo)):
                    for dcp in range(0, ND, 2):
                        nc.tensor.matmul(
                            ps_h[fc // 8][:, fc % 8, :],
                            lhsT=w1T[:, dcp:dcp + 2, fc * 128:(fc + 1) * 128],
                            rhs=x1q[:, dcp:dcp + 2, :],
                            start=False, stop=(k == 3),
                            perf_mode=DR,
                        )
                        k += 1

            # h_hi = fp8 relu straight off psum (ACT) so FFN2-hi can start
            # immediately; h_lo = second psum read (DVE), overlapping the
            # hi DRs on PE
            def h_group8(g):
                sl8 = slice(8 * g, 8 * g + 8)
                nc.scalar.activation(
                    out=h_hi[:, sl8, :],
                    in_=ps_h[g],
                    func=mybir.ActivationFunctionType.Relu)
                nc.vector.scalar_tensor_tensor(
                    out=h_lo[:, sl8, :].rearrange("p c s -> p (c s)"),
                    in0=ps_h[g].rearrange("p c s -> p (c s)"),
                    scalar=0.0,
                    in1=h_hi[:, sl8, :].rearrange("p c s -> p (c s)"),
                    op0=AOP.max, op1=AOP.subtract)

            for fc in range(8):
                ffn1_fc(fc)
            h_group8(0)
            for fc in range(8, 16):
                ffn1_fc(fc)
            h_group8(1)

            # off-chain transposes fill the PE stall while w2 streams in
            ps_x1 = psp.tile([SL, D], F32, tag="scA", name="ps_x1")
            for dc in range(ND):
                nc.tensor.transpose(ps_x1[:, dc * 128:(dc + 1) * 128],
                                    x1Tf[:, dc, :], identf)
            ps_mr = psp.tile([SL, 2], F32, tag="sm", name="ps_mr")
            nc.tensor.transpose(ps_mr[:, 0:1], mu_row, identf1)
            nc.tensor.transpose(ps_mr[:, 1:2], rstd_row, identf1)

            # ---- FFN2: one accumulation group per dc, each in its OWN psum
            # bank so all four can be open at once; the hi-operand DRs for
            # dc0-2 run before h_lo is even ready, the lo DRs + closes follow.
            # dc3 is gated by the final w2 DMA and has the shortest tail.
            ps_o2dc = [
                psp.tile([128, SL], F32, tag="med", name="ps_o2d0"),
                psp.tile([128, SL], F32, tag="t2T", name="ps_o2d1"),
                psp.tile([128, SL], F32, tag="hA", name="ps_o2d2"),
                psp.tile([128, SL], F32, tag="hB", name="ps_o2d3"),
            ]
            ps_o2 = psp.tile([SL, D], BF16, tag="aux", name="ps_o2")
            ps_o2b = psp.tile([SL, D // 2], BF16, tag="sm", name="ps_o2b")
            mustd_s = bass.AP(tensor=mustd.tensor, offset=mustd[0:1, 1, :].offset,
                              ap=[list(mustd.ap[0]), [0, 2], [1, SL]])

            def ffn2_hi(dc):
                for fcp in range(0, NM, 2):
                    nc.tensor.matmul(
                        ps_o2dc[dc],
                        lhsT=w2T[:, dc, fcp:fcp + 2, :],
                        rhs=h_hi[:, fcp:fcp + 2, :],
                        start=(fcp == 0), stop=False,
                        perf_mode=DR,
                        skip_group_check=True,
                    )

            def ffn2_lo_close(dc):
                for fcp in range(0, NM, 2):
                    nc.tensor.matmul(
                        ps_o2dc[dc],
                        lhsT=w2T[:, dc, fcp:fcp + 2, :],
                        rhs=h_lo[:, fcp:fcp + 2, :],
                        start=False, stop=False,
                        perf_mode=DR,
                        skip_group_check=True,
                    )
                nc.tensor.matmul(ps_o2dc[dc],
                                 lhsT=wext[:, NM + dc, :, :],
                                 rhs=mustd_s,
                                 start=False, stop=True,
                                 perf_mode=DR,
                                 skip_group_check=True)

            # xhat = rstd * (x1 - mu) row-major f32 (early: overlaps FFN2)
            mr_col = cst.tile([SL, 2], F32, tag="mr_col")
            nc.vector.tensor_copy(out=mr_col, in_=ps_mr)
            xhat = cst.tile([SL, D], F32, tag="xhat")
            nc.vector.tensor_scalar(out=xhat, in0=ps_x1,
                                    scalar1=mr_col[:, 0:1],
                                    scalar2=mr_col[:, 1:2],
                                    op0=AOP.subtract, op1=AOP.mult)
            if apply_affine:
                nc.vector.tensor_mul(xhat, xhat, bvec["g2v"])
                nc.vector.tensor_add(xhat, xhat, bvec["be2v"])

            o2Ts = cst.tile([128, ND, SL], BF16, tag="o2Ts")
            x2 = cst.tile([SL, D], F32, tag="x2")
            SD = nc.vector.BN_STATS_DIM
            st2 = cst.tile([SL, 4 * SD], F32, tag="st2")

            def trans_dc(dc):
                tgt = ps_o2[:, dc * 128:(dc + 1) * 128] if dc < 2 else \
                    ps_o2b[:, (dc - 2) * 128:(dc - 1) * 128]
                nc.tensor.transpose(tgt, o2Ts[:, dc, :], identb)

            def x2_bn_dc(dc):
                src = ps_o2[:, dc * 128:(dc + 1) * 128] if dc < 2 else \
                    ps_o2b[:, (dc - 2) * 128:(dc - 1) * 128]
                cols = slice(dc * 128, (dc + 1) * 128)
                nc.vector.scalar_tensor_tensor(out=x2[:, cols], in0=src,
                                               scalar=mr_col[:, 1:2],
                                               in1=xhat[:, cols],
                                               op0=AOP.mult, op1=AOP.add)
                nc.vector.bn_stats(out=st2[:, dc * SD:(dc + 1) * SD],
                                   in_=x2[:, cols])

            for dc in range(3):
                ffn2_hi(dc)
            for dc in range(3):
                ffn2_lo_close(dc)
            ffn2_hi(3)
            for dc in range(3):
                nc.scalar.activation(out=o2Ts[:, dc, :], in_=ps_o2dc[dc],
                                     func=mybir.ActivationFunctionType.Copy)
            for dc in range(3):
                trans_dc(dc)
            for dc in range(3):
                x2_bn_dc(dc)
            ffn2_lo_close(3)
            nc.scalar.activation(out=o2Ts[:, 3, :], in_=ps_o2dc[3],
                                 func=mybir.ActivationFunctionType.Copy)
            trans_dc(3)
            x2_bn_dc(3)
            mv2 = cst.tile([SL, nc.vector.BN_AGGR_DIM], F32, tag="mv2")
            nc.vector.bn_aggr(out=mv2, in_=st2)
            std2 = cst.tile([SL, 1], F32, tag="std2")
            nc.scalar.activation(out=std2, in_=mv2[:, 1:2],
                                 func=mybir.ActivationFunctionType.Sqrt,
                                 bias=epsc, scale=1.0)
            rstd2 = cst.tile([SL, 1], F32, tag="rstd2")
            nc.vector.reciprocal(out=rstd2, in_=std2)
            out_sb = cst.tile([SL, D], F32, tag="out")
            nc.vector.tensor_scalar(out=out_sb, in0=x2,
                                    scalar1=mv2[:, 0:1], scalar2=rstd2,
                                    op0=AOP.subtract, op1=AOP.mult)
            if apply_affine:
                nc.vector.tensor_mul(out_sb, out_sb, bvec["g3v"])
                nc.vector.tensor_add(out_sb, out_sb, bvec["be3v"])
            nc.sync.dma_start(out=out_h[:], in_=out_sb)
            if _dbg:
                nc.sync.dma_start(out=io_dbg["d_mu"][:], in_=mu_row)
                nc.sync.dma_start(out=io_dbg["d_std"][:], in_=std_row)
                nc.sync.dma_start(out=io_dbg["d_rrec"][:], in_=rrec)
                dx1 = cst.tile([128, ND, SL], F32, tag="dx1")
                nc.vector.tensor_add(dx1.rearrange("p c s -> p (c s)"),
                                     x1hi.rearrange("p c s -> p (c s)"),
                                     x1lo.rearrange("p c s -> p (c s)"))
                nc.sync.dma_start(out=io_dbg["d_x1"][:], in_=dx1)
                dh = cst.tile([128, NM, SL], F32, tag="dh")
                nc.vector.tensor_add(dh.rearrange("p c s -> p (c s)"),
                                     h_hi.rearrange("p c s -> p (c s)"),
                                     h_lo.rearrange("p c s -> p (c s)"))
                nc.sync.dma_start(out=io_dbg["d_h"][:], in_=dh)
                do2 = cst.tile([128, ND, SL], F32, tag="do2")
                nc.vector.tensor_copy(out=do2.rearrange("p c s -> p (c s)"),
                                      in_=o2Ts.rearrange("p c s -> p (c s)"))
                nc.sync.dma_start(out=io_dbg["d_o2"][:], in_=do2)
                nc.sync.dma_start(out=io_dbg["d_xhat"][:], in_=xhat)
                nc.sync.dma_start(out=io_dbg["d_x2"][:], in_=x2)

    _split_multi_waits(nc)
    return nc


_NC_CACHE = {}


def _f8(x):
    return np.asarray(x, np.float32).astype(F8)


def _prep_inputs(tgt, memory, pos, query_pos, action_idx,
                 W_tgt2, b_tgt2, W1, b1, W2, b2, g2, be2, g3, be3):
    inv = np.float32(1.0 / np.sqrt(D))
    tgt2d = np.ascontiguousarray(tgt[:, 0, :], np.float32)        # [S, D]
    qp2d = np.ascontiguousarray(query_pos[:, 0, :], np.float32)
    mem2d = np.ascontiguousarray(memory[:, 0, :], np.float32)     # [T, D]
    pos2d = np.ascontiguousarray(pos[:, 0, :], np.float32)

    k2d = mem2d + pos2d
    k_p = np.zeros((T + 2 * HALO, D), np.float32)
    k_p[HALO:HALO + T] = k2d
    mem_p = np.zeros((T + 2 * HALO, D), np.float32)
    mem_p[HALO:HALO + T] = mem2d
    q2d = (tgt2d + qp2d) * inv                                    # [S, D]

    # segment ids from action_idx change points (mirrors the reference mask)
    ai = np.asarray(action_idx)
    change = np.concatenate([[0], (ai[1:] != ai[:-1]).astype(np.int64)])
    seg_id = np.cumsum(change)

    aff = _needs_affine(g2, be2, g3, be3)
    W1f = np.asarray(W1, np.float32)
    b1f = np.asarray(b1, np.float32)
    if aff:
        # fold g2/be2 into FFN1: h1 = (x^)@ (W1*g2).T + (b1 + W1@be2)
        W1eff = W1f * np.asarray(g2, np.float32)[None, :]
        b1eff = b1f + W1f @ np.asarray(be2, np.float32)
    else:
        W1eff, b1eff = W1f, b1f

    w1T_h = np.ascontiguousarray(
        W1eff.T.reshape(ND, 128, DFF).transpose(1, 0, 2)).astype(F8)
    # w2 dc-major: w2T[p, dc, fc, j] = W2[dc*128+j, fc*128+p]
    w2T_h = np.ascontiguousarray(
        np.asarray(W2, np.float32).T.reshape(NM, 128, ND, 128)
        .transpose(1, 2, 0, 3)).astype(F8)
    wtT_q = np.ascontiguousarray(
        np.asarray(W_tgt2, np.float32).T.reshape(ND, 128, D)
        .transpose(1, 0, 2)).astype(F8)
    # 513th column per d-chunk: Wt column sums (of the quantized weights)
    wtcol = np.asarray(wtT_q, np.float32).sum(axis=2)              # [128, ND]
    wtT_h = np.zeros((128, ND, D + 32), np.float32)
    wtT_h[:, :, 0:D] = np.asarray(wtT_q, np.float32)
    wtT_h[:, :, D] = wtcol
    wtT_h = np.ascontiguousarray(wtT_h.astype(F8))
    # wext pairs: fc rows ([w1sum_fc],[8*b1eff_fc]); dc rows ([8*b2_dc],[0])
    w1sum = np.asarray(w1T_h, np.float32).sum(axis=0).sum(axis=0)  # [DFF]
    b2f = np.asarray(b2, np.float32)
    wext_h = np.zeros((1, NM + ND, 2, 128), np.float32)
    wext_h[0, 0:NM, 0, :] = w1sum.reshape(NM, 128)
    wext_h[0, 0:NM, 1, :] = 8.0 * b1eff.reshape(NM, 128)
    wext_h[0, NM:NM + ND, 0, :] = 8.0 * b2f.reshape(ND, 128)
    wext_h = np.ascontiguousarray(wext_h).astype(F8)

    in_maps = []
    for c in range(NCORES):
        sl = slice(c * SL, (c + 1) * SL)
        qc = q2d[sl].T.reshape(ND, 128, SL).transpose(1, 0, 2)     # [128,ND,SL]
        q_hi = qc.astype(F8)
        q_lo = (qc - q_hi.astype(np.float32)).astype(F8)
        qT2c = np.ascontiguousarray(
            np.stack([np.asarray(q_hi), np.asarray(q_lo)], axis=1))
        kslab = k_p[c * TSH:c * TSH + SLAB]                       # [2176, D]
        kTc = kslab.T.reshape(ND, 128, SLAB).transpose(1, 0, 2).astype(F8)
        v_h = mem_p[c * TSH:c * TSH + SLAB].reshape(NTC, 128, D).astype(F8)

        # additive band mask in T layout [128, NTC, SL]: 0 where query j
        # (global s=64c+j) attends slab frame t, else -60; pad rows stay -60.
        mk = np.full((SL, SLAB), -60.0, np.float32)
        g0 = c * TSH - HALO
        glo, ghi = max(0, g0), min(T, g0 + SLAB)
        if ghi > glo:
            seg = seg_id[glo:ghi]
            svec = np.arange(c * SL, (c + 1) * SL)
            ok = (np.abs(seg[None, :] - svec[:, None]) <= 1)
            mk[:, glo - g0:ghi - g0][ok] = 0.0
        mkT = np.ascontiguousarray(
            mk.T.reshape(NTC, 128, SL).transpose(1, 0, 2)).astype(F8)

        tgtb = (tgt2d[sl] + np.asarray(b_tgt2, np.float32)).astype(BF)
        tgtbT = np.ascontiguousarray(
            tgtb.T.reshape(ND, 128, SL).transpose(1, 0, 2))
        tgtbf = tgtb.astype(np.float32)
        # rows[0] = tsum/D (pre-divided for the fused device-side mu stt)
        rows_h = np.stack([tgtbf.sum(axis=1) / D,
                           (tgtbf * tgtbf).sum(axis=1)]).reshape(1, 2, SL)
        rows_h = np.ascontiguousarray(rows_h, np.float32)

        im = {
            "qT2": qT2c,
            "kT": np.ascontiguousarray(kTc),
            "v_r": np.ascontiguousarray(v_h),
            "maskT": mkT,
            "w1T": w1T_h,
            "w2T": w2T_h,
            "wtT": wtT_h,
            "wext": wext_h,
            "rows": rows_h,
            "tgtbT": tgtbT,
        }
        if aff:
            im.update({
                "g2v": np.asarray(g2, np.float32),
                "be2v": np.asarray(be2, np.float32),
                "g3v": np.asarray(g3, np.float32),
                "be3v": np.asarray(be3, np.float32),
            })
        in_maps.append(im)
    return in_maps


def _needs_affine(g2, be2, g3, be3):
    return not (np.all(np.asarray(g2) == 1) and np.all(np.asarray(g3) == 1)
                and np.all(np.asarray(be2) == 0) and np.all(np.asarray(be3) == 0))


_LAST = {}


def kernel(**inputs) -> np.ndarray:
    inputs = {k: np.asarray(v) for k, v in inputs.items()}
    aff = _needs_affine(inputs["g2"], inputs["be2"], inputs["g3"], inputs["be3"])
    if aff not in _NC_CACHE:
        _NC_CACHE[aff] = _build_nc(apply_affine=aff)
    nc = _NC_CACHE[aff]
    in_maps = _prep_inputs(**inputs)
    import os
    kw = {}
    if os.environ.get("BASS_TRACE"):
        kw = dict(trace=True, tmpdir=os.environ.get("BASS_TRACE_DIR") or None)
    res = run_bass_kernel_spmd(nc, in_maps, core_ids=list(range(NCORES)), **kw)
    _LAST["res"] = res
    out = np.concatenate([res.results[c]["out"] for c in range(NCORES)], axis=0)
    return np.ascontiguousarray(out.reshape(S, 1, D).astype(np.float32))
